# revision 9
# baseline (speedup 1.0000x reference)
"""Trainium2 Bass kernel for nn_Decoder_86921548137026.

Dynamic decoder: NITER=2 iterations of (maxout pointer scoring over L=1024
positions -> argmax -> gather -> LSTM cell), followed by log_softmax over the
final start/end scores.

Sharding: data-parallel over batch B=32 across 8 cores (4 batches/core),
weights replicated.

v3 changes vs v2 (934us):
  - prologue restructured: critical-path DMA order (wd -> w1r -> E b0 -> w1m
    -> w2/w3), E b1-3 on the gpsimd queue, r/o emitted early; sweep-1 starts
    ~15us instead of ~94us
  - penalty fold moved from S4-post to a per-batch f32 subtract on Sb before
    the transpose (bit-exact, removes it from the boundary path)
  - per-batch argmax/gather pipelined into the sweep (batch b resolved two
    tiles after its strip completes); per-batch log_softmax rows likewise
  - sweeps 3/4 reuse stage-1: sweep-1/2 stage-1 PSUM (A = E@W1 + o_fold) is
    rounded to fp16 and spilled to DRAM; the iter-2 sweeps replace the big
    stage-1 matmul with a vector add of the per-batch delta-o broadcast and
    an SBUF fp16 maxpool (validated offline: 0 argmax flips, rel 3.6e-4)
"""

import numpy as np

H = 200
PMX = 8
B = 32
L = 1024
BIG = np.float32(1e30)
NCORES = 8
BLOC = B // NCORES          # 4 batches per core
NLT = L // 128              # 8 l-tiles per batch
# stage-1/2 output channels = H*P = 1600, swept in PSUM-bank-sized chunks
NCH = [(0, 512), (512, 512), (1024, 512), (1536, 64)]
# h-slice of m1/m2 produced by each n-chunk (1600 = 200h * 8p, h-major)
HSL = [(0, 64), (64, 64), (128, 64), (192, 8)]
REUSE = True               # sweeps 3/4 via DRAM A-spill + delta-o

_cache = {}


def _build_program():
    import contextlib
    import concourse.mybir as mybir
    import concourse.tile as tile
    from concourse import bacc
    from concourse.bass import ds
    from concourse.masks import make_identity

    f32 = mybir.dt.float32
    f16 = mybir.dt.float16
    u32 = mybir.dt.uint32
    AF = mybir.ActivationFunctionType
    OP = mybir.AluOpType
    AX = mybir.AxisListType

    nc = bacc.Bacc("TRN2", target_bir_lowering=False, debug=False,
                   enable_asserts=False, num_devices=NCORES)

    # ---------------- DRAM I/O ----------------
    enc = nc.dram_tensor("enc", [BLOC, 2 * H, L], f32, kind="ExternalInput").ap()
    penT = nc.dram_tensor("penT", [128, BLOC * NLT], f32, kind="ExternalInput").ap()
    selmat_d = nc.dram_tensor("selmat_d", [BLOC, BLOC * 128], f16, kind="ExternalInput").ap()
    wt = {}
    for tag in ("s", "e"):
        wt[f"w1_{tag}"] = nc.dram_tensor(f"w1_{tag}", [3 * H, H * PMX], f32, kind="ExternalInput").ap()
        wt[f"b1_{tag}"] = nc.dram_tensor(f"b1_{tag}", [1, H * PMX], f32, kind="ExternalInput").ap()
        wt[f"w2_{tag}"] = nc.dram_tensor(f"w2_{tag}", [H, H * PMX], f32, kind="ExternalInput").ap()
        wt[f"b2h_{tag}"] = nc.dram_tensor(f"b2h_{tag}", [1, H * PMX], f16, kind="ExternalInput").ap()
        wt[f"b2l_{tag}"] = nc.dram_tensor(f"b2l_{tag}", [1, H * PMX], f16, kind="ExternalInput").ap()
        wt[f"w3_{tag}"] = nc.dram_tensor(f"w3_{tag}", [2 * H, PMX], f32, kind="ExternalInput").ap()
        wt[f"b3h_{tag}"] = nc.dram_tensor(f"b3h_{tag}", [1, PMX], f16, kind="ExternalInput").ap()
        wt[f"b3l_{tag}"] = nc.dram_tensor(f"b3l_{tag}", [1, PMX], f16, kind="ExternalInput").ap()
        wt[f"wd_{tag}"] = nc.dram_tensor(f"wd_{tag}", [5 * H, H], f32, kind="ExternalInput").ap()
    wt["w_ih"] = nc.dram_tensor("w_ih", [4 * H, 4 * H], f32, kind="ExternalInput").ap()
    wt["b_lstm"] = nc.dram_tensor("b_lstm", [1, 4 * H], f32, kind="ExternalInput").ap()
    wt["w_mlp"] = nc.dram_tensor("w_mlp", [H, H], f32, kind="ExternalInput").ap()
    wt["b_mlp"] = nc.dram_tensor("b_mlp", [1, H], f32, kind="ExternalInput").ap()

    lp1 = nc.dram_tensor("lp1", [BLOC, L], f32, kind="ExternalOutput").ap()
    lp2 = nc.dram_tensor("lp2", [BLOC, L], f32, kind="ExternalOutput").ap()
    if REUSE:
        a1d = nc.dram_tensor("a1d", [BLOC, NLT, 128, H * PMX], f16, kind="Internal").ap()
        a2d = nc.dram_tensor("a2d", [BLOC, NLT, 128, H * PMX], f16, kind="Internal").ap()
        o_save_d = nc.dram_tensor("o_save_d", [2, BLOC, H * PMX], f32, kind="Internal").ap()

    with tile.TileContext(nc) as tc, contextlib.ExitStack() as ctx:
        const = ctx.enter_context(tc.tile_pool(name="const", bufs=1))
        wpool = ctx.enter_context(tc.tile_pool(name="wpool", bufs=1))
        epool = ctx.enter_context(tc.tile_pool(name="epool", bufs=1))
        work = ctx.enter_context(tc.tile_pool(name="work", bufs=3))
        single = ctx.enter_context(tc.tile_pool(name="single", bufs=1))
        keep = ctx.enter_context(tc.tile_pool(name="keep", bufs=1))
        ps_a = ctx.enter_context(tc.tile_pool(name="ps_a", bufs=3, space="PSUM"))
        ps_b = ctx.enter_context(tc.tile_pool(name="ps_b", bufs=2, space="PSUM"))
        ps_tr = ctx.enter_context(tc.tile_pool(name="ps_tr", bufs=2, space="PSUM"))
        ps_d = ctx.enter_context(tc.tile_pool(name="ps_d", bufs=1, space="PSUM"))
        stage_cm = tc.tile_pool(name="stage", bufs=2)
        stage = stage_cm.__enter__()

        dma_engines = [nc.sync, nc.scalar, nc.gpsimd]

        # ---------------- constants ----------------
        ident = const.tile([128, 128], f16, name="ident")
        make_identity(nc, ident)
        ident32 = const.tile([128, 128], f32, name="ident32")
        make_identity(nc, ident32)
        # row-selector stationaries: selmat[:, 128b:128(b+1)] broadcasts d16
        # row b to all 128 output partitions (host-built constant)
        selmat = const.tile([BLOC, BLOC * 128], f16, name="selmat")
        nc.sync.dma_start(selmat, selmat_d)

        penT_sb = const.tile([128, BLOC * NLT], f32, name="penT_sb")
        nc.sync.dma_start(penT_sb, penT)

        # ---------------- es/ee init from E column 0 ----------------
        es16 = keep.tile([128, 3, BLOC], f16, name="es16")
        es16c3 = keep.tile([17, BLOC], f16, name="es16c3")
        nc.vector.memset(es16c3, 1.0)              # row 16 = b_lstm fold row
        ee16 = keep.tile([128, 3, BLOC], f16, name="ee16")
        ee16c3 = keep.tile([16, BLOC], f16, name="ee16c3")
        escol = const.tile([128, 4, BLOC], f32, name="escol")
        for b in range(BLOC):
            for c in range(3):
                nc.sync.dma_start(escol[:, c, b:b + 1], enc[b, 128 * c:128 * (c + 1), 0:1])
            nc.sync.dma_start(escol[:16, 3, b:b + 1], enc[b, 384:400, 0:1])
        nc.vector.tensor_copy(es16, escol[:, 0:3, :])
        nc.vector.tensor_copy(es16c3[:16], escol[:16, 3, :])
        nc.vector.tensor_copy(ee16, es16)
        nc.vector.tensor_copy(ee16c3, es16c3[:16])

        # ---------------- weight / E loaders ----------------
        W = {}

        def cast(eng, dst, src):
            if eng is nc.scalar:
                nc.scalar.activation(dst, src, AF.Copy)
            else:
                eng.tensor_copy(dst, src)

        def load_wd(tag, dma, cse):
            # wd fp16 k-chunk tiles over cat(hx[0:200], es[200:600], ee[600:1000])
            wd_chunks = []
            for (k0, kn) in [(200, 128), (328, 128), (456, 128), (584, 16),
                             (600, 128), (728, 128), (856, 128), (984, 16),
                             (0, 128), (128, 72)]:
                wst = stage.tile([128, 1600], f32, tag="stg", bufs=2)
                dma.dma_start(wst[:kn, :H], wt[f"wd_{tag}"][k0:k0 + kn])
                t = wpool.tile([kn, H], f16, name=f"wd16_{tag}_{k0}")
                cast(cse, t, wst[:kn, :H])
                wd_chunks.append((k0, kn, t))
                yield
            W[f"wd_{tag}"] = wd_chunks

        def load_w1r(tag, dma, cse):
            # o-matmul rhs (fp16): rows 400..599 of w1, + b1 as ones-row 72 of c2
            wst = stage.tile([128, 1600], f32, tag="stg", bufs=2)
            dma.dma_start(wst, wt[f"w1_{tag}"][400:528])
            w1rc1 = wpool.tile([128, H * PMX], f16, name=f"w1r16c1_{tag}")
            cast(cse, w1rc1, wst)
            yield
            wst = stage.tile([128, 1600], f32, tag="stg", bufs=2)
            dma.dma_start(wst[:72], wt[f"w1_{tag}"][528:600])
            dma.dma_start(wst[72:73], wt[f"b1_{tag}"])
            w1rc2 = wpool.tile([73, H * PMX], f16, name=f"w1r16c2_{tag}")
            cast(cse, w1rc2, wst[:73])
            yield
            W[f"w1r_{tag}"] = (w1rc1, w1rc2)

        def load_w1m(tag, dma, csw):
            # stage-1 rhs rows 0..383 as fp16 [128, 3, 1600]
            w1m = wpool.tile([128, 3, H * PMX], f16, name=f"w1m_{tag}")
            for c in range(3):
                wst = stage.tile([128, 1600], f32, tag="stg", bufs=2)
                dma.dma_start(wst, wt[f"w1_{tag}"][128 * c:128 * (c + 1)])
                cast(csw, w1m[:, c, :], wst)
                yield
            W[f"w1m_{tag}"] = w1m
            # shared c3 rhs: rows 0..15 = W1[384:400] fp16, rows 16/17 = o hi/lo
            wst = stage.tile([128, 1600], f32, tag="stg", bufs=2)
            dma.dma_start(wst[:16], wt[f"w1_{tag}"][384:400])
            c3pair = []
            for pi in range(2):
                c3 = wpool.tile([18, H * PMX], f16, name=f"c3_{tag}_{pi}")
                cast(csw, c3[:16], wst[:16])
                c3pair.append(c3)
            yield
            W[f"c3_{tag}"] = c3pair

        def load_scoring2(tag, dma, csw, stg):
            # stage-2 rhs
            w2c1 = wpool.tile([128, H * PMX], f16, name=f"w2c1_{tag}")
            wst = stage.tile([128, 1600], f32, tag=stg, bufs=2)
            dma.dma_start(wst, wt[f"w2_{tag}"][:128])
            cast(csw, w2c1, wst)
            yield
            W[f"w2c1_{tag}"] = w2c1
            w2c2 = wpool.tile([74, H * PMX], f16, name=f"w2c2_{tag}")
            wst = stage.tile([128, 1600], f32, tag=stg, bufs=2)
            dma.dma_start(wst[:72], wt[f"w2_{tag}"][128:200])
            cast(csw, w2c2[:72], wst[:72])
            dma.dma_start(w2c2[72:73, :], wt[f"b2h_{tag}"])
            dma.dma_start(w2c2[73:74, :], wt[f"b2l_{tag}"])
            yield
            W[f"w2c2_{tag}"] = w2c2
            # stage-3 rhs chunks
            wst = stage.tile([128, 1600], f32, tag=stg, bufs=2)
            dma.dma_start(wst[:, 0:8], wt[f"w3_{tag}"][0:128])
            dma.dma_start(wst[:72, 8:16], wt[f"w3_{tag}"][128:200])
            dma.dma_start(wst[:, 16:24], wt[f"w3_{tag}"][200:328])
            dma.dma_start(wst[:72, 24:32], wt[f"w3_{tag}"][328:400])
            w3c1 = wpool.tile([128, PMX], f16, name=f"w3c1_{tag}")
            cast(csw, w3c1, wst[:, 0:8])
            w3c2 = wpool.tile([74, PMX], f16, name=f"w3c2_{tag}")
            cast(csw, w3c2[:72], wst[:72, 8:16])
            yield
            dma.dma_start(w3c2[72:73, :], wt[f"b3h_{tag}"])
            dma.dma_start(w3c2[73:74, :], wt[f"b3l_{tag}"])
            w3c3 = wpool.tile([128, PMX], f16, name=f"w3c3_{tag}")
            cast(csw, w3c3, wst[:, 16:24])
            w3c4 = wpool.tile([72, PMX], f16, name=f"w3c4_{tag}")
            cast(csw, w3c4, wst[:72, 24:32])
            yield
            W[f"w3_{tag}"] = (w3c1, w3c2, w3c3, w3c4)

        def load_lstm(dma, cse):
            # LSTM weights as fp16 rhs [kn, 800] over rows = cat(es, ee);
            # es-c3 chunk carries b_lstm as ones-row 16
            ih_chunks = []
            for (k0, kn) in [(0, 128), (128, 128), (256, 128), (384, 16),
                             (400, 128), (528, 128), (656, 128), (784, 16)]:
                wst = stage.tile([128, 1600], f32, tag="stg", bufs=2)
                dma.dma_start(wst[:kn, :800], wt["w_ih"][k0:k0 + kn])
                rows = kn + 1 if k0 == 384 else kn
                t = wpool.tile([rows, 4 * H], f16, name=f"wih16_{k0}")
                if k0 == 384:
                    dma.dma_start(wst[16:17, :800], wt["b_lstm"])
                cast(cse, t[:rows], wst[:rows, :800])
                ih_chunks.append((k0, kn, t))
                yield
            # mlp fp16 rhs [kn, 200]; c2 carries b_mlp as ones-row 72
            wst = stage.tile([128, 1600], f32, tag="stg", bufs=2)
            dma.dma_start(wst[:, :H], wt["w_mlp"][0:128])
            wmlpc1 = wpool.tile([128, H], f16, name="wmlp16c1")
            cast(cse, wmlpc1, wst[:, :H])
            yield
            wst = stage.tile([128, 1600], f32, tag="stg", bufs=2)
            dma.dma_start(wst[:72, :H], wt["w_mlp"][128:200])
            dma.dma_start(wst[72:73, :H], wt["b_mlp"])
            wmlpc2 = wpool.tile([73, H], f16, name="wmlp16c2")
            cast(cse, wmlpc2, wst[:73, :H])
            W["ih_chunks"] = ih_chunks
            W["wmlp"] = (wmlpc1, wmlpc2)
            yield

        # ---------------- E load ----------------
        E16 = []
        E16c3 = []

        def load_E(b, dma, defer=None):
            est = stage.tile([128, 3, L], f32, tag="estg", bufs=1)
            for c in range(3):
                dma.dma_start(est[:, c, :], enc[b, 128 * c:128 * (c + 1), :])
            est2 = stage.tile([16, L], f32, tag="estg2", bufs=1)
            dma.dma_start(est2, enc[b, 384:400, :])
            et = epool.tile([128, 3, L], f16, name=f"E16_{b}")
            ec3 = epool.tile([18, L], f16, name=f"E16c3_{b}")
            nc.vector.memset(ec3, 1.0)
            if defer is None:
                nc.vector.tensor_copy(et, est)
                nc.vector.tensor_copy(ec3[:16], est2)
            else:
                defer.append(lambda et=et, est=est:
                             nc.scalar.activation(et, est, AF.Copy))
                defer.append(lambda ec3=ec3, est2=est2:
                             nc.scalar.activation(ec3[:16], est2, AF.Copy))
            E16.append(et)
            E16c3.append(ec3)

        # ---------------- persistent state tiles ----------------
        hxT = [keep.tile([128, BLOC], f16, name="hxT16_0"),
               keep.tile([72, BLOC], f16, name="hxT16_1")]
        rT1 = keep.tile([128, BLOC], f16, name="rT1")
        rT2 = keep.tile([73, BLOC], f16, name="rT2")
        nc.gpsimd.memset(rT2, 1.0)                 # row 72 = b1 fold row
        h0T1 = keep.tile([128, BLOC], f16, name="h0T1")
        h0T2 = keep.tile([73, BLOC], f16, name="h0T2")
        nc.gpsimd.memset(h0T2, 1.0)                # row 72 = b_mlp fold row

        m1_slots, m2_slots, m1c2_slots = [], [], []
        for i in range(6):
            m1_slots.append(keep.tile([128, H], f16, name=f"m1_slot{i}"))
            m2_slots.append(keep.tile([128, H], f16, name=f"m2_slot{i}"))
            t = keep.tile([74, 128], f16, name=f"m1c2_slot{i}")
            nc.gpsimd.memset(t, 1.0)
            m1c2_slots.append(t)

        # ---------------- helpers ----------------
        def cat_chunks(tag, with_hx):
            """(lhsT [kn,4] fp16, wd16 [kn,H]) pairs for r = tanh(cat @ wd)."""
            ops = []
            for (k0, kn, wtile) in W[f"wd_{tag}"]:
                if k0 < 200:
                    if not with_hx:
                        continue
                    lhsT = hxT[0] if k0 == 0 else hxT[1]
                elif k0 < 600:
                    c = (k0 - 200) // 128
                    lhsT = es16[:, c, :] if c < 3 else es16c3[:16]
                else:
                    c = (k0 - 600) // 128
                    lhsT = ee16[:, c, :] if c < 3 else ee16c3
                ops.append((lhsT, wtile))
            return ops

        def r_matmul(tag, with_hx):
            """r_row = tanh(cat @ wd) -> [4, H] fp16 sbuf."""
            ops = cat_chunks(tag, with_hx)
            pt = ps_d.tile([128, 512], f32, tag="ps_ser")
            for i, (lhsT, rhs) in enumerate(ops):
                nc.tensor.matmul(pt[:BLOC, :H], lhsT, rhs,
                                 start=(i == 0), stop=(i == len(ops) - 1))
            r_row = work.tile([BLOC, H], f16, tag="r_row", bufs=1)
            nc.scalar.activation(r_row, pt[:BLOC, :H], AF.Tanh)
            return r_row

        def r_transpose(r_row):
            ptr = ps_tr.tile([128, 128], f16, tag="ps_tr")
            nc.tensor.transpose(ptr[:, :BLOC], r_row[:, 0:128], ident[:BLOC, :BLOC])
            nc.vector.tensor_copy(rT1, ptr[:, :BLOC])
            ptr2 = ps_tr.tile([128, 128], f16, tag="ps_tr")
            nc.tensor.transpose(ptr2[:72, :BLOC], r_row[:, 128:200], ident[:BLOC, :BLOC])
            nc.vector.tensor_copy(rT2[:72], ptr2[:72, :BLOC])

        def o_rows(tag, r_row, save=False):
            """o = r@w1r + b1 (f32 psum) -> fp16 hi/lo rows [BLOC, 1600]."""
            if save:
                o_tmp = work.tile([BLOC, H * PMX], f32, tag="o_tmp", bufs=1)
            r_transpose(r_row)
            w1rc1, w1rc2 = W[f"w1r_{tag}"]
            oh = single.tile([BLOC, H * PMX], f16, tag="oh")
            ol = single.tile([BLOC, H * PMX], f16, tag="ol")
            for (n0, nn) in NCH:
                pt = ps_d.tile([128, 512], f32, tag="ps_ser")
                nc.tensor.matmul(pt[:BLOC, :nn], rT1, w1rc1[:, n0:n0 + nn], start=True, stop=False)
                nc.tensor.matmul(pt[:BLOC, :nn], rT2, w1rc2[:, n0:n0 + nn], start=False, stop=True)
                nc.scalar.activation(oh[:, n0:n0 + nn], pt[:BLOC, :nn], AF.Copy)
                nc.vector.tensor_tensor(ol[:, n0:n0 + nn], pt[:BLOC, :nn], oh[:, n0:n0 + nn], OP.subtract)
                if save:
                    nc.vector.tensor_copy(o_tmp[:, n0:n0 + nn], pt[:BLOC, :nn])
            if save:
                nc.gpsimd.dma_start(o_save_d[0 if tag == "s" else 1], o_tmp)
            return oh, ol

        def delta_o_rows(tag, r_row):
            """d16 [4,1600] f16 = (r@w1r+b1) - o_save (sweep-1/2's o psum)."""
            o_tmp = work.tile([BLOC, H * PMX], f32, tag="o_tmp", bufs=1)
            nc.gpsimd.dma_start(o_tmp, o_save_d[0 if tag == "s" else 1])
            r_transpose(r_row)
            w1rc1, w1rc2 = W[f"w1r_{tag}"]
            d16 = work.tile([BLOC, H * PMX], f16, tag="d16", bufs=1)
            for (n0, nn) in NCH:
                pt = ps_d.tile([128, 512], f32, tag="ps_ser")
                nc.tensor.matmul(pt[:BLOC, :nn], rT1, w1rc1[:, n0:n0 + nn], start=True, stop=False)
                nc.tensor.matmul(pt[:BLOC, :nn], rT2, w1rc2[:, n0:n0 + nn], start=False, stop=True)
                nc.vector.tensor_tensor(d16[:, n0:n0 + nn], pt[:BLOC, :nn],
                                        o_tmp[:, n0:n0 + nn], OP.subtract)
            return d16

        def lsm_row(s4row, b, out_dram):
            """log_softmax of one S4 row -> out_dram[b]."""
            gmax = work.tile([1, 1], f32, tag="gmax", bufs=4)
            nc.vector.tensor_reduce(gmax, s4row, axis=AX.X, op=OP.max)
            negm = work.tile([1, 1], f32, tag="negm", bufs=4)
            nc.vector.tensor_scalar_mul(negm, gmax, -1.0)
            e4 = work.tile([1, L], f32, tag="rowtmp", bufs=1)
            sume = work.tile([1, 1], f32, tag="sume", bufs=4)
            nc.scalar.activation(e4, s4row, AF.Exp, bias=negm[:, 0:1], accum_out=sume)
            lnz = work.tile([1, 1], f32, tag="lnz", bufs=4)
            nc.scalar.activation(lnz, sume, AF.Ln)
            lse = work.tile([1, 1], f32, tag="lse", bufs=4)
            nc.vector.tensor_tensor(lse, gmax, lnz, OP.add)
            lp4 = work.tile([1, L], f32, tag="rowtmp", bufs=1)
            nc.vector.tensor_scalar(lp4, s4row, lse[:, 0:1], None, op0=OP.subtract)
            dma_engines[b % 3].dma_start(out_dram[b:b + 1, :], lp4)

        def argmax_gather_b(s4row, b, dstbig, dstc3):
            mx = work.tile([1, 8], f32, tag="mx", bufs=4)
            idx = work.tile([1, 8], u32, tag="idx", bufs=4)
            nc.vector.max(out=mx, in_=s4row)
            nc.vector.max_index(out=idx, in_max=mx, in_values=s4row)
            reg = nc.values_load(idx[0:1, 0:1], min_val=0, max_val=L - 1,
                                 skip_runtime_bounds_check=True)
            dma_engines[(2 * b) % 3].dma_start(
                dstbig[:, :, b:b + 1], E16[b][:, :, ds(reg, 1)])
            dma_engines[(2 * b + 1) % 3].dma_start(
                dstc3[:16, b:b + 1], E16c3[b][:16, ds(reg, 1)])

        # shared g2/g3 stages of the scoring pipeline
        def make_g23(tag, s4rows, st, strips):
            w2c1 = W[f"w2c1_{tag}"]
            w2c2 = W[f"w2c2_{tag}"]
            w3c1, w3c2, w3c3, w3c4 = W[f"w3_{tag}"]

            def g2(i):
                m1 = st[i]["m1"]
                pt1 = ps_tr.tile([128, 128], f16, tag="ps_tr")
                nc.tensor.transpose(pt1, m1[:, 0:128], ident)
                m1c1 = work.tile([128, 128], f16, tag="m1c1", bufs=4)
                nc.scalar.activation(m1c1, pt1, AF.Copy)
                pt2 = ps_tr.tile([128, 128], f16, tag="ps_tr")
                nc.tensor.transpose(pt2[:72], m1[:, 128:200], ident)
                m1c2 = m1c2_slots[i % 6]
                nc.scalar.activation(m1c2[:72], pt2[:72], AF.Copy)
                m2 = m2_slots[i % 6]
                for ni, (n0, nn) in enumerate(NCH):
                    pb = ps_b.tile([128, 512], f32, tag="ps_s2")
                    nc.tensor.matmul(pb[:, :nn], m1c1, w2c1[:, n0:n0 + nn], start=True, stop=False)
                    nc.tensor.matmul(pb[:, :nn], m1c2, w2c2[:, n0:n0 + nn], start=False, stop=True)
                    h0, hn = HSL[ni]
                    nc.vector.tensor_reduce(
                        m2[:, h0:h0 + hn],
                        pb[:, :nn].rearrange("p (h q) -> p h q", q=PMX),
                        axis=AX.X, op=OP.max)
                st[i]["m1c1"] = m1c1
                st[i]["m1c2"] = m1c2
                st[i]["m2"] = m2

            def g3(i):
                b, lt = divmod(i, NLT)
                m2 = st[i]["m2"]
                pt3 = ps_tr.tile([128, 128], f16, tag="ps_tr")
                nc.tensor.transpose(pt3, m2[:, 0:128], ident)
                m2c1 = work.tile([128, 128], f16, tag="m2c1", bufs=4)
                nc.scalar.activation(m2c1, pt3, AF.Copy)
                pt4 = ps_tr.tile([128, 128], f16, tag="ps_tr")
                nc.tensor.transpose(pt4[:72], m2[:, 128:200], ident)
                m2c2 = work.tile([72, 128], f16, tag="m2c2", bufs=4)
                nc.scalar.activation(m2c2, pt4[:72], AF.Copy)
                if lt == 0:
                    strips[b] = ps_d.tile([128, 8 * NLT], f32, tag="ps_ser", name="s3strip")
                psl = strips[b][:, 8 * lt:8 * (lt + 1)]
                nc.tensor.matmul(psl, st[i]["m1c1"], w3c1, start=True, stop=False)
                nc.tensor.matmul(psl, st[i]["m1c2"], w3c2, start=False, stop=False)
                nc.tensor.matmul(psl, m2c1, w3c3, start=False, stop=False)
                nc.tensor.matmul(psl, m2c2, w3c4, start=False, stop=True)
                st[i].clear()
                if lt == NLT - 1:
                    Sb = work.tile([128, NLT], f32, tag="Sb")
                    nc.vector.tensor_reduce(Sb,
                                            strips[b].rearrange("p (t q) -> p t q", q=PMX),
                                            axis=AX.X, op=OP.max)
                    nc.vector.tensor_tensor(Sb, Sb, penT_sb[:, b * NLT:(b + 1) * NLT],
                                            OP.subtract)
                    ptb = ps_tr.tile([NLT, 128], f32, tag="ps_tr")
                    nc.tensor.transpose(ptb, Sb, ident32)
                    s4stg = work.tile([NLT, 128], f32, tag="s4stg")
                    nc.scalar.activation(s4stg, ptb, AF.Copy)
                    s4row = work.tile([1, L], f32, tag="s4row", bufs=2)
                    dma_engines[b % 3].dma_start(s4row, s4stg)
                    s4rows[b] = s4row
            return g2, g3

        def run_pipeline(NT, g1, g2, g3, fill, batch_cb):
            # batch b's callback fires two tiles after its strip completes
            # (so the S4-row DMA has landed); last batch fires immediately.
            cb_at = {b * NLT + NLT + 1: b for b in range(BLOC - 1)}
            cb_at[NT - 1] = BLOC - 1
            for i in range(NT + 2):
                if i < NT:
                    g1(i)
                if 1 <= i < NT + 1:
                    g2(i - 1)
                if 2 <= i:
                    j = i - 2
                    g3(j)
                    if batch_cb is not None and j in cb_at:
                        batch_cb(cb_at[j])
                if fill is not None:
                    next(fill, None)
            if fill is not None:
                for _ in fill:
                    pass

        def score_sweep(tag, s4rows, oh, ol, fill=None, batch_cb=None, a_out=None):
            """Full maxout scoring sweep; optionally spills stage-1 psum (fp16)."""
            w1m = W[f"w1m_{tag}"]
            c3pair = W[f"c3_{tag}"]
            NT = BLOC * NLT
            st = [dict() for _ in range(NT)]
            strips = {}
            g2, g3 = make_g23(tag, s4rows, st, strips)

            def g1(i):
                b, lt = divmod(i, NLT)
                c3rhs = c3pair[b % 2]
                if lt == 0:
                    nc.sync.dma_start(c3rhs[16:17, :], oh[b:b + 1, :])
                    nc.sync.dma_start(c3rhs[17:18, :], ol[b:b + 1, :])
                lsl = slice(128 * lt, 128 * (lt + 1))
                m1 = m1_slots[i % 6]
                if a_out is not None:
                    a_w = stage.tile([128, H * PMX], f16, tag="aw", bufs=1)
                for ni, (n0, nn) in enumerate(NCH):
                    pa = ps_a.tile([128, 512], f32, tag="ps_s1")
                    for c in range(3):
                        nc.tensor.matmul(pa[:, :nn], E16[b][:, c, lsl], w1m[:, c, n0:n0 + nn],
                                         start=(c == 0), stop=False)
                    nc.tensor.matmul(pa[:, :nn], E16c3[b][:, lsl], c3rhs[:, n0:n0 + nn],
                                     start=False, stop=True)
                    h0, hn = HSL[ni]
                    nc.vector.tensor_reduce(
                        m1[:, h0:h0 + hn],
                        pa[:, :nn].rearrange("p (h q) -> p h q", q=PMX),
                        axis=AX.X, op=OP.max)
                    if a_out is not None:
                        nc.scalar.activation(a_w[:, n0:n0 + nn], pa[:, :nn], AF.Copy)
                if a_out is not None:
                    dma_engines[(b + lt) % 3].dma_start(a_out[b, lt], a_w)
                st[i]["m1"] = m1

            run_pipeline(NT, g1, g2, g3, fill, batch_cb)

        def reuse_sweep(tag, s4rows, d16, a_in, batch_cb=None):
            """Scoring sweep: stage-1 = identity-matmul of the DRAM A-tile
            plus a row-selected delta-o fold, accumulated in PSUM."""
            NT = BLOC * NLT
            st = [dict() for _ in range(NT)]
            strips = {}
            a_tiles = {}
            g2, g3 = make_g23(tag, s4rows, st, strips)

            def fetch(j):
                if j >= NT:
                    return
                b, lt = divmod(j, NLT)
                at = stage.tile([128, H * PMX], f16, tag="ar", bufs=2)
                dma_engines[j % 3].dma_start(at, a_in[b, lt])
                a_tiles[j] = at

            fetch(0)
            fetch(1)

            def g1(i):
                b, lt = divmod(i, NLT)
                fetch(i + 2)
                at = a_tiles.pop(i)
                m1 = m1_slots[i % 6]
                for ni, (n0, nn) in enumerate(NCH):
                    pa = ps_a.tile([128, 512], f32, tag="ps_s1")
                    nc.tensor.matmul(pa[:, :nn], ident, at[:, n0:n0 + nn],
                                     start=True, stop=False)
                    nc.tensor.matmul(pa[:, :nn], selmat[:, 128 * b:128 * (b + 1)],
                                     d16[:, n0:n0 + nn], start=False, stop=True)
                    h0, hn = HSL[ni]
                    nc.vector.tensor_reduce(
                        m1[:, h0:h0 + hn],
                        pa[:, :nn].rearrange("p (h q) -> p h q", q=PMX),
                        axis=AX.X, op=OP.max)
                st[i]["m1"] = m1

            run_pipeline(NT, g1, g2, g3, None, batch_cb)

        def lstm_update():
            """hx via LSTM cell with hx0=cx0=0 (f-gate and w_hh drop out)."""
            pt_i = ps_d.tile([128, 512], f32, tag="ps_ser")
            pt_go = ps_d.tile([128, 512], f32, tag="ps_ser")
            lhs_for = []
            for (k0, kn, wtile) in W["ih_chunks"]:
                if k0 < 400:
                    c = k0 // 128
                    lhsT = es16[:, c, :] if c < 3 else es16c3  # [17,4] w/ ones
                else:
                    c = (k0 - 400) // 128
                    lhsT = ee16[:, c, :] if c < 3 else ee16c3
                lhs_for.append((lhsT, wtile, kn + (1 if k0 == 384 else 0)))
            n = len(lhs_for)
            for i, (lhsT, wtile, rows) in enumerate(lhs_for):
                nc.tensor.matmul(pt_i[:BLOC, :H], lhsT, wtile[:rows, 0:H],
                                 start=(i == 0), stop=(i == n - 1))
            for i, (lhsT, wtile, rows) in enumerate(lhs_for):
                nc.tensor.matmul(pt_go[:BLOC, :2 * H], lhsT, wtile[:rows, 2 * H:4 * H],
                                 start=(i == 0), stop=(i == n - 1))
            ig = work.tile([BLOC, H], f32, tag="ig", bufs=1)
            nc.scalar.activation(ig, pt_i[:BLOC, :H], AF.Sigmoid)
            gg = work.tile([BLOC, H], f32, tag="gg", bufs=1)
            nc.scalar.activation(gg, pt_go[:BLOC, 0:H], AF.Tanh)
            og = work.tile([BLOC, H], f32, tag="og", bufs=1)
            nc.scalar.activation(og, pt_go[:BLOC, H:2 * H], AF.Sigmoid)
            cx = work.tile([BLOC, H], f32, tag="cx", bufs=1)
            nc.vector.tensor_tensor(cx, ig, gg, OP.mult)
            tcx = work.tile([BLOC, H], f32, tag="tcx", bufs=1)
            nc.scalar.activation(tcx, cx, AF.Tanh)
            h0 = work.tile([BLOC, H], f16, tag="h0", bufs=1)
            nc.vector.tensor_tensor(h0, og, tcx, OP.mult)
            ptr = ps_tr.tile([128, 128], f16, tag="ps_tr")
            nc.tensor.transpose(ptr[:, :BLOC], h0[:, 0:128], ident[:BLOC, :BLOC])
            nc.vector.tensor_copy(h0T1, ptr[:, :BLOC])
            ptr2 = ps_tr.tile([128, 128], f16, tag="ps_tr")
            nc.tensor.transpose(ptr2[:72, :BLOC], h0[:, 128:200], ident[:BLOC, :BLOC])
            nc.vector.tensor_copy(h0T2[:72], ptr2[:72, :BLOC])
            pt = ps_d.tile([128, 512], f32, tag="ps_ser")
            wmlpc1, wmlpc2 = W["wmlp"]
            nc.tensor.matmul(pt[:BLOC, :H], h0T1, wmlpc1, start=True, stop=False)
            nc.tensor.matmul(pt[:BLOC, :H], h0T2, wmlpc2, start=False, stop=True)
            hx_row = work.tile([BLOC, H], f16, tag="hx_row", bufs=1)
            nc.scalar.activation(hx_row, pt[:BLOC, :H], AF.Copy)
            ptr3 = ps_tr.tile([128, 128], f16, tag="ps_tr")
            nc.tensor.transpose(ptr3[:, :BLOC], hx_row[:, 0:128], ident[:BLOC, :BLOC])
            nc.vector.tensor_copy(hxT[0], ptr3[:, :BLOC])
            ptr4 = ps_tr.tile([128, 128], f16, tag="ps_tr")
            nc.tensor.transpose(ptr4[:72, :BLOC], hx_row[:, 128:200], ident[:BLOC, :BLOC])
            nc.vector.tensor_copy(hxT[1], ptr4[:72, :BLOC])

        # ---------------- prologue: s-critical-path loads ----------------
        import itertools
        for _ in load_wd("s", nc.sync, nc.vector):
            pass
        for _ in load_w1r("s", nc.sync, nc.vector):
            pass
        load_E(0, nc.sync)
        for _ in load_w1m("s", nc.sync, nc.scalar):
            pass
        for _ in load_scoring2("s", nc.sync, nc.scalar, "stg"):
            pass
        edefer = []
        load_E(1, nc.gpsimd, edefer)
        load_E(2, nc.gpsimd, edefer)
        load_E(3, nc.gpsimd, edefer)

        def edefer_gen():
            for fn in edefer:
                fn()
                yield
        fill_steps = itertools.chain(
            edefer_gen(),
            load_wd("e", nc.sync, nc.scalar),
            load_w1r("e", nc.sync, nc.scalar),
            load_w1m("e", nc.sync, nc.scalar),
            load_scoring2("e", nc.sync, nc.scalar, "wstg_e"),
            load_lstm(nc.sync, nc.scalar))

        # ---------------- the four passes ----------------
        r_row = r_matmul("s", with_hx=False)
        oh, ol = o_rows("s", r_row, save=REUSE)
        rows1 = {}
        score_sweep("s", rows1, oh, ol, fill=fill_steps,
                    batch_cb=lambda b: argmax_gather_b(rows1[b], b, es16, es16c3),
                    a_out=a1d if REUSE else None)

        r_row = r_matmul("e", with_hx=False)
        oh, ol = o_rows("e", r_row, save=REUSE)
        rows2 = {}
        score_sweep("e", rows2, oh, ol,
                    batch_cb=lambda b: argmax_gather_b(rows2[b], b, ee16, ee16c3),
                    a_out=a2d if REUSE else None)

        lstm_update()

        rows3 = {}
        rows4 = {}

        def cb3(b):
            argmax_gather_b(rows3[b], b, es16, es16c3)
            if b < BLOC - 1:
                lsm_row(rows3[b], b, lp1)

        def cb4(b):
            if b < BLOC - 1:
                lsm_row(rows4[b], b, lp2)

        r_row = r_matmul("s", with_hx=True)
        if REUSE:
            d16 = delta_o_rows("s", r_row)
            reuse_sweep("s", rows3, d16, a1d, batch_cb=cb3)
            r_row = r_matmul("e", with_hx=True)
            d16 = delta_o_rows("e", r_row)
            lsm_row(rows3[BLOC - 1], BLOC - 1, lp1)
            reuse_sweep("e", rows4, d16, a2d, batch_cb=cb4)
        else:
            oh, ol = o_rows("s", r_row)
            score_sweep("s", rows3, oh, ol, batch_cb=cb3)
            r_row = r_matmul("e", with_hx=True)
            oh, ol = o_rows("e", r_row)
            lsm_row(rows3[BLOC - 1], BLOC - 1, lp1)
            score_sweep("e", rows4, oh, ol, batch_cb=cb4)
        lsm_row(rows4[BLOC - 1], BLOC - 1, lp2)

        stage_cm.__exit__(None, None, None)

    nc.compile()
    return nc


def get_program():
    if "nc" not in _cache:
        _cache["nc"] = _build_program()
    return _cache["nc"]


def _split16(x):
    hi = np.asarray(x, np.float32).astype(np.float16)
    lo = (np.asarray(x, np.float32) - hi.astype(np.float32)).astype(np.float16)
    return hi, lo


def make_in_maps(inputs):
    """Per-core input maps: batch shard + trivial host prep (mask, bias splits)."""
    inputs = {k: np.asarray(v) for k, v in inputs.items()}
    enc = np.ascontiguousarray(inputs["encoding_matrix"], dtype=np.float32)
    lens = np.asarray(inputs["passage_lens"]).astype(np.int64)
    pen_full = np.where(np.arange(L)[None, :] < lens[:, None],
                        np.float32(0.0), BIG).astype(np.float32)

    shared = {}
    for tag in ("s", "e"):
        shared[f"w1_{tag}"] = np.ascontiguousarray(inputs[f"w1_{tag}"], np.float32)
        shared[f"b1_{tag}"] = np.ascontiguousarray(inputs[f"b1_{tag}"], np.float32).reshape(1, -1)
        shared[f"w2_{tag}"] = np.ascontiguousarray(inputs[f"w2_{tag}"], np.float32)
        b2h, b2l = _split16(inputs[f"b2_{tag}"])
        shared[f"b2h_{tag}"] = b2h.reshape(1, -1)
        shared[f"b2l_{tag}"] = b2l.reshape(1, -1)
        shared[f"w3_{tag}"] = np.ascontiguousarray(inputs[f"w3_{tag}"], np.float32)
        b3h, b3l = _split16(inputs[f"b3_{tag}"])
        shared[f"b3h_{tag}"] = b3h.reshape(1, -1)
        shared[f"b3l_{tag}"] = b3l.reshape(1, -1)
        shared[f"wd_{tag}"] = np.ascontiguousarray(inputs[f"wd_{tag}"], np.float32)
    shared["w_ih"] = np.ascontiguousarray(inputs["w_ih"], np.float32)
    shared["b_lstm"] = np.ascontiguousarray(inputs["b_lstm"], np.float32).reshape(1, -1)
    shared["w_mlp"] = np.ascontiguousarray(inputs["w_mlp"], np.float32)
    shared["b_mlp"] = np.ascontiguousarray(inputs["b_mlp"], np.float32).reshape(1, -1)

    in_maps = []
    for core in range(NCORES):
        sl = slice(core * BLOC, (core + 1) * BLOC)
        m = dict(shared)
        m["enc"] = np.ascontiguousarray(enc[sl])
        m["selmat_d"] = np.kron(np.eye(BLOC, dtype=np.float16),
                                np.ones((1, 128), dtype=np.float16))
        pc = pen_full[sl].reshape(BLOC, NLT, 128)
        m["penT"] = np.ascontiguousarray(pc.transpose(2, 0, 1).reshape(128, BLOC * NLT))
        in_maps.append(m)
    return in_maps


def run_on_hw(inputs, trace=False):
    from concourse import bass_utils
    nc = get_program()
    in_maps = make_in_maps(inputs)
    res = bass_utils.run_bass_kernel_spmd(nc, in_maps, core_ids=list(range(NCORES)),
                                          trace=trace)
    lp1 = np.concatenate([res.results[c]["lp1"] for c in range(NCORES)], axis=0)
    lp2 = np.concatenate([res.results[c]["lp2"] for c in range(NCORES)], axis=0)
    return (np.asarray(lp1, np.float32), np.asarray(lp2, np.float32)), res


def kernel(**inputs):
    out, _ = run_on_hw(inputs, trace=False)
    return out


# revision 16
# speedup vs baseline: 1.1046x; 1.1046x over previous
"""Trainium2 Bass kernel for nn_Decoder_86921548137026.

Dynamic decoder: NITER=2 iterations of (maxout pointer scoring over L=1024
positions -> argmax -> gather -> LSTM cell), followed by log_softmax over the
final start/end scores.

Sharding: data-parallel over batch B=32 across 8 cores (4 batches/core),
weights replicated.

v3 changes vs v2 (934us):
  - prologue restructured: critical-path DMA order (wd -> w1r -> E b0 -> w1m
    -> w2/w3), E b1-3 on the gpsimd queue, r/o emitted early; sweep-1 starts
    ~15us instead of ~94us
  - penalty fold moved from S4-post to a per-batch f32 subtract on Sb before
    the transpose (bit-exact, removes it from the boundary path)
  - per-batch argmax/gather pipelined into the sweep (batch b resolved two
    tiles after its strip completes); per-batch log_softmax rows likewise
  - sweeps 3/4 reuse stage-1: sweep-1/2 stage-1 PSUM (A = E@W1 + o_fold) is
    rounded to fp16 and spilled to DRAM; the iter-2 sweeps replace the big
    stage-1 matmul with a vector add of the per-batch delta-o broadcast and
    an SBUF fp16 maxpool (validated offline: 0 argmax flips, rel 3.6e-4)
"""

import numpy as np

H = 200
PMX = 8
B = 32
L = 1024
BIG = np.float32(1e30)
NCORES = 8
BLOC = B // NCORES          # 4 batches per core
NLT = L // 128              # 8 l-tiles per batch
# stage-1/2 output channels = H*P = 1600, swept in PSUM-bank-sized chunks
NCH = [(0, 512), (512, 512), (1024, 512), (1536, 64)]
# h-slice of m1/m2 produced by each n-chunk (1600 = 200h * 8p, h-major)
HSL = [(0, 64), (64, 64), (128, 64), (192, 8)]
REUSE = True               # sweeps 3/4 via DRAM A-spill + delta-o

_cache = {}


def _build_program():
    import contextlib
    import concourse.mybir as mybir
    import concourse.tile as tile
    from concourse import bacc
    from concourse.bass import ds
    from concourse.masks import make_identity

    f32 = mybir.dt.float32
    f16 = mybir.dt.float16
    u32 = mybir.dt.uint32
    AF = mybir.ActivationFunctionType
    OP = mybir.AluOpType
    AX = mybir.AxisListType

    nc = bacc.Bacc("TRN2", target_bir_lowering=False, debug=False,
                   enable_asserts=False, num_devices=NCORES)

    # ---------------- DRAM I/O ----------------
    enc = nc.dram_tensor("enc", [BLOC, 2 * H, L], f32, kind="ExternalInput").ap()
    penT = nc.dram_tensor("penT", [128, BLOC * NLT], f32, kind="ExternalInput").ap()
    selmat_d = nc.dram_tensor("selmat_d", [BLOC, BLOC * 128], f16, kind="ExternalInput").ap()
    wt = {}
    for tag in ("s", "e"):
        wt[f"w1_{tag}"] = nc.dram_tensor(f"w1_{tag}", [3 * H, H * PMX], f32, kind="ExternalInput").ap()
        wt[f"b1_{tag}"] = nc.dram_tensor(f"b1_{tag}", [1, H * PMX], f32, kind="ExternalInput").ap()
        wt[f"w2_{tag}"] = nc.dram_tensor(f"w2_{tag}", [H, H * PMX], f32, kind="ExternalInput").ap()
        wt[f"b2h_{tag}"] = nc.dram_tensor(f"b2h_{tag}", [1, H * PMX], f16, kind="ExternalInput").ap()
        wt[f"b2l_{tag}"] = nc.dram_tensor(f"b2l_{tag}", [1, H * PMX], f16, kind="ExternalInput").ap()
        wt[f"w3_{tag}"] = nc.dram_tensor(f"w3_{tag}", [2 * H, PMX], f32, kind="ExternalInput").ap()
        wt[f"b3h_{tag}"] = nc.dram_tensor(f"b3h_{tag}", [1, PMX], f16, kind="ExternalInput").ap()
        wt[f"b3l_{tag}"] = nc.dram_tensor(f"b3l_{tag}", [1, PMX], f16, kind="ExternalInput").ap()
        wt[f"wd_{tag}"] = nc.dram_tensor(f"wd_{tag}", [5 * H, H], f32, kind="ExternalInput").ap()
    wt["w_ih"] = nc.dram_tensor("w_ih", [4 * H, 4 * H], f32, kind="ExternalInput").ap()
    wt["b_lstm"] = nc.dram_tensor("b_lstm", [1, 4 * H], f32, kind="ExternalInput").ap()
    wt["w_mlp"] = nc.dram_tensor("w_mlp", [H, H], f32, kind="ExternalInput").ap()
    wt["b_mlp"] = nc.dram_tensor("b_mlp", [1, H], f32, kind="ExternalInput").ap()

    lp1 = nc.dram_tensor("lp1", [BLOC, L], f32, kind="ExternalOutput").ap()
    lp2 = nc.dram_tensor("lp2", [BLOC, L], f32, kind="ExternalOutput").ap()
    if REUSE:
        a1d = nc.dram_tensor("a1d", [BLOC, NLT, 128, H * PMX], f16, kind="Internal").ap()
        a2d = nc.dram_tensor("a2d", [BLOC, NLT, 128, H * PMX], f16, kind="Internal").ap()
        o_save_d = nc.dram_tensor("o_save_d", [2, BLOC, H * PMX], f32, kind="Internal").ap()

    with tile.TileContext(nc) as tc, contextlib.ExitStack() as ctx:
        const = ctx.enter_context(tc.tile_pool(name="const", bufs=1))
        wpool = ctx.enter_context(tc.tile_pool(name="wpool", bufs=1))
        epool = ctx.enter_context(tc.tile_pool(name="epool", bufs=1))
        work = ctx.enter_context(tc.tile_pool(name="work", bufs=3))
        single = ctx.enter_context(tc.tile_pool(name="single", bufs=1))
        keep = ctx.enter_context(tc.tile_pool(name="keep", bufs=1))
        ps_a = ctx.enter_context(tc.tile_pool(name="ps_a", bufs=3, space="PSUM"))
        ps_b = ctx.enter_context(tc.tile_pool(name="ps_b", bufs=2, space="PSUM"))
        ps_tr = ctx.enter_context(tc.tile_pool(name="ps_tr", bufs=2, space="PSUM"))
        ps_d = ctx.enter_context(tc.tile_pool(name="ps_d", bufs=1, space="PSUM"))
        stage_cm = tc.tile_pool(name="stage", bufs=2)
        stage = stage_cm.__enter__()

        dma_engines = [nc.sync, nc.scalar, nc.gpsimd]

        # ---------------- constants ----------------
        ident = const.tile([128, 128], f16, name="ident")
        make_identity(nc, ident)
        ident32 = const.tile([128, 128], f32, name="ident32")
        make_identity(nc, ident32)
        # row-selector stationaries: selmat[:, 128b:128(b+1)] broadcasts d16
        # row b to all 128 output partitions (host-built constant)
        selmat = const.tile([BLOC, BLOC * 128], f16, name="selmat")
        nc.sync.dma_start(selmat, selmat_d)

        penT_sb = const.tile([128, BLOC * NLT], f32, name="penT_sb")
        nc.sync.dma_start(penT_sb, penT)

        # ---------------- es/ee init from E column 0 ----------------
        es16 = keep.tile([128, 3, BLOC], f16, name="es16")
        es16c3 = keep.tile([17, BLOC], f16, name="es16c3")
        nc.gpsimd.memset(es16c3, 1.0)              # row 16 = b_lstm fold row
        ee16 = keep.tile([128, 3, BLOC], f16, name="ee16")
        ee16c3 = keep.tile([16, BLOC], f16, name="ee16c3")
        escol = const.tile([128, 4, BLOC], f32, name="escol")
        for c in range(3):
            nc.sync.dma_start(escol[:, c, :],
                              enc[0:BLOC, 128 * c:128 * (c + 1), 0:1].rearrange("b p x -> p (b x)"))
        nc.sync.dma_start(escol[:16, 3, :],
                          enc[0:BLOC, 384:400, 0:1].rearrange("b p x -> p (b x)"))
        nc.vector.tensor_copy(es16, escol[:, 0:3, :])
        nc.vector.tensor_copy(es16c3[:16], escol[:16, 3, :])
        nc.vector.tensor_copy(ee16, es16)
        nc.vector.tensor_copy(ee16c3, es16c3[:16])

        # ---------------- weight / E loaders ----------------
        W = {}

        def cast(eng, dst, src):
            if eng is nc.scalar:
                nc.scalar.activation(dst, src, AF.Copy)
            else:
                eng.tensor_copy(dst, src)

        def load_wd(tag, dma, cse):
            # wd fp16 k-chunk tiles over cat(hx[0:200], es[200:600], ee[600:1000])
            wd_chunks = []
            for (k0, kn) in [(200, 128), (328, 128), (456, 128), (584, 16),
                             (600, 128), (728, 128), (856, 128), (984, 16),
                             (0, 128), (128, 72)]:
                wst = stage.tile([128, 800], f32, tag="stg8", bufs=2)
                dma.dma_start(wst[:kn, :H], wt[f"wd_{tag}"][k0:k0 + kn])
                t = wpool.tile([kn, H], f16, name=f"wd16_{tag}_{k0}")
                cast(cse, t, wst[:kn, :H])
                wd_chunks.append((k0, kn, t))
                yield
            W[f"wd_{tag}"] = wd_chunks

        def load_w1r(tag, dma, cse):
            # o-matmul rhs (fp16): rows 400..599 of w1, + b1 as ones-row 72 of c2
            wst = stage.tile([128, 1600], f32, tag="stg", bufs=1)
            dma.dma_start(wst, wt[f"w1_{tag}"][400:528])
            w1rc1 = wpool.tile([128, H * PMX], f16, name=f"w1r16c1_{tag}")
            cast(cse, w1rc1, wst)
            yield
            wst = stage.tile([128, 1600], f32, tag="stg", bufs=1)
            dma.dma_start(wst[:72], wt[f"w1_{tag}"][528:600])
            dma.dma_start(wst[72:73], wt[f"b1_{tag}"])
            w1rc2 = wpool.tile([73, H * PMX], f16, name=f"w1r16c2_{tag}")
            cast(cse, w1rc2, wst[:73])
            yield
            W[f"w1r_{tag}"] = (w1rc1, w1rc2)

        def load_w1m(tag, dma, csw):
            # stage-1 rhs rows 0..383 as fp16 [128, 3, 1600]
            w1m = wpool.tile([128, 3, H * PMX], f16, name=f"w1m_{tag}")
            for c in range(3):
                wst = stage.tile([128, 1600], f32, tag="stg", bufs=1)
                dma.dma_start(wst, wt[f"w1_{tag}"][128 * c:128 * (c + 1)])
                cast(csw, w1m[:, c, :], wst)
                yield
            W[f"w1m_{tag}"] = w1m
            # shared c3 rhs: rows 0..15 = W1[384:400] fp16, rows 16/17 = o hi/lo
            wst = stage.tile([128, 1600], f32, tag="stg", bufs=1)
            dma.dma_start(wst[:16], wt[f"w1_{tag}"][384:400])
            c3pair = []
            for pi in range(2):
                c3 = wpool.tile([18, H * PMX], f16, name=f"c3_{tag}_{pi}")
                cast(csw, c3[:16], wst[:16])
                c3pair.append(c3)
            yield
            W[f"c3_{tag}"] = c3pair

        def load_scoring2(tag, dma, csw, stg):
            # stage-2 rhs
            w2c1 = wpool.tile([128, H * PMX], f16, name=f"w2c1_{tag}")
            wst = stage.tile([128, 1600], f32, tag=stg, bufs=1)
            dma.dma_start(wst, wt[f"w2_{tag}"][:128])
            cast(csw, w2c1, wst)
            yield
            W[f"w2c1_{tag}"] = w2c1
            w2c2 = wpool.tile([74, H * PMX], f16, name=f"w2c2_{tag}")
            wst = stage.tile([128, 1600], f32, tag=stg, bufs=1)
            dma.dma_start(wst[:72], wt[f"w2_{tag}"][128:200])
            cast(csw, w2c2[:72], wst[:72])
            dma.dma_start(w2c2[72:73, :], wt[f"b2h_{tag}"])
            dma.dma_start(w2c2[73:74, :], wt[f"b2l_{tag}"])
            yield
            W[f"w2c2_{tag}"] = w2c2
            # stage-3 rhs chunks
            wst = stage.tile([128, 1600], f32, tag=stg, bufs=1)
            dma.dma_start(wst[:, 0:8], wt[f"w3_{tag}"][0:128])
            dma.dma_start(wst[:72, 8:16], wt[f"w3_{tag}"][128:200])
            dma.dma_start(wst[:, 16:24], wt[f"w3_{tag}"][200:328])
            dma.dma_start(wst[:72, 24:32], wt[f"w3_{tag}"][328:400])
            w3c1 = wpool.tile([128, PMX], f16, name=f"w3c1_{tag}")
            cast(csw, w3c1, wst[:, 0:8])
            w3c2 = wpool.tile([74, PMX], f16, name=f"w3c2_{tag}")
            cast(csw, w3c2[:72], wst[:72, 8:16])
            yield
            dma.dma_start(w3c2[72:73, :], wt[f"b3h_{tag}"])
            dma.dma_start(w3c2[73:74, :], wt[f"b3l_{tag}"])
            w3c3 = wpool.tile([128, PMX], f16, name=f"w3c3_{tag}")
            cast(csw, w3c3, wst[:, 16:24])
            w3c4 = wpool.tile([72, PMX], f16, name=f"w3c4_{tag}")
            cast(csw, w3c4, wst[:72, 24:32])
            yield
            W[f"w3_{tag}"] = (w3c1, w3c2, w3c3, w3c4)

        def load_lstm(dma, cse):
            # LSTM weights as fp16 rhs [kn, 800] over rows = cat(es, ee);
            # es-c3 chunk carries b_lstm as ones-row 16
            ih_chunks = []
            for (k0, kn) in [(0, 128), (128, 128), (256, 128), (384, 16),
                             (400, 128), (528, 128), (656, 128), (784, 16)]:
                wst = stage.tile([128, 1600], f32, tag="stg", bufs=1)
                dma.dma_start(wst[:kn, :800], wt["w_ih"][k0:k0 + kn])
                rows = kn + 1 if k0 == 384 else kn
                t = wpool.tile([rows, 4 * H], f16, name=f"wih16_{k0}")
                if k0 == 384:
                    dma.dma_start(wst[16:17, :800], wt["b_lstm"])
                cast(cse, t[:rows], wst[:rows, :800])
                ih_chunks.append((k0, kn, t))
                yield
            # mlp fp16 rhs [kn, 200]; c2 carries b_mlp as ones-row 72
            wst = stage.tile([128, 1600], f32, tag="stg", bufs=1)
            dma.dma_start(wst[:, :H], wt["w_mlp"][0:128])
            wmlpc1 = wpool.tile([128, H], f16, name="wmlp16c1")
            cast(cse, wmlpc1, wst[:, :H])
            yield
            wst = stage.tile([128, 1600], f32, tag="stg", bufs=1)
            dma.dma_start(wst[:72, :H], wt["w_mlp"][128:200])
            dma.dma_start(wst[72:73, :H], wt["b_mlp"])
            wmlpc2 = wpool.tile([73, H], f16, name="wmlp16c2")
            cast(cse, wmlpc2, wst[:73, :H])
            W["ih_chunks"] = ih_chunks
            W["wmlp"] = (wmlpc1, wmlpc2)
            yield

        # ---------------- E load ----------------
        E16 = []
        E16c3 = []

        def load_E(b, dma, defer=None):
            et = epool.tile([128, 3, L], f16, name=f"E16_{b}")
            ec3 = epool.tile([18, L], f16, name=f"E16c3_{b}")
            nc.gpsimd.memset(ec3, 1.0)
            HL = L // 2
            for h in range(2):
                hs = slice(h * HL, (h + 1) * HL)
                est = stage.tile([128, 3, HL], f32, tag="estg", bufs=2)
                for c in range(3):
                    dma.dma_start(est[:, c, :], enc[b, 128 * c:128 * (c + 1), hs])
                est2 = stage.tile([16, HL], f32, tag="estg2", bufs=1)
                dma.dma_start(est2, enc[b, 384:400, hs])
                if defer is None:
                    nc.vector.tensor_copy(et[:, :, hs], est)
                    nc.vector.tensor_copy(ec3[:16, hs], est2)
                else:
                    defer.append(lambda et=et, est=est, hs=hs:
                                 nc.scalar.activation(et[:, :, hs], est, AF.Copy))
                    defer.append(lambda ec3=ec3, est2=est2, hs=hs:
                                 nc.scalar.activation(ec3[:16, hs], est2, AF.Copy))
            E16.append(et)
            E16c3.append(ec3)

        # ---------------- persistent state tiles ----------------
        hxT = [keep.tile([128, BLOC], f16, name="hxT16_0"),
               keep.tile([72, BLOC], f16, name="hxT16_1")]
        rT1 = keep.tile([128, BLOC], f16, name="rT1")
        rT2 = keep.tile([73, BLOC], f16, name="rT2")
        nc.gpsimd.memset(rT2, 1.0)                 # row 72 = b1 fold row
        h0T1 = keep.tile([128, BLOC], f16, name="h0T1")
        h0T2 = keep.tile([73, BLOC], f16, name="h0T2")
        nc.gpsimd.memset(h0T2, 1.0)                # row 72 = b_mlp fold row

        m1_slots, m2_slots, m1c2_slots = [], [], []
        for i in range(4):
            m1_slots.append(keep.tile([128, H], f16, name=f"m1_slot{i}"))
            m2_slots.append(keep.tile([128, H], f16, name=f"m2_slot{i}"))
            t = keep.tile([74, 128], f16, name=f"m1c2_slot{i}")
            nc.gpsimd.memset(t, 1.0)
            m1c2_slots.append(t)

        # ---------------- helpers ----------------
        def cat_chunks(tag, with_hx):
            """(lhsT [kn,4] fp16, wd16 [kn,H]) pairs for r = tanh(cat @ wd)."""
            ops = []
            for (k0, kn, wtile) in W[f"wd_{tag}"]:
                if k0 < 200:
                    if not with_hx:
                        continue
                    lhsT = hxT[0] if k0 == 0 else hxT[1]
                elif k0 < 600:
                    c = (k0 - 200) // 128
                    lhsT = es16[:, c, :] if c < 3 else es16c3[:16]
                else:
                    c = (k0 - 600) // 128
                    lhsT = ee16[:, c, :] if c < 3 else ee16c3
                ops.append((lhsT, wtile))
            return ops

        def r_matmul(tag, with_hx):
            """r_row = tanh(cat @ wd) -> [4, H] fp16 sbuf."""
            ops = cat_chunks(tag, with_hx)
            pt = ps_d.tile([128, 512], f32, tag="ps_ser")
            for i, (lhsT, rhs) in enumerate(ops):
                nc.tensor.matmul(pt[:BLOC, :H], lhsT, rhs,
                                 start=(i == 0), stop=(i == len(ops) - 1))
            r_row = work.tile([BLOC, H], f16, tag="r_row", bufs=1)
            nc.scalar.activation(r_row, pt[:BLOC, :H], AF.Tanh)
            return r_row

        def r_transpose(r_row):
            ptr = ps_tr.tile([128, 128], f16, tag="ps_tr")
            nc.tensor.transpose(ptr[:, :BLOC], r_row[:, 0:128], ident[:BLOC, :BLOC])
            nc.vector.tensor_copy(rT1, ptr[:, :BLOC])
            ptr2 = ps_tr.tile([128, 128], f16, tag="ps_tr")
            nc.tensor.transpose(ptr2[:72, :BLOC], r_row[:, 128:200], ident[:BLOC, :BLOC])
            nc.vector.tensor_copy(rT2[:72], ptr2[:72, :BLOC])

        def o_rows(tag, r_row, save=False):
            """o = r@w1r + b1 (f32 psum) -> fp16 hi/lo rows [BLOC, 1600]."""
            if save:
                o_tmp = work.tile([BLOC, H * PMX], f32, tag="o_tmp", bufs=1)
            r_transpose(r_row)
            w1rc1, w1rc2 = W[f"w1r_{tag}"]
            oh = single.tile([BLOC, H * PMX], f16, tag="oh")
            ol = single.tile([BLOC, H * PMX], f16, tag="ol")
            for (n0, nn) in NCH:
                pt = ps_d.tile([128, 512], f32, tag="ps_ser")
                nc.tensor.matmul(pt[:BLOC, :nn], rT1, w1rc1[:, n0:n0 + nn], start=True, stop=False)
                nc.tensor.matmul(pt[:BLOC, :nn], rT2, w1rc2[:, n0:n0 + nn], start=False, stop=True)
                nc.scalar.activation(oh[:, n0:n0 + nn], pt[:BLOC, :nn], AF.Copy)
                nc.vector.tensor_tensor(ol[:, n0:n0 + nn], pt[:BLOC, :nn], oh[:, n0:n0 + nn], OP.subtract)
                if save:
                    nc.vector.tensor_copy(o_tmp[:, n0:n0 + nn], pt[:BLOC, :nn])
            if save:
                nc.gpsimd.dma_start(o_save_d[0 if tag == "s" else 1], o_tmp)
            return oh, ol

        def delta_o_rows(tag, r_row):
            """d16 [4,1600] f16 = (r@w1r+b1) - o_save (sweep-1/2's o psum)."""
            o_tmp = work.tile([BLOC, H * PMX], f32, tag="o_tmp", bufs=1)
            nc.gpsimd.dma_start(o_tmp, o_save_d[0 if tag == "s" else 1])
            r_transpose(r_row)
            w1rc1, w1rc2 = W[f"w1r_{tag}"]
            d16 = work.tile([BLOC, H * PMX], f16, tag="d16", bufs=1)
            for (n0, nn) in NCH:
                pt = ps_d.tile([128, 512], f32, tag="ps_ser")
                nc.tensor.matmul(pt[:BLOC, :nn], rT1, w1rc1[:, n0:n0 + nn], start=True, stop=False)
                nc.tensor.matmul(pt[:BLOC, :nn], rT2, w1rc2[:, n0:n0 + nn], start=False, stop=True)
                nc.vector.tensor_tensor(d16[:, n0:n0 + nn], pt[:BLOC, :nn],
                                        o_tmp[:, n0:n0 + nn], OP.subtract)
            return d16

        def bcast_do(d16, b):
            """broadcast d16 row b to a [128,1600] f16 tile via PE row-select."""
            bc = work.tile([128, H * PMX], f16, tag="dbc", bufs=2)
            for (n0, nn) in NCH:
                pb = ps_a.tile([128, 512], f32, tag="ps_s1")
                nc.tensor.matmul(pb[:, :nn], selmat[:, 128 * b:128 * (b + 1)],
                                 d16[:, n0:n0 + nn], start=True, stop=True)
                nc.scalar.activation(bc[:, n0:n0 + nn], pb[:, :nn], AF.Copy)
            return bc

        def lsm_row(s4row, b, out_dram):
            """log_softmax of one S4 row -> out_dram[b]."""
            gmax = work.tile([1, 1], f32, tag="gmax", bufs=4)
            nc.vector.tensor_reduce(gmax, s4row, axis=AX.X, op=OP.max)
            negm = work.tile([1, 1], f32, tag="negm", bufs=4)
            nc.vector.tensor_scalar_mul(negm, gmax, -1.0)
            e4 = work.tile([1, L], f32, tag="rowtmp", bufs=1)
            sume = work.tile([1, 1], f32, tag="sume", bufs=4)
            nc.scalar.activation(e4, s4row, AF.Exp, bias=negm[:, 0:1], accum_out=sume)
            lnz = work.tile([1, 1], f32, tag="lnz", bufs=4)
            nc.scalar.activation(lnz, sume, AF.Ln)
            lse = work.tile([1, 1], f32, tag="lse", bufs=4)
            nc.vector.tensor_tensor(lse, gmax, lnz, OP.add)
            lp4 = work.tile([1, L], f32, tag="rowtmp", bufs=1)
            nc.vector.tensor_scalar(lp4, s4row, lse[:, 0:1], None, op0=OP.subtract)
            dma_engines[b % 3].dma_start(out_dram[b:b + 1, :], lp4)

        def argmax_gather_b(s4row, b, dstbig, dstc3):
            mx = work.tile([1, 8], f32, tag="mx", bufs=4)
            idx = work.tile([1, 8], u32, tag="idx", bufs=4)
            nc.vector.max(out=mx, in_=s4row)
            nc.vector.max_index(out=idx, in_max=mx, in_values=s4row)
            reg = nc.values_load(idx[0:1, 0:1], min_val=0, max_val=L - 1,
                                 skip_runtime_bounds_check=True)
            dma_engines[(2 * b) % 3].dma_start(
                dstbig[:, :, b:b + 1], E16[b][:, :, ds(reg, 1)])
            dma_engines[(2 * b + 1) % 3].dma_start(
                dstc3[:16, b:b + 1], E16c3[b][:16, ds(reg, 1)])

        # shared g2/g3 stages of the scoring pipeline
        def make_g23(tag, s4rows, st, strips, s2_sbuf=False):
            w2c1 = W[f"w2c1_{tag}"]
            w2c2 = W[f"w2c2_{tag}"]
            w3c1, w3c2, w3c3, w3c4 = W[f"w3_{tag}"]

            def g2(i):
                m1 = st[i]["m1"]
                pt1 = ps_tr.tile([128, 128], f16, tag="ps_tr")
                nc.tensor.transpose(pt1, m1[:, 0:128], ident)
                m1c1 = work.tile([128, 128], f16, tag="m1c1", bufs=4)
                nc.scalar.activation(m1c1, pt1, AF.Copy)
                pt2 = ps_tr.tile([128, 128], f16, tag="ps_tr")
                nc.tensor.transpose(pt2[:72], m1[:, 128:200], ident)
                m1c2 = m1c2_slots[i % 4]
                nc.scalar.activation(m1c2[:72], pt2[:72], AF.Copy)
                m2 = m2_slots[i % 4]
                if s2_sbuf:
                    # drain stage-2 psum via scalar fp16 copies; maxpool from
                    # SBUF on vector (round-then-max == max-then-round)
                    s2s = work.tile([128, H * PMX], f16, tag="s2s", bufs=2)
                    for ni, (n0, nn) in enumerate(NCH):
                        pb = ps_b.tile([128, 512], f32, tag="ps_s2")
                        nc.tensor.matmul(pb[:, :nn], m1c1, w2c1[:, n0:n0 + nn], start=True, stop=False)
                        nc.tensor.matmul(pb[:, :nn], m1c2, w2c2[:, n0:n0 + nn], start=False, stop=True)
                        nc.scalar.activation(s2s[:, n0:n0 + nn], pb[:, :nn], AF.Copy)
                    nc.vector.tensor_reduce(
                        m2, s2s.rearrange("p (h q) -> p h q", q=PMX),
                        axis=AX.X, op=OP.max)
                else:
                    for ni, (n0, nn) in enumerate(NCH):
                        pb = ps_b.tile([128, 512], f32, tag="ps_s2")
                        nc.tensor.matmul(pb[:, :nn], m1c1, w2c1[:, n0:n0 + nn], start=True, stop=False)
                        nc.tensor.matmul(pb[:, :nn], m1c2, w2c2[:, n0:n0 + nn], start=False, stop=True)
                        h0, hn = HSL[ni]
                        nc.vector.tensor_reduce(
                            m2[:, h0:h0 + hn],
                            pb[:, :nn].rearrange("p (h q) -> p h q", q=PMX),
                            axis=AX.X, op=OP.max)
                st[i]["m1c1"] = m1c1
                st[i]["m1c2"] = m1c2
                st[i]["m2"] = m2

            def g3(i):
                b, lt = divmod(i, NLT)
                m2 = st[i]["m2"]
                pt3 = ps_tr.tile([128, 128], f16, tag="ps_tr")
                nc.tensor.transpose(pt3, m2[:, 0:128], ident)
                m2c1 = work.tile([128, 128], f16, tag="m2c1", bufs=4)
                nc.scalar.activation(m2c1, pt3, AF.Copy)
                pt4 = ps_tr.tile([128, 128], f16, tag="ps_tr")
                nc.tensor.transpose(pt4[:72], m2[:, 128:200], ident)
                m2c2 = work.tile([72, 128], f16, tag="m2c2", bufs=4)
                nc.scalar.activation(m2c2, pt4[:72], AF.Copy)
                if lt == 0:
                    strips[b] = ps_d.tile([128, 8 * NLT], f32, tag="ps_ser", name="s3strip")
                psl = strips[b][:, 8 * lt:8 * (lt + 1)]
                nc.tensor.matmul(psl, st[i]["m1c1"], w3c1, start=True, stop=False)
                nc.tensor.matmul(psl, st[i]["m1c2"], w3c2, start=False, stop=False)
                nc.tensor.matmul(psl, m2c1, w3c3, start=False, stop=False)
                nc.tensor.matmul(psl, m2c2, w3c4, start=False, stop=True)
                st[i].clear()
                if lt == NLT - 1:
                    Sb = work.tile([128, NLT], f32, tag="Sb")
                    nc.vector.tensor_reduce(Sb,
                                            strips[b].rearrange("p (t q) -> p t q", q=PMX),
                                            axis=AX.X, op=OP.max)
                    nc.vector.tensor_tensor(Sb, Sb, penT_sb[:, b * NLT:(b + 1) * NLT],
                                            OP.subtract)
                    ptb = ps_tr.tile([NLT, 128], f32, tag="ps_tr")
                    nc.tensor.transpose(ptb, Sb, ident32)
                    s4stg = work.tile([NLT, 128], f32, tag="s4stg")
                    nc.scalar.activation(s4stg, ptb, AF.Copy)
                    s4row = work.tile([1, L], f32, tag="s4row", bufs=2)
                    dma_engines[b % 3].dma_start(s4row, s4stg)
                    s4rows[b] = s4row
            return g2, g3

        def run_pipeline(NT, g1, g2, g3, fill, batch_cb):
            # batch b's callback fires two tiles after its strip completes
            # (so the S4-row DMA has landed); last batch fires immediately.
            cb_at = {b * NLT + NLT + 1: b for b in range(BLOC - 1)}
            cb_at[NT - 1] = BLOC - 1
            for i in range(NT + 2):
                if i < NT:
                    g1(i)
                if 1 <= i < NT + 1:
                    g2(i - 1)
                if 2 <= i:
                    j = i - 2
                    g3(j)
                    if batch_cb is not None and j in cb_at:
                        batch_cb(cb_at[j])
                if fill is not None:
                    next(fill, None)
            if fill is not None:
                for _ in fill:
                    pass

        def score_sweep(tag, s4rows, oh, ol, fill=None, batch_cb=None, a_out=None):
            """Full maxout scoring sweep; optionally spills stage-1 psum (fp16)."""
            w1m = W[f"w1m_{tag}"]
            c3pair = W[f"c3_{tag}"]
            NT = BLOC * NLT
            st = [dict() for _ in range(NT)]
            strips = {}
            g2, g3 = make_g23(tag, s4rows, st, strips)

            def g1(i):
                b, lt = divmod(i, NLT)
                c3rhs = c3pair[b % 2]
                if lt == 0:
                    nc.sync.dma_start(c3rhs[16:17, :], oh[b:b + 1, :])
                    nc.sync.dma_start(c3rhs[17:18, :], ol[b:b + 1, :])
                lsl = slice(128 * lt, 128 * (lt + 1))
                m1 = m1_slots[i % 4]
                if a_out is not None:
                    a_w = stage.tile([128, H * PMX], f16, tag="aw", bufs=1)
                for ni, (n0, nn) in enumerate(NCH):
                    pa = ps_a.tile([128, 512], f32, tag="ps_s1")
                    for c in range(3):
                        nc.tensor.matmul(pa[:, :nn], E16[b][:, c, lsl], w1m[:, c, n0:n0 + nn],
                                         start=(c == 0), stop=False)
                    nc.tensor.matmul(pa[:, :nn], E16c3[b][:, lsl], c3rhs[:, n0:n0 + nn],
                                     start=False, stop=True)
                    h0, hn = HSL[ni]
                    nc.vector.tensor_reduce(
                        m1[:, h0:h0 + hn],
                        pa[:, :nn].rearrange("p (h q) -> p h q", q=PMX),
                        axis=AX.X, op=OP.max)
                    if a_out is not None:
                        nc.scalar.activation(a_w[:, n0:n0 + nn], pa[:, :nn], AF.Copy)
                if a_out is not None:
                    dma_engines[(b + lt) % 3].dma_start(a_out[b, lt], a_w)
                st[i]["m1"] = m1

            run_pipeline(NT, g1, g2, g3, fill, batch_cb)

        def reuse_sweep(tag, s4rows, d16, a_in, batch_cb=None):
            """Scoring sweep: stage-1 = vector add of the DRAM A-tile and the
            per-batch delta-o broadcast, then an SBUF fp16 maxpool."""
            NT = BLOC * NLT
            st = [dict() for _ in range(NT)]
            strips = {}
            a_tiles = {}
            dbc = {0: bcast_do(d16, 0), 1: bcast_do(d16, 1)}
            g2, g3 = make_g23(tag, s4rows, st, strips, s2_sbuf=True)

            def fetch(j):
                if j >= NT:
                    return
                b, lt = divmod(j, NLT)
                at = stage.tile([128, H * PMX], f16, tag="ar", bufs=2)
                dma_engines[j % 3].dma_start(at, a_in[b, lt])
                a_tiles[j] = at

            fetch(0)
            fetch(1)

            def g1(i):
                b, lt = divmod(i, NLT)
                fetch(i + 2)
                # broadcast the next batch's delta-o two tiles early
                if lt == NLT - 2 and b + 2 < BLOC:
                    dbc[b + 2] = bcast_do(d16, b + 2)
                at = a_tiles.pop(i)
                t = work.tile([128, H * PMX], f16, tag="t_add", bufs=1)
                nc.vector.tensor_tensor(t, at, dbc[b], OP.add)
                m1 = m1_slots[i % 4]
                nc.vector.tensor_reduce(
                    m1, t.rearrange("p (h q) -> p h q", q=PMX),
                    axis=AX.X, op=OP.max)
                st[i]["m1"] = m1

            run_pipeline(NT, g1, g2, g3, None, batch_cb)

        def lstm_update():
            """hx via LSTM cell with hx0=cx0=0 (f-gate and w_hh drop out)."""
            pt_i = ps_d.tile([128, 512], f32, tag="ps_ser")
            pt_go = ps_d.tile([128, 512], f32, tag="ps_ser")
            lhs_for = []
            for (k0, kn, wtile) in W["ih_chunks"]:
                if k0 < 400:
                    c = k0 // 128
                    lhsT = es16[:, c, :] if c < 3 else es16c3  # [17,4] w/ ones
                else:
                    c = (k0 - 400) // 128
                    lhsT = ee16[:, c, :] if c < 3 else ee16c3
                lhs_for.append((lhsT, wtile, kn + (1 if k0 == 384 else 0)))
            n = len(lhs_for)
            for i, (lhsT, wtile, rows) in enumerate(lhs_for):
                nc.tensor.matmul(pt_i[:BLOC, :H], lhsT, wtile[:rows, 0:H],
                                 start=(i == 0), stop=(i == n - 1))
            for i, (lhsT, wtile, rows) in enumerate(lhs_for):
                nc.tensor.matmul(pt_go[:BLOC, :2 * H], lhsT, wtile[:rows, 2 * H:4 * H],
                                 start=(i == 0), stop=(i == n - 1))
            ig = work.tile([BLOC, H], f32, tag="ig", bufs=1)
            nc.scalar.activation(ig, pt_i[:BLOC, :H], AF.Sigmoid)
            gg = work.tile([BLOC, H], f32, tag="gg", bufs=1)
            nc.scalar.activation(gg, pt_go[:BLOC, 0:H], AF.Tanh)
            og = work.tile([BLOC, H], f32, tag="og", bufs=1)
            nc.scalar.activation(og, pt_go[:BLOC, H:2 * H], AF.Sigmoid)
            cx = work.tile([BLOC, H], f32, tag="cx", bufs=1)
            nc.vector.tensor_tensor(cx, ig, gg, OP.mult)
            tcx = work.tile([BLOC, H], f32, tag="tcx", bufs=1)
            nc.scalar.activation(tcx, cx, AF.Tanh)
            h0 = work.tile([BLOC, H], f16, tag="h0", bufs=1)
            nc.vector.tensor_tensor(h0, og, tcx, OP.mult)
            ptr = ps_tr.tile([128, 128], f16, tag="ps_tr")
            nc.tensor.transpose(ptr[:, :BLOC], h0[:, 0:128], ident[:BLOC, :BLOC])
            nc.vector.tensor_copy(h0T1, ptr[:, :BLOC])
            ptr2 = ps_tr.tile([128, 128], f16, tag="ps_tr")
            nc.tensor.transpose(ptr2[:72, :BLOC], h0[:, 128:200], ident[:BLOC, :BLOC])
            nc.vector.tensor_copy(h0T2[:72], ptr2[:72, :BLOC])
            pt = ps_d.tile([128, 512], f32, tag="ps_ser")
            wmlpc1, wmlpc2 = W["wmlp"]
            nc.tensor.matmul(pt[:BLOC, :H], h0T1, wmlpc1, start=True, stop=False)
            nc.tensor.matmul(pt[:BLOC, :H], h0T2, wmlpc2, start=False, stop=True)
            hx_row = work.tile([BLOC, H], f16, tag="hx_row", bufs=1)
            nc.scalar.activation(hx_row, pt[:BLOC, :H], AF.Copy)
            ptr3 = ps_tr.tile([128, 128], f16, tag="ps_tr")
            nc.tensor.transpose(ptr3[:, :BLOC], hx_row[:, 0:128], ident[:BLOC, :BLOC])
            nc.vector.tensor_copy(hxT[0], ptr3[:, :BLOC])
            ptr4 = ps_tr.tile([128, 128], f16, tag="ps_tr")
            nc.tensor.transpose(ptr4[:72, :BLOC], hx_row[:, 128:200], ident[:BLOC, :BLOC])
            nc.vector.tensor_copy(hxT[1], ptr4[:72, :BLOC])

        # ---------------- prologue: s-critical-path loads ----------------
        import itertools
        for _ in load_wd("s", nc.sync, nc.vector):
            pass
        for _ in load_w1r("s", nc.sync, nc.vector):
            pass
        load_E(0, nc.gpsimd)
        for _ in load_w1m("s", nc.scalar, nc.scalar):
            pass
        for _ in load_scoring2("s", nc.scalar, nc.scalar, "stg"):
            pass
        edefer = []
        load_E(1, nc.gpsimd, edefer)
        load_E(2, nc.gpsimd, edefer)
        load_E(3, nc.gpsimd, edefer)

        def edefer_gen():
            for fn in edefer:
                fn()
                yield
        fill_steps = itertools.chain(
            edefer_gen(),
            load_wd("e", nc.sync, nc.scalar),
            load_w1r("e", nc.sync, nc.scalar),
            load_w1m("e", nc.sync, nc.scalar),
            load_scoring2("e", nc.sync, nc.scalar, "stg"),
            load_lstm(nc.sync, nc.scalar))

        # ---------------- the four passes ----------------
        r_row = r_matmul("s", with_hx=False)
        oh, ol = o_rows("s", r_row, save=REUSE)
        rows1 = {}
        score_sweep("s", rows1, oh, ol, fill=fill_steps,
                    batch_cb=lambda b: argmax_gather_b(rows1[b], b, es16, es16c3),
                    a_out=a1d if REUSE else None)

        r_row = r_matmul("e", with_hx=False)
        oh, ol = o_rows("e", r_row, save=REUSE)
        rows2 = {}
        score_sweep("e", rows2, oh, ol,
                    batch_cb=lambda b: argmax_gather_b(rows2[b], b, ee16, ee16c3),
                    a_out=a2d if REUSE else None)

        lstm_update()

        rows3 = {}
        rows4 = {}

        def cb3(b):
            argmax_gather_b(rows3[b], b, es16, es16c3)
            if b < BLOC - 1:
                lsm_row(rows3[b], b, lp1)

        def cb4(b):
            if b < BLOC - 1:
                lsm_row(rows4[b], b, lp2)

        r_row = r_matmul("s", with_hx=True)
        if REUSE:
            d16 = delta_o_rows("s", r_row)
            reuse_sweep("s", rows3, d16, a1d, batch_cb=cb3)
            r_row = r_matmul("e", with_hx=True)
            d16 = delta_o_rows("e", r_row)
            lsm_row(rows3[BLOC - 1], BLOC - 1, lp1)
            reuse_sweep("e", rows4, d16, a2d, batch_cb=cb4)
        else:
            oh, ol = o_rows("s", r_row)
            score_sweep("s", rows3, oh, ol, batch_cb=cb3)
            r_row = r_matmul("e", with_hx=True)
            oh, ol = o_rows("e", r_row)
            lsm_row(rows3[BLOC - 1], BLOC - 1, lp1)
            score_sweep("e", rows4, oh, ol, batch_cb=cb4)
        lsm_row(rows4[BLOC - 1], BLOC - 1, lp2)

        stage_cm.__exit__(None, None, None)

    nc.compile()
    return nc


def get_program():
    if "nc" not in _cache:
        _cache["nc"] = _build_program()
    return _cache["nc"]


def _split16(x):
    hi = np.asarray(x, np.float32).astype(np.float16)
    lo = (np.asarray(x, np.float32) - hi.astype(np.float32)).astype(np.float16)
    return hi, lo


def make_in_maps(inputs):
    """Per-core input maps: batch shard + trivial host prep (mask, bias splits)."""
    inputs = {k: np.asarray(v) for k, v in inputs.items()}
    enc = np.ascontiguousarray(inputs["encoding_matrix"], dtype=np.float32)
    lens = np.asarray(inputs["passage_lens"]).astype(np.int64)
    pen_full = np.where(np.arange(L)[None, :] < lens[:, None],
                        np.float32(0.0), BIG).astype(np.float32)

    shared = {}
    for tag in ("s", "e"):
        shared[f"w1_{tag}"] = np.ascontiguousarray(inputs[f"w1_{tag}"], np.float32)
        shared[f"b1_{tag}"] = np.ascontiguousarray(inputs[f"b1_{tag}"], np.float32).reshape(1, -1)
        shared[f"w2_{tag}"] = np.ascontiguousarray(inputs[f"w2_{tag}"], np.float32)
        b2h, b2l = _split16(inputs[f"b2_{tag}"])
        shared[f"b2h_{tag}"] = b2h.reshape(1, -1)
        shared[f"b2l_{tag}"] = b2l.reshape(1, -1)
        shared[f"w3_{tag}"] = np.ascontiguousarray(inputs[f"w3_{tag}"], np.float32)
        b3h, b3l = _split16(inputs[f"b3_{tag}"])
        shared[f"b3h_{tag}"] = b3h.reshape(1, -1)
        shared[f"b3l_{tag}"] = b3l.reshape(1, -1)
        shared[f"wd_{tag}"] = np.ascontiguousarray(inputs[f"wd_{tag}"], np.float32)
    shared["w_ih"] = np.ascontiguousarray(inputs["w_ih"], np.float32)
    shared["b_lstm"] = np.ascontiguousarray(inputs["b_lstm"], np.float32).reshape(1, -1)
    shared["w_mlp"] = np.ascontiguousarray(inputs["w_mlp"], np.float32)
    shared["b_mlp"] = np.ascontiguousarray(inputs["b_mlp"], np.float32).reshape(1, -1)

    in_maps = []
    for core in range(NCORES):
        sl = slice(core * BLOC, (core + 1) * BLOC)
        m = dict(shared)
        m["enc"] = np.ascontiguousarray(enc[sl])
        m["selmat_d"] = np.kron(np.eye(BLOC, dtype=np.float16),
                                np.ones((1, 128), dtype=np.float16))
        pc = pen_full[sl].reshape(BLOC, NLT, 128)
        m["penT"] = np.ascontiguousarray(pc.transpose(2, 0, 1).reshape(128, BLOC * NLT))
        in_maps.append(m)
    return in_maps


def run_on_hw(inputs, trace=False):
    from concourse import bass_utils
    nc = get_program()
    in_maps = make_in_maps(inputs)
    res = bass_utils.run_bass_kernel_spmd(nc, in_maps, core_ids=list(range(NCORES)),
                                          trace=trace)
    lp1 = np.concatenate([res.results[c]["lp1"] for c in range(NCORES)], axis=0)
    lp2 = np.concatenate([res.results[c]["lp2"] for c in range(NCORES)], axis=0)
    return (np.asarray(lp1, np.float32), np.asarray(lp2, np.float32)), res


def kernel(**inputs):
    out, _ = run_on_hw(inputs, trace=False)
    return out


# revision 19
# speedup vs baseline: 1.1380x; 1.0302x over previous
"""Trainium2 Bass kernel for nn_Decoder_86921548137026.

Dynamic decoder: NITER=2 iterations of (maxout pointer scoring over L=1024
positions -> argmax -> gather -> LSTM cell), followed by log_softmax over the
final start/end scores.

Sharding: data-parallel over batch B=32 across 8 cores (4 batches/core),
weights replicated.

v3 changes vs v2 (934us):
  - prologue restructured: critical-path DMA order (wd -> w1r -> E b0 -> w1m
    -> w2/w3), E b1-3 on the gpsimd queue, r/o emitted early; sweep-1 starts
    ~15us instead of ~94us
  - penalty fold moved from S4-post to a per-batch f32 subtract on Sb before
    the transpose (bit-exact, removes it from the boundary path)
  - per-batch argmax/gather pipelined into the sweep (batch b resolved two
    tiles after its strip completes); per-batch log_softmax rows likewise
  - sweeps 3/4 reuse stage-1: sweep-1/2 stage-1 PSUM (A = E@W1 + o_fold) is
    rounded to fp16 and spilled to DRAM; the iter-2 sweeps replace the big
    stage-1 matmul with a vector add of the per-batch delta-o broadcast and
    an SBUF fp16 maxpool (validated offline: 0 argmax flips, rel 3.6e-4)
"""

import numpy as np

H = 200
PMX = 8
B = 32
L = 1024
BIG = np.float32(1e30)
NCORES = 8
BLOC = B // NCORES          # 4 batches per core
NLT = L // 128              # 8 l-tiles per batch
# stage-1/2 output channels = H*P = 1600, swept in PSUM-bank-sized chunks
NCH = [(0, 512), (512, 512), (1024, 512), (1536, 64)]
# h-slice of m1/m2 produced by each n-chunk (1600 = 200h * 8p, h-major)
HSL = [(0, 64), (64, 64), (128, 64), (192, 8)]
REUSE = True               # sweeps 3/4 via DRAM A-spill + delta-o

_cache = {}


def _build_program(nt_b):
    import contextlib
    import concourse.mybir as mybir
    import concourse.tile as tile
    from concourse import bacc
    from concourse.bass import ds
    from concourse.masks import make_identity

    f32 = mybir.dt.float32
    f16 = mybir.dt.float16
    u32 = mybir.dt.uint32
    AF = mybir.ActivationFunctionType
    OP = mybir.AluOpType
    AX = mybir.AxisListType

    nc = bacc.Bacc("TRN2", target_bir_lowering=False, debug=False,
                   enable_asserts=False, num_devices=NCORES)

    # ---------------- DRAM I/O ----------------
    enc = nc.dram_tensor("enc", [BLOC, 2 * H, L], f32, kind="ExternalInput").ap()
    penT = nc.dram_tensor("penT", [128, BLOC * NLT], f32, kind="ExternalInput").ap()
    selmat_d = nc.dram_tensor("selmat_d", [BLOC, BLOC * 128], f16, kind="ExternalInput").ap()
    wt = {}
    for tag in ("s", "e"):
        wt[f"w1_{tag}"] = nc.dram_tensor(f"w1_{tag}", [3 * H, H * PMX], f32, kind="ExternalInput").ap()
        wt[f"b1_{tag}"] = nc.dram_tensor(f"b1_{tag}", [1, H * PMX], f32, kind="ExternalInput").ap()
        wt[f"w2_{tag}"] = nc.dram_tensor(f"w2_{tag}", [H, H * PMX], f32, kind="ExternalInput").ap()
        wt[f"b2h_{tag}"] = nc.dram_tensor(f"b2h_{tag}", [1, H * PMX], f16, kind="ExternalInput").ap()
        wt[f"b2l_{tag}"] = nc.dram_tensor(f"b2l_{tag}", [1, H * PMX], f16, kind="ExternalInput").ap()
        wt[f"w3_{tag}"] = nc.dram_tensor(f"w3_{tag}", [2 * H, PMX], f32, kind="ExternalInput").ap()
        wt[f"b3h_{tag}"] = nc.dram_tensor(f"b3h_{tag}", [1, PMX], f16, kind="ExternalInput").ap()
        wt[f"b3l_{tag}"] = nc.dram_tensor(f"b3l_{tag}", [1, PMX], f16, kind="ExternalInput").ap()
        wt[f"wd_{tag}"] = nc.dram_tensor(f"wd_{tag}", [5 * H, H], f32, kind="ExternalInput").ap()
    wt["w_ih"] = nc.dram_tensor("w_ih", [4 * H, 4 * H], f32, kind="ExternalInput").ap()
    wt["b_lstm"] = nc.dram_tensor("b_lstm", [1, 4 * H], f32, kind="ExternalInput").ap()
    wt["w_mlp"] = nc.dram_tensor("w_mlp", [H, H], f32, kind="ExternalInput").ap()
    wt["b_mlp"] = nc.dram_tensor("b_mlp", [1, H], f32, kind="ExternalInput").ap()

    lp1 = nc.dram_tensor("lp1", [BLOC, L], f32, kind="ExternalOutput").ap()
    lp2 = nc.dram_tensor("lp2", [BLOC, L], f32, kind="ExternalOutput").ap()
    if REUSE:
        a1d = nc.dram_tensor("a1d", [BLOC, NLT, 128, H * PMX], f16, kind="Internal").ap()
        a2d = nc.dram_tensor("a2d", [BLOC, NLT, 128, H * PMX], f16, kind="Internal").ap()
        o_save_d = nc.dram_tensor("o_save_d", [2, BLOC, H * PMX], f32, kind="Internal").ap()

    # one SPMD program for all cores: batch-slot b uses the max tile count
    # over cores so every core's valid region is covered
    NTB = [max(nt_b[c * BLOC + b] for c in range(NCORES)) for b in range(BLOC)]
    TILES = [(b, lt) for b in range(BLOC) for lt in range(NTB[b])]
    NT_ALL = len(TILES)

    with tile.TileContext(nc) as tc, contextlib.ExitStack() as ctx:
        const = ctx.enter_context(tc.tile_pool(name="const", bufs=1))
        wpool = ctx.enter_context(tc.tile_pool(name="wpool", bufs=1))
        epool = ctx.enter_context(tc.tile_pool(name="epool", bufs=1))
        work = ctx.enter_context(tc.tile_pool(name="work", bufs=3))
        single = ctx.enter_context(tc.tile_pool(name="single", bufs=1))
        keep = ctx.enter_context(tc.tile_pool(name="keep", bufs=1))
        ps_a = ctx.enter_context(tc.tile_pool(name="ps_a", bufs=3, space="PSUM"))
        ps_b = ctx.enter_context(tc.tile_pool(name="ps_b", bufs=2, space="PSUM"))
        ps_tr = ctx.enter_context(tc.tile_pool(name="ps_tr", bufs=2, space="PSUM"))
        ps_d = ctx.enter_context(tc.tile_pool(name="ps_d", bufs=1, space="PSUM"))
        stage_cm = tc.tile_pool(name="stage", bufs=2)
        stage = stage_cm.__enter__()

        dma_engines = [nc.sync, nc.scalar, nc.gpsimd]

        # ---------------- constants ----------------
        ident = const.tile([128, 128], f16, name="ident")
        make_identity(nc, ident)
        ident32 = const.tile([128, 128], f32, name="ident32")
        make_identity(nc, ident32)
        # row-selector stationaries: selmat[:, 128b:128(b+1)] broadcasts d16
        # row b to all 128 output partitions (host-built constant)
        selmat = const.tile([BLOC, BLOC * 128], f16, name="selmat")
        nc.sync.dma_start(selmat, selmat_d)

        penT_sb = const.tile([128, BLOC * NLT], f32, name="penT_sb")
        nc.sync.dma_start(penT_sb, penT)

        # ---------------- es/ee init from E column 0 ----------------
        es16 = keep.tile([128, 3, BLOC], f16, name="es16")
        es16c3 = keep.tile([17, BLOC], f16, name="es16c3")
        nc.gpsimd.memset(es16c3, 1.0)              # row 16 = b_lstm fold row
        ee16 = keep.tile([128, 3, BLOC], f16, name="ee16")
        ee16c3 = keep.tile([16, BLOC], f16, name="ee16c3")
        escol = const.tile([128, 4, BLOC], f32, name="escol")
        for c in range(3):
            nc.sync.dma_start(escol[:, c, :],
                              enc[0:BLOC, 128 * c:128 * (c + 1), 0:1].rearrange("b p x -> p (b x)"))
        nc.sync.dma_start(escol[:16, 3, :],
                          enc[0:BLOC, 384:400, 0:1].rearrange("b p x -> p (b x)"))
        nc.vector.tensor_copy(es16, escol[:, 0:3, :])
        nc.vector.tensor_copy(es16c3[:16], escol[:16, 3, :])
        nc.vector.tensor_copy(ee16, es16)
        nc.vector.tensor_copy(ee16c3, es16c3[:16])

        # ---------------- weight / E loaders ----------------
        W = {}

        def cast(eng, dst, src):
            if eng is nc.scalar:
                nc.scalar.activation(dst, src, AF.Copy)
            else:
                eng.tensor_copy(dst, src)

        def load_wd(tag, dma, cse):
            # wd fp16 k-chunk tiles over cat(hx[0:200], es[200:600], ee[600:1000])
            wd_chunks = []
            for (k0, kn) in [(200, 128), (328, 128), (456, 128), (584, 16),
                             (600, 128), (728, 128), (856, 128), (984, 16),
                             (0, 128), (128, 72)]:
                wst = stage.tile([128, 800], f32, tag="stg8", bufs=2)
                dma.dma_start(wst[:kn, :H], wt[f"wd_{tag}"][k0:k0 + kn])
                t = wpool.tile([kn, H], f16, name=f"wd16_{tag}_{k0}")
                cast(cse, t, wst[:kn, :H])
                wd_chunks.append((k0, kn, t))
                yield
            W[f"wd_{tag}"] = wd_chunks

        def load_w1r(tag, dma, cse):
            # o-matmul rhs (fp16): rows 400..599 of w1, + b1 as ones-row 72 of c2
            wst = stage.tile([128, 1600], f32, tag="stg", bufs=1)
            dma.dma_start(wst, wt[f"w1_{tag}"][400:528])
            w1rc1 = wpool.tile([128, H * PMX], f16, name=f"w1r16c1_{tag}")
            cast(cse, w1rc1, wst)
            yield
            wst = stage.tile([128, 1600], f32, tag="stg", bufs=1)
            dma.dma_start(wst[:72], wt[f"w1_{tag}"][528:600])
            dma.dma_start(wst[72:73], wt[f"b1_{tag}"])
            w1rc2 = wpool.tile([73, H * PMX], f16, name=f"w1r16c2_{tag}")
            cast(cse, w1rc2, wst[:73])
            yield
            W[f"w1r_{tag}"] = (w1rc1, w1rc2)

        def load_w1m(tag, dma, csw):
            # stage-1 rhs rows 0..383 as fp16 [128, 3, 1600]
            w1m = wpool.tile([128, 3, H * PMX], f16, name=f"w1m_{tag}")
            for c in range(3):
                wst = stage.tile([128, 1600], f32, tag="stg", bufs=1)
                dma.dma_start(wst, wt[f"w1_{tag}"][128 * c:128 * (c + 1)])
                cast(csw, w1m[:, c, :], wst)
                yield
            W[f"w1m_{tag}"] = w1m
            # shared c3 rhs: rows 0..15 = W1[384:400] fp16, rows 16/17 = o hi/lo
            wst = stage.tile([128, 1600], f32, tag="stg", bufs=1)
            dma.dma_start(wst[:16], wt[f"w1_{tag}"][384:400])
            c3pair = []
            for pi in range(2):
                c3 = wpool.tile([18, H * PMX], f16, name=f"c3_{tag}_{pi}")
                cast(csw, c3[:16], wst[:16])
                c3pair.append(c3)
            yield
            W[f"c3_{tag}"] = c3pair

        def load_scoring2(tag, dma, csw, stg):
            # stage-2 rhs
            w2c1 = wpool.tile([128, H * PMX], f16, name=f"w2c1_{tag}")
            wst = stage.tile([128, 1600], f32, tag=stg, bufs=1)
            dma.dma_start(wst, wt[f"w2_{tag}"][:128])
            cast(csw, w2c1, wst)
            yield
            W[f"w2c1_{tag}"] = w2c1
            w2c2 = wpool.tile([74, H * PMX], f16, name=f"w2c2_{tag}")
            wst = stage.tile([128, 1600], f32, tag=stg, bufs=1)
            dma.dma_start(wst[:72], wt[f"w2_{tag}"][128:200])
            cast(csw, w2c2[:72], wst[:72])
            dma.dma_start(w2c2[72:73, :], wt[f"b2h_{tag}"])
            dma.dma_start(w2c2[73:74, :], wt[f"b2l_{tag}"])
            yield
            W[f"w2c2_{tag}"] = w2c2
            # stage-3 rhs chunks
            wst = stage.tile([128, 1600], f32, tag=stg, bufs=1)
            dma.dma_start(wst[:, 0:8], wt[f"w3_{tag}"][0:128])
            dma.dma_start(wst[:72, 8:16], wt[f"w3_{tag}"][128:200])
            dma.dma_start(wst[:, 16:24], wt[f"w3_{tag}"][200:328])
            dma.dma_start(wst[:72, 24:32], wt[f"w3_{tag}"][328:400])
            w3c1 = wpool.tile([128, PMX], f16, name=f"w3c1_{tag}")
            cast(csw, w3c1, wst[:, 0:8])
            w3c2 = wpool.tile([74, PMX], f16, name=f"w3c2_{tag}")
            cast(csw, w3c2[:72], wst[:72, 8:16])
            yield
            dma.dma_start(w3c2[72:73, :], wt[f"b3h_{tag}"])
            dma.dma_start(w3c2[73:74, :], wt[f"b3l_{tag}"])
            w3c3 = wpool.tile([128, PMX], f16, name=f"w3c3_{tag}")
            cast(csw, w3c3, wst[:, 16:24])
            w3c4 = wpool.tile([72, PMX], f16, name=f"w3c4_{tag}")
            cast(csw, w3c4, wst[:72, 24:32])
            yield
            W[f"w3_{tag}"] = (w3c1, w3c2, w3c3, w3c4)

        def load_lstm(dma, cse):
            # LSTM weights as fp16 rhs [kn, 800] over rows = cat(es, ee);
            # es-c3 chunk carries b_lstm as ones-row 16
            ih_chunks = []
            for (k0, kn) in [(0, 128), (128, 128), (256, 128), (384, 16),
                             (400, 128), (528, 128), (656, 128), (784, 16)]:
                wst = stage.tile([128, 1600], f32, tag="stg", bufs=1)
                dma.dma_start(wst[:kn, :800], wt["w_ih"][k0:k0 + kn])
                rows = kn + 1 if k0 == 384 else kn
                t = wpool.tile([rows, 4 * H], f16, name=f"wih16_{k0}")
                if k0 == 384:
                    dma.dma_start(wst[16:17, :800], wt["b_lstm"])
                cast(cse, t[:rows], wst[:rows, :800])
                ih_chunks.append((k0, kn, t))
                yield
            # mlp fp16 rhs [kn, 200]; c2 carries b_mlp as ones-row 72
            wst = stage.tile([128, 1600], f32, tag="stg", bufs=1)
            dma.dma_start(wst[:, :H], wt["w_mlp"][0:128])
            wmlpc1 = wpool.tile([128, H], f16, name="wmlp16c1")
            cast(cse, wmlpc1, wst[:, :H])
            yield
            wst = stage.tile([128, 1600], f32, tag="stg", bufs=1)
            dma.dma_start(wst[:72, :H], wt["w_mlp"][128:200])
            dma.dma_start(wst[72:73, :H], wt["b_mlp"])
            wmlpc2 = wpool.tile([73, H], f16, name="wmlp16c2")
            cast(cse, wmlpc2, wst[:73, :H])
            W["ih_chunks"] = ih_chunks
            W["wmlp"] = (wmlpc1, wmlpc2)
            yield

        # ---------------- E load ----------------
        E16 = []
        E16c3 = []

        def load_E(b, dma, defer=None):
            et = epool.tile([128, 3, L], f16, name=f"E16_{b}")
            ec3 = epool.tile([18, L], f16, name=f"E16c3_{b}")
            nc.gpsimd.memset(ec3, 1.0)
            HL = L // 2
            for h in range(2):
                hs = slice(h * HL, (h + 1) * HL)
                est = stage.tile([128, 3, HL], f32, tag="estg", bufs=2)
                for c in range(3):
                    dma.dma_start(est[:, c, :], enc[b, 128 * c:128 * (c + 1), hs])
                est2 = stage.tile([16, HL], f32, tag="estg2", bufs=1)
                dma.dma_start(est2, enc[b, 384:400, hs])
                if defer is None:
                    nc.vector.tensor_copy(et[:, :, hs], est)
                    nc.vector.tensor_copy(ec3[:16, hs], est2)
                else:
                    defer.append(lambda et=et, est=est, hs=hs:
                                 nc.vector.tensor_copy(et[:, :, hs], est))
                    defer.append(lambda ec3=ec3, est2=est2, hs=hs:
                                 nc.vector.tensor_copy(ec3[:16, hs], est2))
            E16.append(et)
            E16c3.append(ec3)

        # ---------------- persistent state tiles ----------------
        hxT = [keep.tile([128, BLOC], f16, name="hxT16_0"),
               keep.tile([72, BLOC], f16, name="hxT16_1")]
        rT1 = keep.tile([128, BLOC], f16, name="rT1")
        rT2 = keep.tile([73, BLOC], f16, name="rT2")
        nc.gpsimd.memset(rT2, 1.0)                 # row 72 = b1 fold row
        h0T1 = keep.tile([128, BLOC], f16, name="h0T1")
        h0T2 = keep.tile([73, BLOC], f16, name="h0T2")
        nc.gpsimd.memset(h0T2, 1.0)                # row 72 = b_mlp fold row

        m1_slots, m2_slots, m1c2_slots = [], [], []
        for i in range(6):
            m1_slots.append(keep.tile([128, H], f16, name=f"m1_slot{i}"))
            m2_slots.append(keep.tile([128, H], f16, name=f"m2_slot{i}"))
            t = keep.tile([74, 128], f16, name=f"m1c2_slot{i}")
            nc.gpsimd.memset(t, 1.0)
            m1c2_slots.append(t)

        # ---------------- helpers ----------------
        def cat_chunks(tag, with_hx):
            """(lhsT [kn,4] fp16, wd16 [kn,H]) pairs for r = tanh(cat @ wd)."""
            ops = []
            for (k0, kn, wtile) in W[f"wd_{tag}"]:
                if k0 < 200:
                    if not with_hx:
                        continue
                    lhsT = hxT[0] if k0 == 0 else hxT[1]
                elif k0 < 600:
                    c = (k0 - 200) // 128
                    lhsT = es16[:, c, :] if c < 3 else es16c3[:16]
                else:
                    c = (k0 - 600) // 128
                    lhsT = ee16[:, c, :] if c < 3 else ee16c3
                ops.append((lhsT, wtile))
            return ops

        def r_matmul(tag, with_hx):
            """r_row = tanh(cat @ wd) -> [4, H] fp16 sbuf."""
            ops = cat_chunks(tag, with_hx)
            pt = ps_d.tile([128, 512], f32, tag="ps_ser")
            for i, (lhsT, rhs) in enumerate(ops):
                nc.tensor.matmul(pt[:BLOC, :H], lhsT, rhs,
                                 start=(i == 0), stop=(i == len(ops) - 1))
            r_row = work.tile([BLOC, H], f16, tag="r_row", bufs=1)
            nc.scalar.activation(r_row, pt[:BLOC, :H], AF.Tanh)
            return r_row

        def r_transpose(r_row):
            ptr = ps_tr.tile([128, 128], f16, tag="ps_tr")
            nc.tensor.transpose(ptr[:, :BLOC], r_row[:, 0:128], ident[:BLOC, :BLOC])
            nc.vector.tensor_copy(rT1, ptr[:, :BLOC])
            ptr2 = ps_tr.tile([128, 128], f16, tag="ps_tr")
            nc.tensor.transpose(ptr2[:72, :BLOC], r_row[:, 128:200], ident[:BLOC, :BLOC])
            nc.vector.tensor_copy(rT2[:72], ptr2[:72, :BLOC])

        def o_rows(tag, r_row, save=False):
            """o = r@w1r + b1 (f32 psum) -> fp16 hi/lo rows [BLOC, 1600]."""
            if save:
                o_tmp = work.tile([BLOC, H * PMX], f32, tag="o_tmp", bufs=1)
            r_transpose(r_row)
            w1rc1, w1rc2 = W[f"w1r_{tag}"]
            oh = single.tile([BLOC, H * PMX], f16, tag="oh")
            ol = single.tile([BLOC, H * PMX], f16, tag="ol")
            for (n0, nn) in NCH:
                pt = ps_d.tile([128, 512], f32, tag="ps_ser")
                nc.tensor.matmul(pt[:BLOC, :nn], rT1, w1rc1[:, n0:n0 + nn], start=True, stop=False)
                nc.tensor.matmul(pt[:BLOC, :nn], rT2, w1rc2[:, n0:n0 + nn], start=False, stop=True)
                nc.scalar.activation(oh[:, n0:n0 + nn], pt[:BLOC, :nn], AF.Copy)
                nc.vector.tensor_tensor(ol[:, n0:n0 + nn], pt[:BLOC, :nn], oh[:, n0:n0 + nn], OP.subtract)
                if save:
                    nc.vector.tensor_copy(o_tmp[:, n0:n0 + nn], pt[:BLOC, :nn])
            if save:
                nc.gpsimd.dma_start(o_save_d[0 if tag == "s" else 1], o_tmp)
            return oh, ol

        def delta_o_rows(tag, r_row):
            """d16 [4,1600] f16 = (r@w1r+b1) - o_save (sweep-1/2's o psum)."""
            o_tmp = work.tile([BLOC, H * PMX], f32, tag="o_tmp", bufs=1)
            nc.gpsimd.dma_start(o_tmp, o_save_d[0 if tag == "s" else 1])
            r_transpose(r_row)
            w1rc1, w1rc2 = W[f"w1r_{tag}"]
            d16 = work.tile([BLOC, H * PMX], f16, tag="d16", bufs=1)
            for (n0, nn) in NCH:
                pt = ps_d.tile([128, 512], f32, tag="ps_ser")
                nc.tensor.matmul(pt[:BLOC, :nn], rT1, w1rc1[:, n0:n0 + nn], start=True, stop=False)
                nc.tensor.matmul(pt[:BLOC, :nn], rT2, w1rc2[:, n0:n0 + nn], start=False, stop=True)
                nc.vector.tensor_tensor(d16[:, n0:n0 + nn], pt[:BLOC, :nn],
                                        o_tmp[:, n0:n0 + nn], OP.subtract)
            return d16

        def bcast_do(d16, b):
            """broadcast d16 row b to a [128,1600] f16 tile via PE row-select."""
            bc = work.tile([128, H * PMX], f16, tag="dbc", bufs=2)
            for (n0, nn) in NCH:
                pb = ps_a.tile([128, 512], f32, tag="ps_s1")
                nc.tensor.matmul(pb[:, :nn], selmat[:, 128 * b:128 * (b + 1)],
                                 d16[:, n0:n0 + nn], start=True, stop=True)
                nc.scalar.activation(bc[:, n0:n0 + nn], pb[:, :nn], AF.Copy)
            return bc

        def lsm_row(s4row, b, out_dram):
            """log_softmax of one S4 row -> out_dram[b]."""
            gmax = work.tile([1, 1], f32, tag="gmax", bufs=4)
            nc.vector.tensor_reduce(gmax, s4row, axis=AX.X, op=OP.max)
            negm = work.tile([1, 1], f32, tag="negm", bufs=4)
            nc.vector.tensor_scalar_mul(negm, gmax, -1.0)
            e4 = work.tile([1, L], f32, tag="rowtmp", bufs=1)
            sume = work.tile([1, 1], f32, tag="sume", bufs=4)
            nc.scalar.activation(e4, s4row, AF.Exp, bias=negm[:, 0:1], accum_out=sume)
            lnz = work.tile([1, 1], f32, tag="lnz", bufs=4)
            nc.scalar.activation(lnz, sume, AF.Ln)
            lse = work.tile([1, 1], f32, tag="lse", bufs=4)
            nc.vector.tensor_tensor(lse, gmax, lnz, OP.add)
            lp4 = work.tile([1, L], f32, tag="rowtmp", bufs=1)
            nc.vector.tensor_scalar(lp4, s4row, lse[:, 0:1], None, op0=OP.subtract)
            dma_engines[b % 3].dma_start(out_dram[b:b + 1, :], lp4)

        def argmax_gather_b(s4row, b, dstbig, dstc3):
            mx = work.tile([1, 8], f32, tag="mx", bufs=4)
            idx = work.tile([1, 8], u32, tag="idx", bufs=4)
            nc.vector.max(out=mx, in_=s4row)
            nc.vector.max_index(out=idx, in_max=mx, in_values=s4row)
            reg = nc.values_load(idx[0:1, 0:1], min_val=0, max_val=L - 1,
                                 skip_runtime_bounds_check=True)
            dma_engines[(2 * b) % 3].dma_start(
                dstbig[:, :, b:b + 1], E16[b][:, :, ds(reg, 1)])
            dma_engines[(2 * b + 1) % 3].dma_start(
                dstc3[:16, b:b + 1], E16c3[b][:16, ds(reg, 1)])

        # shared g2/g3 stages of the scoring pipeline
        def make_g23(tag, s4rows, st, strips, s2_sbuf=False):
            w2c1 = W[f"w2c1_{tag}"]
            w2c2 = W[f"w2c2_{tag}"]
            w3c1, w3c2, w3c3, w3c4 = W[f"w3_{tag}"]

            def g2(i):
                m1 = st[i]["m1"]
                pt1 = ps_tr.tile([128, 128], f16, tag="ps_tr")
                nc.tensor.transpose(pt1, m1[:, 0:128], ident)
                m1c1 = work.tile([128, 128], f16, tag="m1c1", bufs=4)
                nc.scalar.activation(m1c1, pt1, AF.Copy)
                pt2 = ps_tr.tile([128, 128], f16, tag="ps_tr")
                nc.tensor.transpose(pt2[:72], m1[:, 128:200], ident)
                m1c2 = m1c2_slots[i % 6]
                nc.scalar.activation(m1c2[:72], pt2[:72], AF.Copy)
                m2 = m2_slots[i % 6]
                if s2_sbuf:
                    # drain stage-2 psum via scalar fp16 copies; maxpool from
                    # SBUF on vector (round-then-max == max-then-round)
                    s2s = work.tile([128, H * PMX], f16, tag="s2s", bufs=2)
                    for ni, (n0, nn) in enumerate(NCH):
                        pb = ps_b.tile([128, 512], f32, tag="ps_s2")
                        nc.tensor.matmul(pb[:, :nn], m1c1, w2c1[:, n0:n0 + nn], start=True, stop=False)
                        nc.tensor.matmul(pb[:, :nn], m1c2, w2c2[:, n0:n0 + nn], start=False, stop=True)
                        nc.scalar.activation(s2s[:, n0:n0 + nn], pb[:, :nn], AF.Copy)
                    nc.vector.tensor_reduce(
                        m2, s2s.rearrange("p (h q) -> p h q", q=PMX),
                        axis=AX.X, op=OP.max)
                else:
                    for ni, (n0, nn) in enumerate(NCH):
                        pb = ps_b.tile([128, 512], f32, tag="ps_s2")
                        nc.tensor.matmul(pb[:, :nn], m1c1, w2c1[:, n0:n0 + nn], start=True, stop=False)
                        nc.tensor.matmul(pb[:, :nn], m1c2, w2c2[:, n0:n0 + nn], start=False, stop=True)
                        h0, hn = HSL[ni]
                        nc.vector.tensor_reduce(
                            m2[:, h0:h0 + hn],
                            pb[:, :nn].rearrange("p (h q) -> p h q", q=PMX),
                            axis=AX.X, op=OP.max)
                st[i]["m1c1"] = m1c1
                st[i]["m1c2"] = m1c2
                st[i]["m2"] = m2

            def g3(i):
                b, lt = TILES[i]
                ntb = NTB[b]
                m2 = st[i]["m2"]
                pt3 = ps_tr.tile([128, 128], f16, tag="ps_tr")
                nc.tensor.transpose(pt3, m2[:, 0:128], ident)
                m2c1 = work.tile([128, 128], f16, tag="m2c1", bufs=4)
                nc.scalar.activation(m2c1, pt3, AF.Copy)
                pt4 = ps_tr.tile([128, 128], f16, tag="ps_tr")
                nc.tensor.transpose(pt4[:72], m2[:, 128:200], ident)
                m2c2 = work.tile([72, 128], f16, tag="m2c2", bufs=4)
                nc.scalar.activation(m2c2, pt4[:72], AF.Copy)
                if lt == 0:
                    strips[b] = ps_d.tile([128, 8 * NLT], f32, tag="ps_ser", name="s3strip")
                psl = strips[b][:, 8 * lt:8 * (lt + 1)]
                nc.tensor.matmul(psl, st[i]["m1c1"], w3c1, start=True, stop=False)
                nc.tensor.matmul(psl, st[i]["m1c2"], w3c2, start=False, stop=False)
                nc.tensor.matmul(psl, m2c1, w3c3, start=False, stop=False)
                nc.tensor.matmul(psl, m2c2, w3c4, start=False, stop=True)
                st[i].clear()
                if lt == ntb - 1:
                    Sb = work.tile([128, NLT], f32, tag="Sb")
                    nc.vector.tensor_reduce(Sb[:, :ntb],
                                            strips[b][:, :8 * ntb].rearrange("p (t q) -> p t q", q=PMX),
                                            axis=AX.X, op=OP.max)
                    nc.vector.tensor_tensor(Sb[:, :ntb], Sb[:, :ntb],
                                            penT_sb[:, b * NLT:b * NLT + ntb],
                                            OP.subtract)
                    ptb = ps_tr.tile([NLT, 128], f32, tag="ps_tr")
                    nc.tensor.transpose(ptb[:ntb], Sb[:, :ntb], ident32)
                    s4stg = work.tile([NLT, 128], f32, tag="s4stg")
                    nc.scalar.activation(s4stg[:ntb], ptb[:ntb], AF.Copy)
                    s4row = work.tile([1, L], f32, tag="s4row", bufs=2)
                    if ntb < NLT:
                        nc.vector.memset(s4row[:, 128 * ntb:], -BIG)
                    dma_engines[b % 3].dma_start(s4row[:, :128 * ntb], s4stg[:ntb])
                    s4rows[b] = s4row
            return g2, g3

        def run_pipeline(g1, g2, g3, fill, batch_cb):
            # batch b's callback fires two tiles after its strip completes
            # (so the S4-row DMA has landed); last batch fires immediately.
            NT = NT_ALL
            last_of = {}
            for j, (b, lt) in enumerate(TILES):
                last_of[b] = j
            cb_at = {}
            for b in range(BLOC - 1):
                cb_at[min(last_of[b] + 2, NT - 1)] = b
            cb_at[NT - 1] = BLOC - 1
            for i in range(NT + 2):
                if i < NT:
                    g1(i)
                if 1 <= i < NT + 1:
                    g2(i - 1)
                if 2 <= i:
                    j = i - 2
                    g3(j)
                    if batch_cb is not None and j in cb_at:
                        batch_cb(cb_at[j])
                if fill is not None:
                    next(fill, None)
            if fill is not None:
                for _ in fill:
                    pass

        def score_sweep(tag, s4rows, oh, ol, fill=None, batch_cb=None, a_out=None):
            """Full maxout scoring sweep; optionally spills stage-1 psum (fp16)."""
            w1m = W[f"w1m_{tag}"]
            c3pair = W[f"c3_{tag}"]
            st = [dict() for _ in range(NT_ALL)]
            strips = {}
            g2, g3 = make_g23(tag, s4rows, st, strips)

            def g1(i):
                b, lt = TILES[i]
                c3rhs = c3pair[b % 2]
                if lt == 0:
                    nc.sync.dma_start(c3rhs[16:17, :], oh[b:b + 1, :])
                    nc.sync.dma_start(c3rhs[17:18, :], ol[b:b + 1, :])
                lsl = slice(128 * lt, 128 * (lt + 1))
                m1 = m1_slots[i % 6]
                if a_out is not None:
                    a_w = work.tile([128, H * PMX], f16, tag="t_add", bufs=1)
                for ni, (n0, nn) in enumerate(NCH):
                    pa = ps_a.tile([128, 512], f32, tag="ps_s1")
                    for c in range(3):
                        nc.tensor.matmul(pa[:, :nn], E16[b][:, c, lsl], w1m[:, c, n0:n0 + nn],
                                         start=(c == 0), stop=False)
                    nc.tensor.matmul(pa[:, :nn], E16c3[b][:, lsl], c3rhs[:, n0:n0 + nn],
                                     start=False, stop=True)
                    h0, hn = HSL[ni]
                    nc.vector.tensor_reduce(
                        m1[:, h0:h0 + hn],
                        pa[:, :nn].rearrange("p (h q) -> p h q", q=PMX),
                        axis=AX.X, op=OP.max)
                    if a_out is not None:
                        nc.scalar.activation(a_w[:, n0:n0 + nn], pa[:, :nn], AF.Copy)
                if a_out is not None:
                    dma_engines[(b + lt) % 3].dma_start(a_out[b, lt], a_w)
                st[i]["m1"] = m1

            run_pipeline(g1, g2, g3, fill, batch_cb)

        def reuse_sweep(tag, s4rows, d16, a_in, batch_cb=None):
            """Scoring sweep: stage-1 = vector add of the DRAM A-tile and the
            per-batch delta-o broadcast, then an SBUF fp16 maxpool."""
            st = [dict() for _ in range(NT_ALL)]
            strips = {}
            a_tiles = {}
            dbc = {0: bcast_do(d16, 0), 1: bcast_do(d16, 1)}
            g2, g3 = make_g23(tag, s4rows, st, strips, s2_sbuf=True)

            def fetch(j):
                if j >= NT_ALL:
                    return
                b, lt = TILES[j]
                at = stage.tile([128, H * PMX], f16, tag="ar", bufs=2)
                dma_engines[j % 3].dma_start(at, a_in[b, lt])
                a_tiles[j] = at

            fetch(0)
            fetch(1)

            def g1(i):
                b, lt = TILES[i]
                fetch(i + 2)
                # broadcast the next batch's delta-o two tiles early
                if lt == NTB[b] - 2 and b + 2 < BLOC:
                    dbc[b + 2] = bcast_do(d16, b + 2)
                at = a_tiles.pop(i)
                t = work.tile([128, H * PMX], f16, tag="t_add", bufs=1)
                nc.vector.tensor_tensor(t, at, dbc[b], OP.add)
                m1 = m1_slots[i % 6]
                nc.vector.tensor_reduce(
                    m1, t.rearrange("p (h q) -> p h q", q=PMX),
                    axis=AX.X, op=OP.max)
                st[i]["m1"] = m1

            run_pipeline(g1, g2, g3, None, batch_cb)

        def lstm_update():
            """hx via LSTM cell with hx0=cx0=0 (f-gate and w_hh drop out)."""
            pt_i = ps_d.tile([128, 512], f32, tag="ps_ser")
            pt_go = ps_d.tile([128, 512], f32, tag="ps_ser")
            lhs_for = []
            for (k0, kn, wtile) in W["ih_chunks"]:
                if k0 < 400:
                    c = k0 // 128
                    lhsT = es16[:, c, :] if c < 3 else es16c3  # [17,4] w/ ones
                else:
                    c = (k0 - 400) // 128
                    lhsT = ee16[:, c, :] if c < 3 else ee16c3
                lhs_for.append((lhsT, wtile, kn + (1 if k0 == 384 else 0)))
            n = len(lhs_for)
            for i, (lhsT, wtile, rows) in enumerate(lhs_for):
                nc.tensor.matmul(pt_i[:BLOC, :H], lhsT, wtile[:rows, 0:H],
                                 start=(i == 0), stop=(i == n - 1))
            for i, (lhsT, wtile, rows) in enumerate(lhs_for):
                nc.tensor.matmul(pt_go[:BLOC, :2 * H], lhsT, wtile[:rows, 2 * H:4 * H],
                                 start=(i == 0), stop=(i == n - 1))
            ig = work.tile([BLOC, H], f32, tag="ig", bufs=1)
            nc.scalar.activation(ig, pt_i[:BLOC, :H], AF.Sigmoid)
            gg = work.tile([BLOC, H], f32, tag="gg", bufs=1)
            nc.scalar.activation(gg, pt_go[:BLOC, 0:H], AF.Tanh)
            og = work.tile([BLOC, H], f32, tag="og", bufs=1)
            nc.scalar.activation(og, pt_go[:BLOC, H:2 * H], AF.Sigmoid)
            cx = work.tile([BLOC, H], f32, tag="cx", bufs=1)
            nc.vector.tensor_tensor(cx, ig, gg, OP.mult)
            tcx = work.tile([BLOC, H], f32, tag="tcx", bufs=1)
            nc.scalar.activation(tcx, cx, AF.Tanh)
            h0 = work.tile([BLOC, H], f16, tag="h0", bufs=1)
            nc.vector.tensor_tensor(h0, og, tcx, OP.mult)
            ptr = ps_tr.tile([128, 128], f16, tag="ps_tr")
            nc.tensor.transpose(ptr[:, :BLOC], h0[:, 0:128], ident[:BLOC, :BLOC])
            nc.vector.tensor_copy(h0T1, ptr[:, :BLOC])
            ptr2 = ps_tr.tile([128, 128], f16, tag="ps_tr")
            nc.tensor.transpose(ptr2[:72, :BLOC], h0[:, 128:200], ident[:BLOC, :BLOC])
            nc.vector.tensor_copy(h0T2[:72], ptr2[:72, :BLOC])
            pt = ps_d.tile([128, 512], f32, tag="ps_ser")
            wmlpc1, wmlpc2 = W["wmlp"]
            nc.tensor.matmul(pt[:BLOC, :H], h0T1, wmlpc1, start=True, stop=False)
            nc.tensor.matmul(pt[:BLOC, :H], h0T2, wmlpc2, start=False, stop=True)
            hx_row = work.tile([BLOC, H], f16, tag="hx_row", bufs=1)
            nc.scalar.activation(hx_row, pt[:BLOC, :H], AF.Copy)
            ptr3 = ps_tr.tile([128, 128], f16, tag="ps_tr")
            nc.tensor.transpose(ptr3[:, :BLOC], hx_row[:, 0:128], ident[:BLOC, :BLOC])
            nc.vector.tensor_copy(hxT[0], ptr3[:, :BLOC])
            ptr4 = ps_tr.tile([128, 128], f16, tag="ps_tr")
            nc.tensor.transpose(ptr4[:72, :BLOC], hx_row[:, 128:200], ident[:BLOC, :BLOC])
            nc.vector.tensor_copy(hxT[1], ptr4[:72, :BLOC])

        # ---------------- prologue: s-critical-path loads ----------------
        import itertools
        for _ in load_wd("s", nc.sync, nc.vector):
            pass
        for _ in load_w1r("s", nc.sync, nc.vector):
            pass
        load_E(0, nc.gpsimd)
        for _ in load_w1m("s", nc.sync, nc.scalar):
            pass
        edefer = []
        load_E(1, nc.gpsimd, edefer)
        load_E(2, nc.gpsimd, edefer)
        load_E(3, nc.gpsimd, edefer)

        def edefer_gen():
            for fn in edefer:
                fn()
                yield
        fill_steps = itertools.chain(
            edefer_gen(),
            load_wd("e", nc.sync, nc.scalar),
            load_w1r("e", nc.sync, nc.scalar),
            load_w1m("e", nc.sync, nc.scalar),
            load_scoring2("e", nc.sync, nc.scalar, "stg"),
            load_lstm(nc.sync, nc.scalar))

        # ---------------- the four passes ----------------
        r_row = r_matmul("s", with_hx=False)
        oh, ol = o_rows("s", r_row, save=REUSE)
        for _ in load_scoring2("s", nc.sync, nc.scalar, "stg"):
            pass
        rows1 = {}
        score_sweep("s", rows1, oh, ol, fill=fill_steps,
                    batch_cb=lambda b: argmax_gather_b(rows1[b], b, es16, es16c3),
                    a_out=a1d if REUSE else None)

        r_row = r_matmul("e", with_hx=False)
        oh, ol = o_rows("e", r_row, save=REUSE)
        rows2 = {}
        score_sweep("e", rows2, oh, ol,
                    batch_cb=lambda b: argmax_gather_b(rows2[b], b, ee16, ee16c3),
                    a_out=a2d if REUSE else None)

        lstm_update()

        rows3 = {}
        rows4 = {}

        def cb3(b):
            argmax_gather_b(rows3[b], b, es16, es16c3)
            if b < BLOC - 1:
                lsm_row(rows3[b], b, lp1)

        def cb4(b):
            if b < BLOC - 1:
                lsm_row(rows4[b], b, lp2)

        r_row = r_matmul("s", with_hx=True)
        if REUSE:
            d16 = delta_o_rows("s", r_row)
            reuse_sweep("s", rows3, d16, a1d, batch_cb=cb3)
            r_row = r_matmul("e", with_hx=True)
            d16 = delta_o_rows("e", r_row)
            lsm_row(rows3[BLOC - 1], BLOC - 1, lp1)
            reuse_sweep("e", rows4, d16, a2d, batch_cb=cb4)
        else:
            oh, ol = o_rows("s", r_row)
            score_sweep("s", rows3, oh, ol, batch_cb=cb3)
            r_row = r_matmul("e", with_hx=True)
            oh, ol = o_rows("e", r_row)
            lsm_row(rows3[BLOC - 1], BLOC - 1, lp1)
            score_sweep("e", rows4, oh, ol, batch_cb=cb4)
        lsm_row(rows4[BLOC - 1], BLOC - 1, lp2)

        stage_cm.__exit__(None, None, None)

    nc.compile()
    return nc


def get_program(inputs):
    lens = np.asarray(inputs["passage_lens"]).astype(np.int64)
    nt_b = tuple(int(min(NLT, (l + 127) // 128)) for l in lens)
    if _cache.get("key") != nt_b:
        _cache["nc"] = _build_program(nt_b)
        _cache["key"] = nt_b
    return _cache["nc"]


def _split16(x):
    hi = np.asarray(x, np.float32).astype(np.float16)
    lo = (np.asarray(x, np.float32) - hi.astype(np.float32)).astype(np.float16)
    return hi, lo


def make_in_maps(inputs):
    """Per-core input maps: batch shard + trivial host prep (mask, bias splits)."""
    inputs = {k: np.asarray(v) for k, v in inputs.items()}
    enc = np.ascontiguousarray(inputs["encoding_matrix"], dtype=np.float32)
    lens = np.asarray(inputs["passage_lens"]).astype(np.int64)
    pen_full = np.where(np.arange(L)[None, :] < lens[:, None],
                        np.float32(0.0), BIG).astype(np.float32)

    shared = {}
    for tag in ("s", "e"):
        shared[f"w1_{tag}"] = np.ascontiguousarray(inputs[f"w1_{tag}"], np.float32)
        shared[f"b1_{tag}"] = np.ascontiguousarray(inputs[f"b1_{tag}"], np.float32).reshape(1, -1)
        shared[f"w2_{tag}"] = np.ascontiguousarray(inputs[f"w2_{tag}"], np.float32)
        b2h, b2l = _split16(inputs[f"b2_{tag}"])
        shared[f"b2h_{tag}"] = b2h.reshape(1, -1)
        shared[f"b2l_{tag}"] = b2l.reshape(1, -1)
        shared[f"w3_{tag}"] = np.ascontiguousarray(inputs[f"w3_{tag}"], np.float32)
        b3h, b3l = _split16(inputs[f"b3_{tag}"])
        shared[f"b3h_{tag}"] = b3h.reshape(1, -1)
        shared[f"b3l_{tag}"] = b3l.reshape(1, -1)
        shared[f"wd_{tag}"] = np.ascontiguousarray(inputs[f"wd_{tag}"], np.float32)
    shared["w_ih"] = np.ascontiguousarray(inputs["w_ih"], np.float32)
    shared["b_lstm"] = np.ascontiguousarray(inputs["b_lstm"], np.float32).reshape(1, -1)
    shared["w_mlp"] = np.ascontiguousarray(inputs["w_mlp"], np.float32)
    shared["b_mlp"] = np.ascontiguousarray(inputs["b_mlp"], np.float32).reshape(1, -1)

    in_maps = []
    for core in range(NCORES):
        sl = slice(core * BLOC, (core + 1) * BLOC)
        m = dict(shared)
        m["enc"] = np.ascontiguousarray(enc[sl])
        m["selmat_d"] = np.kron(np.eye(BLOC, dtype=np.float16),
                                np.ones((1, 128), dtype=np.float16))
        pc = pen_full[sl].reshape(BLOC, NLT, 128)
        m["penT"] = np.ascontiguousarray(pc.transpose(2, 0, 1).reshape(128, BLOC * NLT))
        in_maps.append(m)
    return in_maps


def run_on_hw(inputs, trace=False):
    from concourse import bass_utils
    nc = get_program(inputs)
    in_maps = make_in_maps(inputs)
    res = bass_utils.run_bass_kernel_spmd(nc, in_maps, core_ids=list(range(NCORES)),
                                          trace=trace)
    lp1 = np.concatenate([res.results[c]["lp1"] for c in range(NCORES)], axis=0)
    lp2 = np.concatenate([res.results[c]["lp2"] for c in range(NCORES)], axis=0)
    return (np.asarray(lp1, np.float32), np.asarray(lp2, np.float32)), res


def kernel(**inputs):
    out, _ = run_on_hw(inputs, trace=False)
    return out


# revision 22
# speedup vs baseline: 1.3876x; 1.2194x over previous
"""Trainium2 Bass kernel for nn_Decoder_86921548137026.

Dynamic decoder: NITER=2 iterations of (maxout pointer scoring over L=1024
positions -> argmax -> gather -> LSTM cell), followed by log_softmax over the
final start/end scores.

Sharding: data-parallel over batch B=32 across 8 cores (4 batches/core),
weights replicated.

v3 changes vs v2 (934us):
  - prologue restructured: critical-path DMA order (wd -> w1r -> E b0 -> w1m
    -> w2/w3), E b1-3 on the gpsimd queue, r/o emitted early; sweep-1 starts
    ~15us instead of ~94us
  - penalty fold moved from S4-post to a per-batch f32 subtract on Sb before
    the transpose (bit-exact, removes it from the boundary path)
  - per-batch argmax/gather pipelined into the sweep (batch b resolved two
    tiles after its strip completes); per-batch log_softmax rows likewise
  - sweeps 3/4 reuse stage-1: sweep-1/2 stage-1 PSUM (A = E@W1 + o_fold) is
    rounded to fp16 and spilled to DRAM; the iter-2 sweeps replace the big
    stage-1 matmul with a vector add of the per-batch delta-o broadcast and
    an SBUF fp16 maxpool (validated offline: 0 argmax flips, rel 3.6e-4)
"""

import numpy as np

H = 200
PMX = 8
B = 32
L = 1024
BIG = np.float32(1e30)
NCORES = 8
BLOC = B // NCORES          # 4 batches per core
NLT = L // 128              # 8 l-tiles per batch
# stage-1/2 output channels = H*P = 1600, swept in PSUM-bank-sized chunks
NCH = [(0, 512), (512, 512), (1024, 512), (1536, 64)]
# h-slice of m1/m2 produced by each n-chunk (1600 = 200h * 8p, h-major)
HSL = [(0, 64), (64, 64), (128, 64), (192, 8)]
REUSE = False              # full sweeps everywhere: no A-spill HBM traffic

_cache = {}


def _build_program(nt_b):
    import contextlib
    import concourse.mybir as mybir
    import concourse.tile as tile
    from concourse import bacc
    from concourse.bass import ds
    from concourse.masks import make_identity

    f32 = mybir.dt.float32
    f16 = mybir.dt.float16
    u32 = mybir.dt.uint32
    AF = mybir.ActivationFunctionType
    OP = mybir.AluOpType
    AX = mybir.AxisListType

    nc = bacc.Bacc("TRN2", target_bir_lowering=False, debug=False,
                   enable_asserts=False, num_devices=NCORES)

    # ---------------- DRAM I/O ----------------
    enc = nc.dram_tensor("enc", [BLOC, 2 * H, L], f32, kind="ExternalInput").ap()
    penT = nc.dram_tensor("penT", [128, BLOC * NLT], f32, kind="ExternalInput").ap()
    selmat_d = nc.dram_tensor("selmat_d", [BLOC, BLOC * 128], f16, kind="ExternalInput").ap()
    wt = {}
    for tag in ("s", "e"):
        wt[f"w1_{tag}"] = nc.dram_tensor(f"w1_{tag}", [3 * H, H * PMX], f32, kind="ExternalInput").ap()
        wt[f"b1_{tag}"] = nc.dram_tensor(f"b1_{tag}", [1, H * PMX], f32, kind="ExternalInput").ap()
        wt[f"w2_{tag}"] = nc.dram_tensor(f"w2_{tag}", [H, H * PMX], f32, kind="ExternalInput").ap()
        wt[f"b2h_{tag}"] = nc.dram_tensor(f"b2h_{tag}", [1, H * PMX], f16, kind="ExternalInput").ap()
        wt[f"b2l_{tag}"] = nc.dram_tensor(f"b2l_{tag}", [1, H * PMX], f16, kind="ExternalInput").ap()
        wt[f"w3_{tag}"] = nc.dram_tensor(f"w3_{tag}", [2 * H, PMX], f32, kind="ExternalInput").ap()
        wt[f"b3h_{tag}"] = nc.dram_tensor(f"b3h_{tag}", [1, PMX], f16, kind="ExternalInput").ap()
        wt[f"b3l_{tag}"] = nc.dram_tensor(f"b3l_{tag}", [1, PMX], f16, kind="ExternalInput").ap()
        wt[f"wd_{tag}"] = nc.dram_tensor(f"wd_{tag}", [5 * H, H], f32, kind="ExternalInput").ap()
    wt["w_ih"] = nc.dram_tensor("w_ih", [4 * H, 4 * H], f32, kind="ExternalInput").ap()
    wt["b_lstm"] = nc.dram_tensor("b_lstm", [1, 4 * H], f32, kind="ExternalInput").ap()
    wt["w_mlp"] = nc.dram_tensor("w_mlp", [H, H], f32, kind="ExternalInput").ap()
    wt["b_mlp"] = nc.dram_tensor("b_mlp", [1, H], f32, kind="ExternalInput").ap()

    lp1 = nc.dram_tensor("lp1", [BLOC, L], f32, kind="ExternalOutput").ap()
    lp2 = nc.dram_tensor("lp2", [BLOC, L], f32, kind="ExternalOutput").ap()
    if REUSE:
        a1d = nc.dram_tensor("a1d", [BLOC, NLT, 128, H * PMX], f16, kind="Internal").ap()
        a2d = nc.dram_tensor("a2d", [BLOC, NLT, 128, H * PMX], f16, kind="Internal").ap()
        o_save_d = nc.dram_tensor("o_save_d", [2, BLOC, H * PMX], f32, kind="Internal").ap()

    # one SPMD program for all cores: batch-slot b uses the max tile count
    # over cores so every core's valid region is covered
    NTB = [max(nt_b[b * NCORES + c] for c in range(NCORES)) for b in range(BLOC)]
    TILES = [(b, lt) for b in range(BLOC) for lt in range(NTB[b])]
    NT_ALL = len(TILES)

    with tile.TileContext(nc) as tc, contextlib.ExitStack() as ctx:
        const = ctx.enter_context(tc.tile_pool(name="const", bufs=1))
        wpool = ctx.enter_context(tc.tile_pool(name="wpool", bufs=1))
        epool = ctx.enter_context(tc.tile_pool(name="epool", bufs=1))
        work = ctx.enter_context(tc.tile_pool(name="work", bufs=3))
        single = ctx.enter_context(tc.tile_pool(name="single", bufs=1))
        keep = ctx.enter_context(tc.tile_pool(name="keep", bufs=1))
        ps_a = ctx.enter_context(tc.tile_pool(name="ps_a", bufs=3, space="PSUM"))
        ps_b = ctx.enter_context(tc.tile_pool(name="ps_b", bufs=2, space="PSUM"))
        ps_tr = ctx.enter_context(tc.tile_pool(name="ps_tr", bufs=2, space="PSUM"))
        ps_d = ctx.enter_context(tc.tile_pool(name="ps_d", bufs=1, space="PSUM"))
        stage_cm = tc.tile_pool(name="stage", bufs=2)
        stage = stage_cm.__enter__()

        dma_engines = [nc.sync, nc.scalar, nc.gpsimd]

        # ---------------- constants ----------------
        ident = const.tile([128, 128], f16, name="ident")
        make_identity(nc, ident)
        ident32 = const.tile([128, 128], f32, name="ident32")
        make_identity(nc, ident32)
        # row-selector stationaries: selmat[:, 128b:128(b+1)] broadcasts d16
        # row b to all 128 output partitions (host-built constant)
        selmat = const.tile([BLOC, BLOC * 128], f16, name="selmat")
        nc.sync.dma_start(selmat, selmat_d)

        penT_sb = const.tile([128, BLOC * NLT], f32, name="penT_sb")
        nc.sync.dma_start(penT_sb, penT)

        # ---------------- es/ee init from E column 0 ----------------
        es16 = keep.tile([128, 3, BLOC], f16, name="es16")
        es16c3 = keep.tile([17, BLOC], f16, name="es16c3")
        nc.gpsimd.memset(es16c3, 1.0)              # row 16 = b_lstm fold row
        ee16 = keep.tile([128, 3, BLOC], f16, name="ee16")
        ee16c3 = keep.tile([16, BLOC], f16, name="ee16c3")
        escol = const.tile([128, 4, BLOC], f32, name="escol")
        for c in range(3):
            nc.sync.dma_start(escol[:, c, :],
                              enc[0:BLOC, 128 * c:128 * (c + 1), 0:1].rearrange("b p x -> p (b x)"))
        nc.sync.dma_start(escol[:16, 3, :],
                          enc[0:BLOC, 384:400, 0:1].rearrange("b p x -> p (b x)"))
        nc.vector.tensor_copy(es16, escol[:, 0:3, :])
        nc.vector.tensor_copy(es16c3[:16], escol[:16, 3, :])
        nc.vector.tensor_copy(ee16, es16)
        nc.vector.tensor_copy(ee16c3, es16c3[:16])

        # ---------------- weight / E loaders ----------------
        W = {}

        def cast(eng, dst, src):
            if eng is nc.scalar:
                nc.scalar.activation(dst, src, AF.Copy)
            else:
                eng.tensor_copy(dst, src)

        def load_wd(tag, dma, cse):
            # wd fp16 k-chunk tiles over cat(hx[0:200], es[200:600], ee[600:1000])
            wd_chunks = []
            for (k0, kn) in [(200, 128), (328, 128), (456, 128), (584, 16),
                             (600, 128), (728, 128), (856, 128), (984, 16),
                             (0, 128), (128, 72)]:
                wst = stage.tile([128, 800], f32, tag="stg8", bufs=2)
                dma.dma_start(wst[:kn, :H], wt[f"wd_{tag}"][k0:k0 + kn])
                t = wpool.tile([kn, H], f16, name=f"wd16_{tag}_{k0}")
                cast(cse, t, wst[:kn, :H])
                wd_chunks.append((k0, kn, t))
                yield
            W[f"wd_{tag}"] = wd_chunks

        def load_w1r(tag, dma, cse):
            # o-matmul rhs (fp16): rows 400..599 of w1, + b1 as ones-row 72 of c2
            wst = stage.tile([128, 1600], f32, tag="stg", bufs=1)
            dma.dma_start(wst, wt[f"w1_{tag}"][400:528])
            w1rc1 = wpool.tile([128, H * PMX], f16, name=f"w1r16c1_{tag}")
            cast(cse, w1rc1, wst)
            yield
            wst = stage.tile([128, 1600], f32, tag="stg", bufs=1)
            dma.dma_start(wst[:72], wt[f"w1_{tag}"][528:600])
            dma.dma_start(wst[72:73], wt[f"b1_{tag}"])
            w1rc2 = wpool.tile([73, H * PMX], f16, name=f"w1r16c2_{tag}")
            cast(cse, w1rc2, wst[:73])
            yield
            W[f"w1r_{tag}"] = (w1rc1, w1rc2)

        def load_w1m(tag, dma, csw):
            # stage-1 rhs rows 0..383 as fp16 [128, 3, 1600]
            w1m = wpool.tile([128, 3, H * PMX], f16, name=f"w1m_{tag}")
            for c in range(3):
                wst = stage.tile([128, 1600], f32, tag="stg", bufs=1)
                dma.dma_start(wst, wt[f"w1_{tag}"][128 * c:128 * (c + 1)])
                cast(csw, w1m[:, c, :], wst)
                yield
            W[f"w1m_{tag}"] = w1m
            # shared c3 rhs: rows 0..15 = W1[384:400] fp16, rows 16/17 = o hi/lo
            wst = stage.tile([128, 1600], f32, tag="stg", bufs=1)
            dma.dma_start(wst[:16], wt[f"w1_{tag}"][384:400])
            c3pair = []
            for pi in range(2):
                c3 = wpool.tile([18, H * PMX], f16, name=f"c3_{tag}_{pi}")
                cast(csw, c3[:16], wst[:16])
                c3pair.append(c3)
            yield
            W[f"c3_{tag}"] = c3pair

        def load_scoring2(tag, dma, csw, stg):
            # stage-2 rhs
            w2c1 = wpool.tile([128, H * PMX], f16, name=f"w2c1_{tag}")
            wst = stage.tile([128, 1600], f32, tag=stg, bufs=1)
            dma.dma_start(wst, wt[f"w2_{tag}"][:128])
            cast(csw, w2c1, wst)
            yield
            W[f"w2c1_{tag}"] = w2c1
            w2c2 = wpool.tile([74, H * PMX], f16, name=f"w2c2_{tag}")
            wst = stage.tile([128, 1600], f32, tag=stg, bufs=1)
            dma.dma_start(wst[:72], wt[f"w2_{tag}"][128:200])
            cast(csw, w2c2[:72], wst[:72])
            dma.dma_start(w2c2[72:73, :], wt[f"b2h_{tag}"])
            dma.dma_start(w2c2[73:74, :], wt[f"b2l_{tag}"])
            yield
            W[f"w2c2_{tag}"] = w2c2
            # stage-3 rhs chunks
            wst = stage.tile([128, 1600], f32, tag=stg, bufs=1)
            dma.dma_start(wst[:, 0:8], wt[f"w3_{tag}"][0:128])
            dma.dma_start(wst[:72, 8:16], wt[f"w3_{tag}"][128:200])
            dma.dma_start(wst[:, 16:24], wt[f"w3_{tag}"][200:328])
            dma.dma_start(wst[:72, 24:32], wt[f"w3_{tag}"][328:400])
            w3c1 = wpool.tile([128, PMX], f16, name=f"w3c1_{tag}")
            cast(csw, w3c1, wst[:, 0:8])
            w3c2 = wpool.tile([74, PMX], f16, name=f"w3c2_{tag}")
            cast(csw, w3c2[:72], wst[:72, 8:16])
            yield
            dma.dma_start(w3c2[72:73, :], wt[f"b3h_{tag}"])
            dma.dma_start(w3c2[73:74, :], wt[f"b3l_{tag}"])
            w3c3 = wpool.tile([128, PMX], f16, name=f"w3c3_{tag}")
            cast(csw, w3c3, wst[:, 16:24])
            w3c4 = wpool.tile([72, PMX], f16, name=f"w3c4_{tag}")
            cast(csw, w3c4, wst[:72, 24:32])
            yield
            W[f"w3_{tag}"] = (w3c1, w3c2, w3c3, w3c4)

        def load_lstm(dma, cse):
            # LSTM weights as fp16 rhs [kn, 800] over rows = cat(es, ee);
            # es-c3 chunk carries b_lstm as ones-row 16
            ih_chunks = []
            for (k0, kn) in [(0, 128), (128, 128), (256, 128), (384, 16),
                             (400, 128), (528, 128), (656, 128), (784, 16)]:
                wst = stage.tile([128, 1600], f32, tag="stg", bufs=1)
                dma.dma_start(wst[:kn, :800], wt["w_ih"][k0:k0 + kn])
                rows = kn + 1 if k0 == 384 else kn
                t = wpool.tile([rows, 4 * H], f16, name=f"wih16_{k0}")
                if k0 == 384:
                    dma.dma_start(wst[16:17, :800], wt["b_lstm"])
                cast(cse, t[:rows], wst[:rows, :800])
                ih_chunks.append((k0, kn, t))
                yield
            # mlp fp16 rhs [kn, 200]; c2 carries b_mlp as ones-row 72
            wst = stage.tile([128, 1600], f32, tag="stg", bufs=1)
            dma.dma_start(wst[:, :H], wt["w_mlp"][0:128])
            wmlpc1 = wpool.tile([128, H], f16, name="wmlp16c1")
            cast(cse, wmlpc1, wst[:, :H])
            yield
            wst = stage.tile([128, 1600], f32, tag="stg", bufs=1)
            dma.dma_start(wst[:72, :H], wt["w_mlp"][128:200])
            dma.dma_start(wst[72:73, :H], wt["b_mlp"])
            wmlpc2 = wpool.tile([73, H], f16, name="wmlp16c2")
            cast(cse, wmlpc2, wst[:73, :H])
            W["ih_chunks"] = ih_chunks
            W["wmlp"] = (wmlpc1, wmlpc2)
            yield

        # ---------------- E load ----------------
        E16 = []
        E16c3 = []

        def load_E(b, dma, defer=None):
            et = epool.tile([128, 3, L], f16, name=f"E16_{b}")
            ec3 = epool.tile([18, L], f16, name=f"E16c3_{b}")
            nc.gpsimd.memset(ec3, 1.0)
            HL = L // 2
            for h in range(2):
                hs = slice(h * HL, (h + 1) * HL)
                est = stage.tile([128, 3, HL], f32, tag="estg", bufs=2)
                for c in range(3):
                    dma.dma_start(est[:, c, :], enc[b, 128 * c:128 * (c + 1), hs])
                est2 = stage.tile([16, HL], f32, tag="estg2", bufs=1)
                dma.dma_start(est2, enc[b, 384:400, hs])
                if defer is None:
                    nc.vector.tensor_copy(et[:, :, hs], est)
                    nc.vector.tensor_copy(ec3[:16, hs], est2)
                else:
                    defer.append(lambda et=et, est=est, hs=hs:
                                 nc.vector.tensor_copy(et[:, :, hs], est))
                    defer.append(lambda ec3=ec3, est2=est2, hs=hs:
                                 nc.vector.tensor_copy(ec3[:16, hs], est2))
            E16.append(et)
            E16c3.append(ec3)

        # ---------------- persistent state tiles ----------------
        hxT = [keep.tile([128, BLOC], f16, name="hxT16_0"),
               keep.tile([72, BLOC], f16, name="hxT16_1")]
        rT1 = keep.tile([128, BLOC], f16, name="rT1")
        rT2 = keep.tile([73, BLOC], f16, name="rT2")
        nc.gpsimd.memset(rT2, 1.0)                 # row 72 = b1 fold row
        h0T1 = keep.tile([128, BLOC], f16, name="h0T1")
        h0T2 = keep.tile([73, BLOC], f16, name="h0T2")
        nc.gpsimd.memset(h0T2, 1.0)                # row 72 = b_mlp fold row

        m1_slots, m2_slots, m1c2_slots = [], [], []
        for i in range(6):
            m1_slots.append(keep.tile([128, H], f16, name=f"m1_slot{i}"))
            m2_slots.append(keep.tile([128, H], f16, name=f"m2_slot{i}"))
            t = keep.tile([74, 128], f16, name=f"m1c2_slot{i}")
            nc.gpsimd.memset(t, 1.0)
            m1c2_slots.append(t)

        # ---------------- helpers ----------------
        def cat_chunks(tag, with_hx):
            """(lhsT [kn,4] fp16, wd16 [kn,H]) pairs for r = tanh(cat @ wd)."""
            ops = []
            for (k0, kn, wtile) in W[f"wd_{tag}"]:
                if k0 < 200:
                    if not with_hx:
                        continue
                    lhsT = hxT[0] if k0 == 0 else hxT[1]
                elif k0 < 600:
                    c = (k0 - 200) // 128
                    lhsT = es16[:, c, :] if c < 3 else es16c3[:16]
                else:
                    c = (k0 - 600) // 128
                    lhsT = ee16[:, c, :] if c < 3 else ee16c3
                ops.append((lhsT, wtile))
            return ops

        def r_matmul(tag, with_hx):
            """r_row = tanh(cat @ wd) -> [4, H] fp16 sbuf."""
            ops = cat_chunks(tag, with_hx)
            pt = ps_d.tile([128, 512], f32, tag="ps_ser")
            for i, (lhsT, rhs) in enumerate(ops):
                nc.tensor.matmul(pt[:BLOC, :H], lhsT, rhs,
                                 start=(i == 0), stop=(i == len(ops) - 1))
            r_row = work.tile([BLOC, H], f16, tag="r_row", bufs=1)
            nc.scalar.activation(r_row, pt[:BLOC, :H], AF.Tanh)
            return r_row

        def r_transpose(r_row):
            ptr = ps_tr.tile([128, 128], f16, tag="ps_tr")
            nc.tensor.transpose(ptr[:, :BLOC], r_row[:, 0:128], ident[:BLOC, :BLOC])
            nc.vector.tensor_copy(rT1, ptr[:, :BLOC])
            ptr2 = ps_tr.tile([128, 128], f16, tag="ps_tr")
            nc.tensor.transpose(ptr2[:72, :BLOC], r_row[:, 128:200], ident[:BLOC, :BLOC])
            nc.vector.tensor_copy(rT2[:72], ptr2[:72, :BLOC])

        def o_rows(tag, r_row, save=False):
            """o = r@w1r + b1 (f32 psum) -> fp16 hi/lo rows [BLOC, 1600]."""
            if save:
                o_tmp = work.tile([BLOC, H * PMX], f32, tag="o_tmp", bufs=1)
            r_transpose(r_row)
            w1rc1, w1rc2 = W[f"w1r_{tag}"]
            oh = single.tile([BLOC, H * PMX], f16, tag="oh")
            ol = single.tile([BLOC, H * PMX], f16, tag="ol")
            for (n0, nn) in NCH:
                pt = ps_d.tile([128, 512], f32, tag="ps_ser")
                nc.tensor.matmul(pt[:BLOC, :nn], rT1, w1rc1[:, n0:n0 + nn], start=True, stop=False)
                nc.tensor.matmul(pt[:BLOC, :nn], rT2, w1rc2[:, n0:n0 + nn], start=False, stop=True)
                nc.scalar.activation(oh[:, n0:n0 + nn], pt[:BLOC, :nn], AF.Copy)
                nc.vector.tensor_tensor(ol[:, n0:n0 + nn], pt[:BLOC, :nn], oh[:, n0:n0 + nn], OP.subtract)
                if save:
                    nc.vector.tensor_copy(o_tmp[:, n0:n0 + nn], pt[:BLOC, :nn])
            if save:
                nc.gpsimd.dma_start(o_save_d[0 if tag == "s" else 1], o_tmp)
            return oh, ol

        def delta_o_rows(tag, r_row):
            """d16 [4,1600] f16 = (r@w1r+b1) - o_save (sweep-1/2's o psum)."""
            o_tmp = work.tile([BLOC, H * PMX], f32, tag="o_tmp", bufs=1)
            nc.gpsimd.dma_start(o_tmp, o_save_d[0 if tag == "s" else 1])
            r_transpose(r_row)
            w1rc1, w1rc2 = W[f"w1r_{tag}"]
            d16 = work.tile([BLOC, H * PMX], f16, tag="d16", bufs=1)
            for (n0, nn) in NCH:
                pt = ps_d.tile([128, 512], f32, tag="ps_ser")
                nc.tensor.matmul(pt[:BLOC, :nn], rT1, w1rc1[:, n0:n0 + nn], start=True, stop=False)
                nc.tensor.matmul(pt[:BLOC, :nn], rT2, w1rc2[:, n0:n0 + nn], start=False, stop=True)
                nc.vector.tensor_tensor(d16[:, n0:n0 + nn], pt[:BLOC, :nn],
                                        o_tmp[:, n0:n0 + nn], OP.subtract)
            return d16

        def bcast_do(d16, b):
            """broadcast d16 row b to a [128,1600] f16 tile via PE row-select."""
            bc = work.tile([128, H * PMX], f16, tag="dbc", bufs=2)
            for (n0, nn) in NCH:
                pb = ps_a.tile([128, 512], f32, tag="ps_s1")
                nc.tensor.matmul(pb[:, :nn], selmat[:, 128 * b:128 * (b + 1)],
                                 d16[:, n0:n0 + nn], start=True, stop=True)
                nc.scalar.activation(bc[:, n0:n0 + nn], pb[:, :nn], AF.Copy)
            return bc

        def lsm_row(s4row, b, out_dram):
            """log_softmax of one S4 row -> out_dram[b]."""
            gmax = work.tile([1, 1], f32, tag="gmax", bufs=4)
            nc.vector.tensor_reduce(gmax, s4row, axis=AX.X, op=OP.max)
            negm = work.tile([1, 1], f32, tag="negm", bufs=4)
            nc.vector.tensor_scalar_mul(negm, gmax, -1.0)
            e4 = work.tile([1, L], f32, tag="rowtmp", bufs=1)
            sume = work.tile([1, 1], f32, tag="sume", bufs=4)
            nc.scalar.activation(e4, s4row, AF.Exp, bias=negm[:, 0:1], accum_out=sume)
            lnz = work.tile([1, 1], f32, tag="lnz", bufs=4)
            nc.scalar.activation(lnz, sume, AF.Ln)
            lse = work.tile([1, 1], f32, tag="lse", bufs=4)
            nc.vector.tensor_tensor(lse, gmax, lnz, OP.add)
            lp4 = work.tile([1, L], f32, tag="rowtmp", bufs=1)
            nc.vector.tensor_scalar(lp4, s4row, lse[:, 0:1], None, op0=OP.subtract)
            dma_engines[b % 3].dma_start(out_dram[b:b + 1, :], lp4)

        def argmax_gather_b(s4row, b, dstbig, dstc3):
            mx = work.tile([1, 8], f32, tag="mx", bufs=4)
            idx = work.tile([1, 8], u32, tag="idx", bufs=4)
            nc.vector.max(out=mx, in_=s4row)
            nc.vector.max_index(out=idx, in_max=mx, in_values=s4row)
            reg = nc.values_load(idx[0:1, 0:1], min_val=0, max_val=L - 1,
                                 skip_runtime_bounds_check=True)
            dma_engines[(2 * b) % 3].dma_start(
                dstbig[:, :, b:b + 1], E16[b][:, :, ds(reg, 1)])
            dma_engines[(2 * b + 1) % 3].dma_start(
                dstc3[:16, b:b + 1], E16c3[b][:16, ds(reg, 1)])

        # shared g2/g3 stages of the scoring pipeline
        def make_g23(tag, s4rows, st, strips, s2_sbuf=False):
            w2c1 = W[f"w2c1_{tag}"]
            w2c2 = W[f"w2c2_{tag}"]
            w3c1, w3c2, w3c3, w3c4 = W[f"w3_{tag}"]

            def g2(i):
                m1 = st[i]["m1"]
                pt1 = ps_tr.tile([128, 128], f16, tag="ps_tr")
                nc.tensor.transpose(pt1, m1[:, 0:128], ident)
                m1c1 = work.tile([128, 128], f16, tag="m1c1", bufs=4)
                nc.scalar.activation(m1c1, pt1, AF.Copy)
                pt2 = ps_tr.tile([128, 128], f16, tag="ps_tr")
                nc.tensor.transpose(pt2[:72], m1[:, 128:200], ident)
                m1c2 = m1c2_slots[i % 6]
                nc.scalar.activation(m1c2[:72], pt2[:72], AF.Copy)
                m2 = m2_slots[i % 6]
                if s2_sbuf:
                    # drain stage-2 psum via scalar fp16 copies; maxpool from
                    # SBUF on vector (round-then-max == max-then-round)
                    s2s = work.tile([128, H * PMX], f16, tag="s2s", bufs=2)
                    for ni, (n0, nn) in enumerate(NCH):
                        pb = ps_b.tile([128, 512], f32, tag="ps_s2")
                        nc.tensor.matmul(pb[:, :nn], m1c1, w2c1[:, n0:n0 + nn], start=True, stop=False)
                        nc.tensor.matmul(pb[:, :nn], m1c2, w2c2[:, n0:n0 + nn], start=False, stop=True)
                        nc.scalar.activation(s2s[:, n0:n0 + nn], pb[:, :nn], AF.Copy)
                    nc.vector.tensor_reduce(
                        m2, s2s.rearrange("p (h q) -> p h q", q=PMX),
                        axis=AX.X, op=OP.max)
                else:
                    for ni, (n0, nn) in enumerate(NCH):
                        pb = ps_b.tile([128, 512], f32, tag="ps_s2")
                        nc.tensor.matmul(pb[:, :nn], m1c1, w2c1[:, n0:n0 + nn], start=True, stop=False)
                        nc.tensor.matmul(pb[:, :nn], m1c2, w2c2[:, n0:n0 + nn], start=False, stop=True)
                        h0, hn = HSL[ni]
                        nc.vector.tensor_reduce(
                            m2[:, h0:h0 + hn],
                            pb[:, :nn].rearrange("p (h q) -> p h q", q=PMX),
                            axis=AX.X, op=OP.max)
                st[i]["m1c1"] = m1c1
                st[i]["m1c2"] = m1c2
                st[i]["m2"] = m2

            def g3(i):
                b, lt = TILES[i]
                ntb = NTB[b]
                m2 = st[i]["m2"]
                pt3 = ps_tr.tile([128, 128], f16, tag="ps_tr")
                nc.tensor.transpose(pt3, m2[:, 0:128], ident)
                m2c1 = work.tile([128, 128], f16, tag="m2c1", bufs=4)
                nc.scalar.activation(m2c1, pt3, AF.Copy)
                pt4 = ps_tr.tile([128, 128], f16, tag="ps_tr")
                nc.tensor.transpose(pt4[:72], m2[:, 128:200], ident)
                m2c2 = work.tile([72, 128], f16, tag="m2c2", bufs=4)
                nc.scalar.activation(m2c2, pt4[:72], AF.Copy)
                if lt == 0:
                    strips[b] = ps_d.tile([128, 8 * NLT], f32, tag="ps_ser", name="s3strip")
                psl = strips[b][:, 8 * lt:8 * (lt + 1)]
                nc.tensor.matmul(psl, st[i]["m1c1"], w3c1, start=True, stop=False)
                nc.tensor.matmul(psl, st[i]["m1c2"], w3c2, start=False, stop=False)
                nc.tensor.matmul(psl, m2c1, w3c3, start=False, stop=False)
                nc.tensor.matmul(psl, m2c2, w3c4, start=False, stop=True)
                st[i].clear()
                if lt == ntb - 1:
                    Sb = work.tile([128, NLT], f32, tag="Sb")
                    nc.vector.tensor_reduce(Sb[:, :ntb],
                                            strips[b][:, :8 * ntb].rearrange("p (t q) -> p t q", q=PMX),
                                            axis=AX.X, op=OP.max)
                    nc.vector.tensor_tensor(Sb[:, :ntb], Sb[:, :ntb],
                                            penT_sb[:, b * NLT:b * NLT + ntb],
                                            OP.subtract)
                    ptb = ps_tr.tile([NLT, 128], f32, tag="ps_tr")
                    nc.tensor.transpose(ptb[:ntb], Sb[:, :ntb], ident32)
                    s4stg = work.tile([NLT, 128], f32, tag="s4stg")
                    nc.scalar.activation(s4stg[:ntb], ptb[:ntb], AF.Copy)
                    s4row = work.tile([1, L], f32, tag="s4row", bufs=2)
                    if ntb < NLT:
                        nc.vector.memset(s4row[:, 128 * ntb:], -BIG)
                    dma_engines[b % 3].dma_start(s4row[:, :128 * ntb], s4stg[:ntb])
                    s4rows[b] = s4row
            return g2, g3

        def run_pipeline(g1, g2, g3, fill, batch_cb):
            # batch b's callback fires two tiles after its strip completes
            # (so the S4-row DMA has landed); last batch fires immediately.
            NT = NT_ALL
            last_of = {}
            for j, (b, lt) in enumerate(TILES):
                last_of[b] = j
            cb_at = {}
            for b in range(BLOC - 1):
                cb_at[min(last_of[b] + 2, NT - 1)] = b
            cb_at[NT - 1] = BLOC - 1
            for i in range(NT + 2):
                if i < NT:
                    g1(i)
                if 1 <= i < NT + 1:
                    g2(i - 1)
                if 2 <= i:
                    j = i - 2
                    g3(j)
                    if batch_cb is not None and j in cb_at:
                        batch_cb(cb_at[j])
                if fill is not None:
                    next(fill, None)
            if fill is not None:
                for _ in fill:
                    pass

        def score_sweep(tag, s4rows, oh, ol, fill=None, batch_cb=None, a_out=None):
            """Full maxout scoring sweep; optionally spills stage-1 psum (fp16)."""
            w1m = W[f"w1m_{tag}"]
            c3pair = W[f"c3_{tag}"]
            st = [dict() for _ in range(NT_ALL)]
            strips = {}
            g2, g3 = make_g23(tag, s4rows, st, strips)

            def g1(i):
                b, lt = TILES[i]
                c3rhs = c3pair[b % 2]
                if lt == 0:
                    nc.sync.dma_start(c3rhs[16:17, :], oh[b:b + 1, :])
                    nc.sync.dma_start(c3rhs[17:18, :], ol[b:b + 1, :])
                lsl = slice(128 * lt, 128 * (lt + 1))
                m1 = m1_slots[i % 6]
                if a_out is not None:
                    a_w = work.tile([128, H * PMX], f16, tag="t_add", bufs=1)
                for ni, (n0, nn) in enumerate(NCH):
                    pa = ps_a.tile([128, 512], f32, tag="ps_s1")
                    for c in range(3):
                        nc.tensor.matmul(pa[:, :nn], E16[b][:, c, lsl], w1m[:, c, n0:n0 + nn],
                                         start=(c == 0), stop=False)
                    nc.tensor.matmul(pa[:, :nn], E16c3[b][:, lsl], c3rhs[:, n0:n0 + nn],
                                     start=False, stop=True)
                    h0, hn = HSL[ni]
                    nc.vector.tensor_reduce(
                        m1[:, h0:h0 + hn],
                        pa[:, :nn].rearrange("p (h q) -> p h q", q=PMX),
                        axis=AX.X, op=OP.max)
                    if a_out is not None:
                        nc.scalar.activation(a_w[:, n0:n0 + nn], pa[:, :nn], AF.Copy)
                if a_out is not None:
                    dma_engines[(b + lt) % 3].dma_start(a_out[b, lt], a_w)
                st[i]["m1"] = m1

            run_pipeline(g1, g2, g3, fill, batch_cb)

        def reuse_sweep(tag, s4rows, d16, a_in, batch_cb=None):
            """Scoring sweep: stage-1 = vector add of the DRAM A-tile and the
            per-batch delta-o broadcast, then an SBUF fp16 maxpool."""
            st = [dict() for _ in range(NT_ALL)]
            strips = {}
            a_tiles = {}
            dbc = {0: bcast_do(d16, 0), 1: bcast_do(d16, 1)}
            g2, g3 = make_g23(tag, s4rows, st, strips, s2_sbuf=True)

            def fetch(j):
                if j >= NT_ALL:
                    return
                b, lt = TILES[j]
                at = stage.tile([128, H * PMX], f16, tag="ar", bufs=2)
                dma_engines[j % 3].dma_start(at, a_in[b, lt])
                a_tiles[j] = at

            fetch(0)
            fetch(1)

            def g1(i):
                b, lt = TILES[i]
                fetch(i + 2)
                # broadcast the next batch's delta-o two tiles early
                if lt == NTB[b] - 2 and b + 2 < BLOC:
                    dbc[b + 2] = bcast_do(d16, b + 2)
                at = a_tiles.pop(i)
                t = work.tile([128, H * PMX], f16, tag="t_add", bufs=1)
                nc.vector.tensor_tensor(t, at, dbc[b], OP.add)
                m1 = m1_slots[i % 6]
                nc.vector.tensor_reduce(
                    m1, t.rearrange("p (h q) -> p h q", q=PMX),
                    axis=AX.X, op=OP.max)
                st[i]["m1"] = m1

            run_pipeline(g1, g2, g3, None, batch_cb)

        def lstm_update():
            """hx via LSTM cell with hx0=cx0=0 (f-gate and w_hh drop out)."""
            pt_i = ps_d.tile([128, 512], f32, tag="ps_ser")
            pt_go = ps_d.tile([128, 512], f32, tag="ps_ser")
            lhs_for = []
            for (k0, kn, wtile) in W["ih_chunks"]:
                if k0 < 400:
                    c = k0 // 128
                    lhsT = es16[:, c, :] if c < 3 else es16c3  # [17,4] w/ ones
                else:
                    c = (k0 - 400) // 128
                    lhsT = ee16[:, c, :] if c < 3 else ee16c3
                lhs_for.append((lhsT, wtile, kn + (1 if k0 == 384 else 0)))
            n = len(lhs_for)
            for i, (lhsT, wtile, rows) in enumerate(lhs_for):
                nc.tensor.matmul(pt_i[:BLOC, :H], lhsT, wtile[:rows, 0:H],
                                 start=(i == 0), stop=(i == n - 1))
            for i, (lhsT, wtile, rows) in enumerate(lhs_for):
                nc.tensor.matmul(pt_go[:BLOC, :2 * H], lhsT, wtile[:rows, 2 * H:4 * H],
                                 start=(i == 0), stop=(i == n - 1))
            ig = work.tile([BLOC, H], f32, tag="ig", bufs=1)
            nc.scalar.activation(ig, pt_i[:BLOC, :H], AF.Sigmoid)
            gg = work.tile([BLOC, H], f32, tag="gg", bufs=1)
            nc.scalar.activation(gg, pt_go[:BLOC, 0:H], AF.Tanh)
            og = work.tile([BLOC, H], f32, tag="og", bufs=1)
            nc.scalar.activation(og, pt_go[:BLOC, H:2 * H], AF.Sigmoid)
            cx = work.tile([BLOC, H], f32, tag="cx", bufs=1)
            nc.vector.tensor_tensor(cx, ig, gg, OP.mult)
            tcx = work.tile([BLOC, H], f32, tag="tcx", bufs=1)
            nc.scalar.activation(tcx, cx, AF.Tanh)
            h0 = work.tile([BLOC, H], f16, tag="h0", bufs=1)
            nc.vector.tensor_tensor(h0, og, tcx, OP.mult)
            ptr = ps_tr.tile([128, 128], f16, tag="ps_tr")
            nc.tensor.transpose(ptr[:, :BLOC], h0[:, 0:128], ident[:BLOC, :BLOC])
            nc.vector.tensor_copy(h0T1, ptr[:, :BLOC])
            ptr2 = ps_tr.tile([128, 128], f16, tag="ps_tr")
            nc.tensor.transpose(ptr2[:72, :BLOC], h0[:, 128:200], ident[:BLOC, :BLOC])
            nc.vector.tensor_copy(h0T2[:72], ptr2[:72, :BLOC])
            pt = ps_d.tile([128, 512], f32, tag="ps_ser")
            wmlpc1, wmlpc2 = W["wmlp"]
            nc.tensor.matmul(pt[:BLOC, :H], h0T1, wmlpc1, start=True, stop=False)
            nc.tensor.matmul(pt[:BLOC, :H], h0T2, wmlpc2, start=False, stop=True)
            hx_row = work.tile([BLOC, H], f16, tag="hx_row", bufs=1)
            nc.scalar.activation(hx_row, pt[:BLOC, :H], AF.Copy)
            ptr3 = ps_tr.tile([128, 128], f16, tag="ps_tr")
            nc.tensor.transpose(ptr3[:, :BLOC], hx_row[:, 0:128], ident[:BLOC, :BLOC])
            nc.vector.tensor_copy(hxT[0], ptr3[:, :BLOC])
            ptr4 = ps_tr.tile([128, 128], f16, tag="ps_tr")
            nc.tensor.transpose(ptr4[:72, :BLOC], hx_row[:, 128:200], ident[:BLOC, :BLOC])
            nc.vector.tensor_copy(hxT[1], ptr4[:72, :BLOC])

        # ---------------- prologue: s-critical-path loads ----------------
        import itertools
        for _ in load_wd("s", nc.sync, nc.vector):
            pass
        for _ in load_w1r("s", nc.sync, nc.vector):
            pass
        load_E(0, nc.gpsimd)
        for _ in load_w1m("s", nc.sync, nc.scalar):
            pass
        edefer = []
        load_E(1, nc.gpsimd, edefer)
        load_E(2, nc.gpsimd, edefer)
        load_E(3, nc.gpsimd, edefer)

        def edefer_gen():
            for fn in edefer:
                fn()
                yield
        fill_steps = itertools.chain(
            edefer_gen(),
            load_wd("e", nc.sync, nc.scalar),
            load_w1r("e", nc.sync, nc.scalar),
            load_w1m("e", nc.sync, nc.scalar),
            load_scoring2("e", nc.sync, nc.scalar, "stg"),
            load_lstm(nc.sync, nc.scalar))

        # ---------------- the four passes ----------------
        r_row = r_matmul("s", with_hx=False)
        oh, ol = o_rows("s", r_row, save=REUSE)
        for _ in load_scoring2("s", nc.sync, nc.scalar, "stg"):
            pass
        rows1 = {}
        score_sweep("s", rows1, oh, ol, fill=fill_steps,
                    batch_cb=lambda b: argmax_gather_b(rows1[b], b, es16, es16c3),
                    a_out=a1d if REUSE else None)

        r_row = r_matmul("e", with_hx=False)
        oh, ol = o_rows("e", r_row, save=REUSE)
        rows2 = {}
        score_sweep("e", rows2, oh, ol,
                    batch_cb=lambda b: argmax_gather_b(rows2[b], b, ee16, ee16c3),
                    a_out=a2d if REUSE else None)

        lstm_update()

        rows3 = {}
        rows4 = {}

        def cb3(b):
            argmax_gather_b(rows3[b], b, es16, es16c3)
            if b < BLOC - 1:
                lsm_row(rows3[b], b, lp1)

        def cb4(b):
            if b < BLOC - 1:
                lsm_row(rows4[b], b, lp2)

        r_row = r_matmul("s", with_hx=True)
        if REUSE:
            d16 = delta_o_rows("s", r_row)
            reuse_sweep("s", rows3, d16, a1d, batch_cb=cb3)
            r_row = r_matmul("e", with_hx=True)
            d16 = delta_o_rows("e", r_row)
            lsm_row(rows3[BLOC - 1], BLOC - 1, lp1)
            reuse_sweep("e", rows4, d16, a2d, batch_cb=cb4)
        else:
            oh, ol = o_rows("s", r_row)
            score_sweep("s", rows3, oh, ol, batch_cb=cb3)
            r_row = r_matmul("e", with_hx=True)
            oh, ol = o_rows("e", r_row)
            lsm_row(rows3[BLOC - 1], BLOC - 1, lp1)
            score_sweep("e", rows4, oh, ol, batch_cb=cb4)
        lsm_row(rows4[BLOC - 1], BLOC - 1, lp2)

        stage_cm.__exit__(None, None, None)

    nc.compile()
    return nc


def batch_order(lens):
    """order[b*NCORES + c] = original batch index at core c, slot b.
    Sorting by length groups similar lengths into each slot so the
    per-slot tile count (max over cores) stays tight."""
    return np.argsort(np.asarray(lens).astype(np.int64), kind="stable")


def get_program(inputs):
    lens = np.asarray(inputs["passage_lens"]).astype(np.int64)
    order = batch_order(lens)
    nt_b = tuple(int(min(NLT, (int(lens[order[b * NCORES + c]]) + 127) // 128))
                 for b in range(BLOC) for c in range(NCORES))
    if _cache.get("key") != nt_b:
        _cache["nc"] = _build_program(nt_b)
        _cache["key"] = nt_b
    return _cache["nc"]


def _split16(x):
    hi = np.asarray(x, np.float32).astype(np.float16)
    lo = (np.asarray(x, np.float32) - hi.astype(np.float32)).astype(np.float16)
    return hi, lo


def make_in_maps(inputs):
    """Per-core input maps: batch shard + trivial host prep (mask, bias splits)."""
    inputs = {k: np.asarray(v) for k, v in inputs.items()}
    enc = np.ascontiguousarray(inputs["encoding_matrix"], dtype=np.float32)
    lens = np.asarray(inputs["passage_lens"]).astype(np.int64)
    pen_full = np.where(np.arange(L)[None, :] < lens[:, None],
                        np.float32(0.0), BIG).astype(np.float32)

    shared = {}
    for tag in ("s", "e"):
        shared[f"w1_{tag}"] = np.ascontiguousarray(inputs[f"w1_{tag}"], np.float32)
        shared[f"b1_{tag}"] = np.ascontiguousarray(inputs[f"b1_{tag}"], np.float32).reshape(1, -1)
        shared[f"w2_{tag}"] = np.ascontiguousarray(inputs[f"w2_{tag}"], np.float32)
        b2h, b2l = _split16(inputs[f"b2_{tag}"])
        shared[f"b2h_{tag}"] = b2h.reshape(1, -1)
        shared[f"b2l_{tag}"] = b2l.reshape(1, -1)
        shared[f"w3_{tag}"] = np.ascontiguousarray(inputs[f"w3_{tag}"], np.float32)
        b3h, b3l = _split16(inputs[f"b3_{tag}"])
        shared[f"b3h_{tag}"] = b3h.reshape(1, -1)
        shared[f"b3l_{tag}"] = b3l.reshape(1, -1)
        shared[f"wd_{tag}"] = np.ascontiguousarray(inputs[f"wd_{tag}"], np.float32)
    shared["w_ih"] = np.ascontiguousarray(inputs["w_ih"], np.float32)
    shared["b_lstm"] = np.ascontiguousarray(inputs["b_lstm"], np.float32).reshape(1, -1)
    shared["w_mlp"] = np.ascontiguousarray(inputs["w_mlp"], np.float32)
    shared["b_mlp"] = np.ascontiguousarray(inputs["b_mlp"], np.float32).reshape(1, -1)

    order = batch_order(lens)
    in_maps = []
    for core in range(NCORES):
        idx = [int(order[b * NCORES + core]) for b in range(BLOC)]
        m = dict(shared)
        m["enc"] = np.ascontiguousarray(enc[idx])
        m["selmat_d"] = np.kron(np.eye(BLOC, dtype=np.float16),
                                np.ones((1, 128), dtype=np.float16))
        pc = pen_full[idx].reshape(BLOC, NLT, 128)
        m["penT"] = np.ascontiguousarray(pc.transpose(2, 0, 1).reshape(128, BLOC * NLT))
        in_maps.append(m)
    return in_maps


def run_on_hw(inputs, trace=False):
    from concourse import bass_utils
    nc = get_program(inputs)
    in_maps = make_in_maps(inputs)
    res = bass_utils.run_bass_kernel_spmd(nc, in_maps, core_ids=list(range(NCORES)),
                                          trace=trace)
    order = batch_order(inputs["passage_lens"])
    lp1 = np.empty((B, L), np.float32)
    lp2 = np.empty((B, L), np.float32)
    for c in range(NCORES):
        for b in range(BLOC):
            ob = int(order[b * NCORES + c])
            lp1[ob] = np.asarray(res.results[c]["lp1"])[b]
            lp2[ob] = np.asarray(res.results[c]["lp2"])[b]
    return (lp1, lp2), res


def kernel(**inputs):
    out, _ = run_on_hw(inputs, trace=False)
    return out


# revision 23
# speedup vs baseline: 1.4010x; 1.0097x over previous
"""Trainium2 Bass kernel for nn_Decoder_86921548137026.

Dynamic decoder: NITER=2 iterations of (maxout pointer scoring over L=1024
positions -> argmax -> gather -> LSTM cell), followed by log_softmax over the
final start/end scores.

Sharding: data-parallel over batch B=32 across 8 cores (4 batches/core),
weights replicated.

v3 changes vs v2 (934us):
  - prologue restructured: critical-path DMA order (wd -> w1r -> E b0 -> w1m
    -> w2/w3), E b1-3 on the gpsimd queue, r/o emitted early; sweep-1 starts
    ~15us instead of ~94us
  - penalty fold moved from S4-post to a per-batch f32 subtract on Sb before
    the transpose (bit-exact, removes it from the boundary path)
  - per-batch argmax/gather pipelined into the sweep (batch b resolved two
    tiles after its strip completes); per-batch log_softmax rows likewise
  - sweeps 3/4 reuse stage-1: sweep-1/2 stage-1 PSUM (A = E@W1 + o_fold) is
    rounded to fp16 and spilled to DRAM; the iter-2 sweeps replace the big
    stage-1 matmul with a vector add of the per-batch delta-o broadcast and
    an SBUF fp16 maxpool (validated offline: 0 argmax flips, rel 3.6e-4)
"""

import numpy as np

H = 200
PMX = 8
B = 32
L = 1024
BIG = np.float32(1e30)
NCORES = 8
BLOC = B // NCORES          # 4 batches per core
NLT = L // 128              # 8 l-tiles per batch
# stage-1/2 output channels = H*P = 1600, swept in PSUM-bank-sized chunks
NCH = [(0, 512), (512, 512), (1024, 512), (1536, 64)]
# h-slice of m1/m2 produced by each n-chunk (1600 = 200h * 8p, h-major)
HSL = [(0, 64), (64, 64), (128, 64), (192, 8)]
REUSE = False              # full sweeps everywhere: no A-spill HBM traffic

_cache = {}


def _build_program(nt_b):
    import contextlib
    import concourse.mybir as mybir
    import concourse.tile as tile
    from concourse import bacc
    from concourse.bass import ds
    from concourse.masks import make_identity

    f32 = mybir.dt.float32
    f16 = mybir.dt.float16
    u32 = mybir.dt.uint32
    AF = mybir.ActivationFunctionType
    OP = mybir.AluOpType
    AX = mybir.AxisListType

    nc = bacc.Bacc("TRN2", target_bir_lowering=False, debug=False,
                   enable_asserts=False, num_devices=NCORES)

    # ---------------- DRAM I/O ----------------
    enc = nc.dram_tensor("enc", [BLOC, 2 * H, L], f32, kind="ExternalInput").ap()
    penT = nc.dram_tensor("penT", [128, BLOC * NLT], f32, kind="ExternalInput").ap()
    selmat_d = nc.dram_tensor("selmat_d", [BLOC, BLOC * 128], f16, kind="ExternalInput").ap()
    wt = {}
    for tag in ("s", "e"):
        wt[f"w1_{tag}"] = nc.dram_tensor(f"w1_{tag}", [3 * H, H * PMX], f32, kind="ExternalInput").ap()
        wt[f"b1_{tag}"] = nc.dram_tensor(f"b1_{tag}", [1, H * PMX], f32, kind="ExternalInput").ap()
        wt[f"w2_{tag}"] = nc.dram_tensor(f"w2_{tag}", [H, H * PMX], f32, kind="ExternalInput").ap()
        wt[f"b2h_{tag}"] = nc.dram_tensor(f"b2h_{tag}", [1, H * PMX], f16, kind="ExternalInput").ap()
        wt[f"b2l_{tag}"] = nc.dram_tensor(f"b2l_{tag}", [1, H * PMX], f16, kind="ExternalInput").ap()
        wt[f"w3_{tag}"] = nc.dram_tensor(f"w3_{tag}", [2 * H, PMX], f32, kind="ExternalInput").ap()
        wt[f"b3h_{tag}"] = nc.dram_tensor(f"b3h_{tag}", [1, PMX], f16, kind="ExternalInput").ap()
        wt[f"b3l_{tag}"] = nc.dram_tensor(f"b3l_{tag}", [1, PMX], f16, kind="ExternalInput").ap()
        wt[f"wd_{tag}"] = nc.dram_tensor(f"wd_{tag}", [5 * H, H], f32, kind="ExternalInput").ap()
    wt["w_ih"] = nc.dram_tensor("w_ih", [4 * H, 4 * H], f32, kind="ExternalInput").ap()
    wt["b_lstm"] = nc.dram_tensor("b_lstm", [1, 4 * H], f32, kind="ExternalInput").ap()
    wt["w_mlp"] = nc.dram_tensor("w_mlp", [H, H], f32, kind="ExternalInput").ap()
    wt["b_mlp"] = nc.dram_tensor("b_mlp", [1, H], f32, kind="ExternalInput").ap()

    lp1 = nc.dram_tensor("lp1", [BLOC, L], f32, kind="ExternalOutput").ap()
    lp2 = nc.dram_tensor("lp2", [BLOC, L], f32, kind="ExternalOutput").ap()
    if REUSE:
        a1d = nc.dram_tensor("a1d", [BLOC, NLT, 128, H * PMX], f16, kind="Internal").ap()
        a2d = nc.dram_tensor("a2d", [BLOC, NLT, 128, H * PMX], f16, kind="Internal").ap()
        o_save_d = nc.dram_tensor("o_save_d", [2, BLOC, H * PMX], f32, kind="Internal").ap()

    # one SPMD program for all cores: batch-slot b uses the max tile count
    # over cores so every core's valid region is covered
    NTB = [max(nt_b[b * NCORES + c] for c in range(NCORES)) for b in range(BLOC)]
    TILES = [(b, lt) for b in range(BLOC) for lt in range(NTB[b])]
    NT_ALL = len(TILES)

    with tile.TileContext(nc) as tc, contextlib.ExitStack() as ctx:
        const = ctx.enter_context(tc.tile_pool(name="const", bufs=1))
        wpool = ctx.enter_context(tc.tile_pool(name="wpool", bufs=1))
        epool = ctx.enter_context(tc.tile_pool(name="epool", bufs=1))
        work = ctx.enter_context(tc.tile_pool(name="work", bufs=3))
        single = ctx.enter_context(tc.tile_pool(name="single", bufs=1))
        keep = ctx.enter_context(tc.tile_pool(name="keep", bufs=1))
        ps_a = ctx.enter_context(tc.tile_pool(name="ps_a", bufs=3, space="PSUM"))
        ps_b = ctx.enter_context(tc.tile_pool(name="ps_b", bufs=2, space="PSUM"))
        ps_tr = ctx.enter_context(tc.tile_pool(name="ps_tr", bufs=2, space="PSUM"))
        ps_d = ctx.enter_context(tc.tile_pool(name="ps_d", bufs=1, space="PSUM"))
        stage_cm = tc.tile_pool(name="stage", bufs=2)
        stage = stage_cm.__enter__()

        dma_engines = [nc.sync, nc.scalar, nc.gpsimd]

        # ---------------- constants ----------------
        ident = const.tile([128, 128], f16, name="ident")
        make_identity(nc, ident)
        ident32 = const.tile([128, 128], f32, name="ident32")
        make_identity(nc, ident32)
        # row-selector stationaries: selmat[:, 128b:128(b+1)] broadcasts d16
        # row b to all 128 output partitions (host-built constant)
        selmat = const.tile([BLOC, BLOC * 128], f16, name="selmat")
        nc.sync.dma_start(selmat, selmat_d)

        penT_sb = const.tile([128, BLOC * NLT], f32, name="penT_sb")
        nc.sync.dma_start(penT_sb, penT)

        # ---------------- es/ee init from E column 0 ----------------
        es16 = keep.tile([128, 3, BLOC], f16, name="es16")
        es16c3 = keep.tile([17, BLOC], f16, name="es16c3")
        nc.gpsimd.memset(es16c3, 1.0)              # row 16 = b_lstm fold row
        ee16 = keep.tile([128, 3, BLOC], f16, name="ee16")
        ee16c3 = keep.tile([16, BLOC], f16, name="ee16c3")
        escol = const.tile([128, 4, BLOC], f32, name="escol")
        for c in range(3):
            nc.sync.dma_start(escol[:, c, :],
                              enc[0:BLOC, 128 * c:128 * (c + 1), 0:1].rearrange("b p x -> p (b x)"))
        nc.sync.dma_start(escol[:16, 3, :],
                          enc[0:BLOC, 384:400, 0:1].rearrange("b p x -> p (b x)"))
        nc.vector.tensor_copy(es16, escol[:, 0:3, :])
        nc.vector.tensor_copy(es16c3[:16], escol[:16, 3, :])
        nc.vector.tensor_copy(ee16, es16)
        nc.vector.tensor_copy(ee16c3, es16c3[:16])

        # ---------------- weight / E loaders ----------------
        W = {}

        def cast(eng, dst, src):
            if eng is nc.scalar:
                nc.scalar.activation(dst, src, AF.Copy)
            else:
                eng.tensor_copy(dst, src)

        def load_wd(tag, dma, cse):
            # wd fp16 k-chunk tiles over cat(hx[0:200], es[200:600], ee[600:1000])
            wd_chunks = []
            for (k0, kn) in [(200, 128), (328, 128), (456, 128), (584, 16),
                             (600, 128), (728, 128), (856, 128), (984, 16),
                             (0, 128), (128, 72)]:
                wst = stage.tile([128, 800], f32, tag="stg8", bufs=2)
                dma.dma_start(wst[:kn, :H], wt[f"wd_{tag}"][k0:k0 + kn])
                t = wpool.tile([kn, H], f16, name=f"wd16_{tag}_{k0}")
                cast(cse, t, wst[:kn, :H])
                wd_chunks.append((k0, kn, t))
                yield
            W[f"wd_{tag}"] = wd_chunks

        def load_w1r(tag, dma, cse):
            # o-matmul rhs (fp16): rows 400..599 of w1, + b1 as ones-row 72 of c2
            wst = stage.tile([128, 1600], f32, tag="stg", bufs=2)
            dma.dma_start(wst, wt[f"w1_{tag}"][400:528])
            w1rc1 = wpool.tile([128, H * PMX], f16, name=f"w1r16c1_{tag}")
            cast(cse, w1rc1, wst)
            yield
            wst = stage.tile([128, 1600], f32, tag="stg", bufs=2)
            dma.dma_start(wst[:72], wt[f"w1_{tag}"][528:600])
            dma.dma_start(wst[72:73], wt[f"b1_{tag}"])
            w1rc2 = wpool.tile([73, H * PMX], f16, name=f"w1r16c2_{tag}")
            cast(cse, w1rc2, wst[:73])
            yield
            W[f"w1r_{tag}"] = (w1rc1, w1rc2)

        def load_w1m(tag, dma, csw):
            # stage-1 rhs rows 0..383 as fp16 [128, 3, 1600]
            w1m = wpool.tile([128, 3, H * PMX], f16, name=f"w1m_{tag}")
            for c in range(3):
                wst = stage.tile([128, 1600], f32, tag="stg", bufs=2)
                dma.dma_start(wst, wt[f"w1_{tag}"][128 * c:128 * (c + 1)])
                cast(csw, w1m[:, c, :], wst)
                yield
            W[f"w1m_{tag}"] = w1m
            # shared c3 rhs: rows 0..15 = W1[384:400] fp16, rows 16/17 = o hi/lo
            wst = stage.tile([128, 1600], f32, tag="stg", bufs=2)
            dma.dma_start(wst[:16], wt[f"w1_{tag}"][384:400])
            c3pair = []
            for pi in range(2):
                c3 = wpool.tile([18, H * PMX], f16, name=f"c3_{tag}_{pi}")
                cast(csw, c3[:16], wst[:16])
                c3pair.append(c3)
            yield
            W[f"c3_{tag}"] = c3pair

        def load_scoring2(tag, dma, csw, stg):
            # stage-2 rhs
            w2c1 = wpool.tile([128, H * PMX], f16, name=f"w2c1_{tag}")
            wst = stage.tile([128, 1600], f32, tag=stg, bufs=2)
            dma.dma_start(wst, wt[f"w2_{tag}"][:128])
            cast(csw, w2c1, wst)
            yield
            W[f"w2c1_{tag}"] = w2c1
            w2c2 = wpool.tile([74, H * PMX], f16, name=f"w2c2_{tag}")
            wst = stage.tile([128, 1600], f32, tag=stg, bufs=2)
            dma.dma_start(wst[:72], wt[f"w2_{tag}"][128:200])
            cast(csw, w2c2[:72], wst[:72])
            dma.dma_start(w2c2[72:73, :], wt[f"b2h_{tag}"])
            dma.dma_start(w2c2[73:74, :], wt[f"b2l_{tag}"])
            yield
            W[f"w2c2_{tag}"] = w2c2
            # stage-3 rhs chunks
            wst = stage.tile([128, 1600], f32, tag=stg, bufs=2)
            dma.dma_start(wst[:, 0:8], wt[f"w3_{tag}"][0:128])
            dma.dma_start(wst[:72, 8:16], wt[f"w3_{tag}"][128:200])
            dma.dma_start(wst[:, 16:24], wt[f"w3_{tag}"][200:328])
            dma.dma_start(wst[:72, 24:32], wt[f"w3_{tag}"][328:400])
            w3c1 = wpool.tile([128, PMX], f16, name=f"w3c1_{tag}")
            cast(csw, w3c1, wst[:, 0:8])
            w3c2 = wpool.tile([74, PMX], f16, name=f"w3c2_{tag}")
            cast(csw, w3c2[:72], wst[:72, 8:16])
            yield
            dma.dma_start(w3c2[72:73, :], wt[f"b3h_{tag}"])
            dma.dma_start(w3c2[73:74, :], wt[f"b3l_{tag}"])
            w3c3 = wpool.tile([128, PMX], f16, name=f"w3c3_{tag}")
            cast(csw, w3c3, wst[:, 16:24])
            w3c4 = wpool.tile([72, PMX], f16, name=f"w3c4_{tag}")
            cast(csw, w3c4, wst[:72, 24:32])
            yield
            W[f"w3_{tag}"] = (w3c1, w3c2, w3c3, w3c4)

        def load_lstm(dma, cse):
            # LSTM weights as fp16 rhs [kn, 800] over rows = cat(es, ee);
            # es-c3 chunk carries b_lstm as ones-row 16
            ih_chunks = []
            for (k0, kn) in [(0, 128), (128, 128), (256, 128), (384, 16),
                             (400, 128), (528, 128), (656, 128), (784, 16)]:
                wst = stage.tile([128, 1600], f32, tag="stg", bufs=2)
                dma.dma_start(wst[:kn, :800], wt["w_ih"][k0:k0 + kn])
                rows = kn + 1 if k0 == 384 else kn
                t = wpool.tile([rows, 4 * H], f16, name=f"wih16_{k0}")
                if k0 == 384:
                    dma.dma_start(wst[16:17, :800], wt["b_lstm"])
                cast(cse, t[:rows], wst[:rows, :800])
                ih_chunks.append((k0, kn, t))
                yield
            # mlp fp16 rhs [kn, 200]; c2 carries b_mlp as ones-row 72
            wst = stage.tile([128, 1600], f32, tag="stg", bufs=2)
            dma.dma_start(wst[:, :H], wt["w_mlp"][0:128])
            wmlpc1 = wpool.tile([128, H], f16, name="wmlp16c1")
            cast(cse, wmlpc1, wst[:, :H])
            yield
            wst = stage.tile([128, 1600], f32, tag="stg", bufs=2)
            dma.dma_start(wst[:72, :H], wt["w_mlp"][128:200])
            dma.dma_start(wst[72:73, :H], wt["b_mlp"])
            wmlpc2 = wpool.tile([73, H], f16, name="wmlp16c2")
            cast(cse, wmlpc2, wst[:73, :H])
            W["ih_chunks"] = ih_chunks
            W["wmlp"] = (wmlpc1, wmlpc2)
            yield

        # ---------------- E load ----------------
        E16 = []
        E16c3 = []

        def load_E(b, dma, defer=None):
            et = epool.tile([128, 3, L], f16, name=f"E16_{b}")
            ec3 = epool.tile([18, L], f16, name=f"E16c3_{b}")
            nc.gpsimd.memset(ec3, 1.0)
            HL = L // 2
            for h in range(2):
                hs = slice(h * HL, (h + 1) * HL)
                est = stage.tile([128, 3, HL], f32, tag="estg", bufs=2)
                for c in range(3):
                    dma.dma_start(est[:, c, :], enc[b, 128 * c:128 * (c + 1), hs])
                est2 = stage.tile([16, HL], f32, tag="estg2", bufs=1)
                dma.dma_start(est2, enc[b, 384:400, hs])
                if defer is None:
                    nc.vector.tensor_copy(et[:, :, hs], est)
                    nc.vector.tensor_copy(ec3[:16, hs], est2)
                else:
                    defer.append(lambda et=et, est=est, hs=hs:
                                 nc.vector.tensor_copy(et[:, :, hs], est))
                    defer.append(lambda ec3=ec3, est2=est2, hs=hs:
                                 nc.vector.tensor_copy(ec3[:16, hs], est2))
            E16.append(et)
            E16c3.append(ec3)

        # ---------------- persistent state tiles ----------------
        hxT = [keep.tile([128, BLOC], f16, name="hxT16_0"),
               keep.tile([72, BLOC], f16, name="hxT16_1")]
        rT1 = keep.tile([128, BLOC], f16, name="rT1")
        rT2 = keep.tile([73, BLOC], f16, name="rT2")
        nc.gpsimd.memset(rT2, 1.0)                 # row 72 = b1 fold row
        h0T1 = keep.tile([128, BLOC], f16, name="h0T1")
        h0T2 = keep.tile([73, BLOC], f16, name="h0T2")
        nc.gpsimd.memset(h0T2, 1.0)                # row 72 = b_mlp fold row

        m1_slots, m2_slots, m1c2_slots = [], [], []
        for i in range(6):
            m1_slots.append(keep.tile([128, H], f16, name=f"m1_slot{i}"))
            m2_slots.append(keep.tile([128, H], f16, name=f"m2_slot{i}"))
            t = keep.tile([74, 128], f16, name=f"m1c2_slot{i}")
            nc.gpsimd.memset(t, 1.0)
            m1c2_slots.append(t)

        # ---------------- helpers ----------------
        def cat_chunks(tag, with_hx):
            """(lhsT [kn,4] fp16, wd16 [kn,H]) pairs for r = tanh(cat @ wd)."""
            ops = []
            for (k0, kn, wtile) in W[f"wd_{tag}"]:
                if k0 < 200:
                    if not with_hx:
                        continue
                    lhsT = hxT[0] if k0 == 0 else hxT[1]
                elif k0 < 600:
                    c = (k0 - 200) // 128
                    lhsT = es16[:, c, :] if c < 3 else es16c3[:16]
                else:
                    c = (k0 - 600) // 128
                    lhsT = ee16[:, c, :] if c < 3 else ee16c3
                ops.append((lhsT, wtile))
            return ops

        def r_matmul(tag, with_hx):
            """r_row = tanh(cat @ wd) -> [4, H] fp16 sbuf."""
            ops = cat_chunks(tag, with_hx)
            pt = ps_d.tile([128, 512], f32, tag="ps_ser")
            for i, (lhsT, rhs) in enumerate(ops):
                nc.tensor.matmul(pt[:BLOC, :H], lhsT, rhs,
                                 start=(i == 0), stop=(i == len(ops) - 1))
            r_row = work.tile([BLOC, H], f16, tag="r_row", bufs=1)
            nc.scalar.activation(r_row, pt[:BLOC, :H], AF.Tanh)
            return r_row

        def r_transpose(r_row):
            ptr = ps_tr.tile([128, 128], f16, tag="ps_tr")
            nc.tensor.transpose(ptr[:, :BLOC], r_row[:, 0:128], ident[:BLOC, :BLOC])
            nc.vector.tensor_copy(rT1, ptr[:, :BLOC])
            ptr2 = ps_tr.tile([128, 128], f16, tag="ps_tr")
            nc.tensor.transpose(ptr2[:72, :BLOC], r_row[:, 128:200], ident[:BLOC, :BLOC])
            nc.vector.tensor_copy(rT2[:72], ptr2[:72, :BLOC])

        def o_rows(tag, r_row, save=False):
            """o = r@w1r + b1 (f32 psum) -> fp16 hi/lo rows [BLOC, 1600]."""
            if save:
                o_tmp = work.tile([BLOC, H * PMX], f32, tag="o_tmp", bufs=1)
            r_transpose(r_row)
            w1rc1, w1rc2 = W[f"w1r_{tag}"]
            oh = single.tile([BLOC, H * PMX], f16, tag="oh")
            ol = single.tile([BLOC, H * PMX], f16, tag="ol")
            for (n0, nn) in NCH:
                pt = ps_d.tile([128, 512], f32, tag="ps_ser")
                nc.tensor.matmul(pt[:BLOC, :nn], rT1, w1rc1[:, n0:n0 + nn], start=True, stop=False)
                nc.tensor.matmul(pt[:BLOC, :nn], rT2, w1rc2[:, n0:n0 + nn], start=False, stop=True)
                nc.scalar.activation(oh[:, n0:n0 + nn], pt[:BLOC, :nn], AF.Copy)
                nc.vector.tensor_tensor(ol[:, n0:n0 + nn], pt[:BLOC, :nn], oh[:, n0:n0 + nn], OP.subtract)
                if save:
                    nc.vector.tensor_copy(o_tmp[:, n0:n0 + nn], pt[:BLOC, :nn])
            if save:
                nc.gpsimd.dma_start(o_save_d[0 if tag == "s" else 1], o_tmp)
            return oh, ol

        def delta_o_rows(tag, r_row):
            """d16 [4,1600] f16 = (r@w1r+b1) - o_save (sweep-1/2's o psum)."""
            o_tmp = work.tile([BLOC, H * PMX], f32, tag="o_tmp", bufs=1)
            nc.gpsimd.dma_start(o_tmp, o_save_d[0 if tag == "s" else 1])
            r_transpose(r_row)
            w1rc1, w1rc2 = W[f"w1r_{tag}"]
            d16 = work.tile([BLOC, H * PMX], f16, tag="d16", bufs=1)
            for (n0, nn) in NCH:
                pt = ps_d.tile([128, 512], f32, tag="ps_ser")
                nc.tensor.matmul(pt[:BLOC, :nn], rT1, w1rc1[:, n0:n0 + nn], start=True, stop=False)
                nc.tensor.matmul(pt[:BLOC, :nn], rT2, w1rc2[:, n0:n0 + nn], start=False, stop=True)
                nc.vector.tensor_tensor(d16[:, n0:n0 + nn], pt[:BLOC, :nn],
                                        o_tmp[:, n0:n0 + nn], OP.subtract)
            return d16

        def bcast_do(d16, b):
            """broadcast d16 row b to a [128,1600] f16 tile via PE row-select."""
            bc = work.tile([128, H * PMX], f16, tag="dbc", bufs=2)
            for (n0, nn) in NCH:
                pb = ps_a.tile([128, 512], f32, tag="ps_s1")
                nc.tensor.matmul(pb[:, :nn], selmat[:, 128 * b:128 * (b + 1)],
                                 d16[:, n0:n0 + nn], start=True, stop=True)
                nc.scalar.activation(bc[:, n0:n0 + nn], pb[:, :nn], AF.Copy)
            return bc

        def lsm_row(s4row, b, out_dram):
            """log_softmax of one S4 row -> out_dram[b]."""
            gmax = work.tile([1, 1], f32, tag="gmax", bufs=4)
            nc.vector.tensor_reduce(gmax, s4row, axis=AX.X, op=OP.max)
            negm = work.tile([1, 1], f32, tag="negm", bufs=4)
            nc.vector.tensor_scalar_mul(negm, gmax, -1.0)
            e4 = work.tile([1, L], f32, tag="rowtmp", bufs=1)
            sume = work.tile([1, 1], f32, tag="sume", bufs=4)
            nc.scalar.activation(e4, s4row, AF.Exp, bias=negm[:, 0:1], accum_out=sume)
            lnz = work.tile([1, 1], f32, tag="lnz", bufs=4)
            nc.scalar.activation(lnz, sume, AF.Ln)
            lse = work.tile([1, 1], f32, tag="lse", bufs=4)
            nc.vector.tensor_tensor(lse, gmax, lnz, OP.add)
            lp4 = work.tile([1, L], f32, tag="rowtmp", bufs=1)
            nc.vector.tensor_scalar(lp4, s4row, lse[:, 0:1], None, op0=OP.subtract)
            dma_engines[b % 3].dma_start(out_dram[b:b + 1, :], lp4)

        def argmax_gather_b(s4row, b, dstbig, dstc3):
            mx = work.tile([1, 8], f32, tag="mx", bufs=4)
            idx = work.tile([1, 8], u32, tag="idx", bufs=4)
            nc.vector.max(out=mx, in_=s4row)
            nc.vector.max_index(out=idx, in_max=mx, in_values=s4row)
            reg = nc.values_load(idx[0:1, 0:1], min_val=0, max_val=L - 1,
                                 skip_runtime_bounds_check=True)
            dma_engines[(2 * b) % 3].dma_start(
                dstbig[:, :, b:b + 1], E16[b][:, :, ds(reg, 1)])
            dma_engines[(2 * b + 1) % 3].dma_start(
                dstc3[:16, b:b + 1], E16c3[b][:16, ds(reg, 1)])

        # shared g2/g3 stages of the scoring pipeline
        def make_g23(tag, s4rows, st, strips, s2_sbuf=False):
            w2c1 = W[f"w2c1_{tag}"]
            w2c2 = W[f"w2c2_{tag}"]
            w3c1, w3c2, w3c3, w3c4 = W[f"w3_{tag}"]

            def g2(i):
                m1 = st[i]["m1"]
                pt1 = ps_tr.tile([128, 128], f16, tag="ps_tr")
                nc.tensor.transpose(pt1, m1[:, 0:128], ident)
                m1c1 = work.tile([128, 128], f16, tag="m1c1", bufs=4)
                nc.scalar.activation(m1c1, pt1, AF.Copy)
                pt2 = ps_tr.tile([128, 128], f16, tag="ps_tr")
                nc.tensor.transpose(pt2[:72], m1[:, 128:200], ident)
                m1c2 = m1c2_slots[i % 6]
                nc.scalar.activation(m1c2[:72], pt2[:72], AF.Copy)
                m2 = m2_slots[i % 6]
                if s2_sbuf:
                    # drain stage-2 psum via scalar fp16 copies; maxpool from
                    # SBUF on vector (round-then-max == max-then-round)
                    s2s = work.tile([128, H * PMX], f16, tag="s2s", bufs=2)
                    for ni, (n0, nn) in enumerate(NCH):
                        pb = ps_b.tile([128, 512], f32, tag="ps_s2")
                        nc.tensor.matmul(pb[:, :nn], m1c1, w2c1[:, n0:n0 + nn], start=True, stop=False)
                        nc.tensor.matmul(pb[:, :nn], m1c2, w2c2[:, n0:n0 + nn], start=False, stop=True)
                        nc.scalar.activation(s2s[:, n0:n0 + nn], pb[:, :nn], AF.Copy)
                    nc.vector.tensor_reduce(
                        m2, s2s.rearrange("p (h q) -> p h q", q=PMX),
                        axis=AX.X, op=OP.max)
                else:
                    for ni, (n0, nn) in enumerate(NCH):
                        pb = ps_b.tile([128, 512], f32, tag="ps_s2")
                        nc.tensor.matmul(pb[:, :nn], m1c1, w2c1[:, n0:n0 + nn], start=True, stop=False)
                        nc.tensor.matmul(pb[:, :nn], m1c2, w2c2[:, n0:n0 + nn], start=False, stop=True)
                        h0, hn = HSL[ni]
                        nc.vector.tensor_reduce(
                            m2[:, h0:h0 + hn],
                            pb[:, :nn].rearrange("p (h q) -> p h q", q=PMX),
                            axis=AX.X, op=OP.max)
                st[i]["m1c1"] = m1c1
                st[i]["m1c2"] = m1c2
                st[i]["m2"] = m2

            def g3(i):
                b, lt = TILES[i]
                ntb = NTB[b]
                m2 = st[i]["m2"]
                pt3 = ps_tr.tile([128, 128], f16, tag="ps_tr")
                nc.tensor.transpose(pt3, m2[:, 0:128], ident)
                m2c1 = work.tile([128, 128], f16, tag="m2c1", bufs=4)
                nc.scalar.activation(m2c1, pt3, AF.Copy)
                pt4 = ps_tr.tile([128, 128], f16, tag="ps_tr")
                nc.tensor.transpose(pt4[:72], m2[:, 128:200], ident)
                m2c2 = work.tile([72, 128], f16, tag="m2c2", bufs=4)
                nc.scalar.activation(m2c2, pt4[:72], AF.Copy)
                if lt == 0:
                    strips[b] = ps_d.tile([128, 8 * NLT], f32, tag="ps_ser", name="s3strip")
                psl = strips[b][:, 8 * lt:8 * (lt + 1)]
                nc.tensor.matmul(psl, st[i]["m1c1"], w3c1, start=True, stop=False)
                nc.tensor.matmul(psl, st[i]["m1c2"], w3c2, start=False, stop=False)
                nc.tensor.matmul(psl, m2c1, w3c3, start=False, stop=False)
                nc.tensor.matmul(psl, m2c2, w3c4, start=False, stop=True)
                st[i].clear()
                if lt == ntb - 1:
                    Sb = work.tile([128, NLT], f32, tag="Sb")
                    nc.vector.tensor_reduce(Sb[:, :ntb],
                                            strips[b][:, :8 * ntb].rearrange("p (t q) -> p t q", q=PMX),
                                            axis=AX.X, op=OP.max)
                    nc.vector.tensor_tensor(Sb[:, :ntb], Sb[:, :ntb],
                                            penT_sb[:, b * NLT:b * NLT + ntb],
                                            OP.subtract)
                    ptb = ps_tr.tile([NLT, 128], f32, tag="ps_tr")
                    nc.tensor.transpose(ptb[:ntb], Sb[:, :ntb], ident32)
                    s4stg = work.tile([NLT, 128], f32, tag="s4stg")
                    nc.scalar.activation(s4stg[:ntb], ptb[:ntb], AF.Copy)
                    s4row = work.tile([1, L], f32, tag="s4row", bufs=2)
                    if ntb < NLT:
                        nc.vector.memset(s4row[:, 128 * ntb:], -BIG)
                    dma_engines[b % 3].dma_start(s4row[:, :128 * ntb], s4stg[:ntb])
                    s4rows[b] = s4row
            return g2, g3

        def run_pipeline(g1, g2, g3, fill, batch_cb):
            # batch b's callback fires two tiles after its strip completes
            # (so the S4-row DMA has landed); last batch fires immediately.
            NT = NT_ALL
            last_of = {}
            for j, (b, lt) in enumerate(TILES):
                last_of[b] = j
            cb_at = {}
            for b in range(BLOC - 1):
                cb_at[min(last_of[b] + 2, NT - 1)] = b
            cb_at[NT - 1] = BLOC - 1
            for i in range(NT + 2):
                if i < NT:
                    g1(i)
                if 1 <= i < NT + 1:
                    g2(i - 1)
                if 2 <= i:
                    j = i - 2
                    g3(j)
                    if batch_cb is not None and j in cb_at:
                        batch_cb(cb_at[j])
                if fill is not None:
                    next(fill, None)
            if fill is not None:
                for _ in fill:
                    pass

        def score_sweep(tag, s4rows, oh, ol, fill=None, batch_cb=None, a_out=None):
            """Full maxout scoring sweep; optionally spills stage-1 psum (fp16)."""
            w1m = W[f"w1m_{tag}"]
            c3pair = W[f"c3_{tag}"]
            st = [dict() for _ in range(NT_ALL)]
            strips = {}
            g2, g3 = make_g23(tag, s4rows, st, strips)

            def g1(i):
                b, lt = TILES[i]
                c3rhs = c3pair[b % 2]
                if lt == 0:
                    nc.sync.dma_start(c3rhs[16:17, :], oh[b:b + 1, :])
                    nc.sync.dma_start(c3rhs[17:18, :], ol[b:b + 1, :])
                lsl = slice(128 * lt, 128 * (lt + 1))
                m1 = m1_slots[i % 6]
                if a_out is not None:
                    a_w = work.tile([128, H * PMX], f16, tag="t_add", bufs=1)
                for ni, (n0, nn) in enumerate(NCH):
                    pa = ps_a.tile([128, 512], f32, tag="ps_s1")
                    for c in range(3):
                        nc.tensor.matmul(pa[:, :nn], E16[b][:, c, lsl], w1m[:, c, n0:n0 + nn],
                                         start=(c == 0), stop=False)
                    nc.tensor.matmul(pa[:, :nn], E16c3[b][:, lsl], c3rhs[:, n0:n0 + nn],
                                     start=False, stop=True)
                    h0, hn = HSL[ni]
                    nc.vector.tensor_reduce(
                        m1[:, h0:h0 + hn],
                        pa[:, :nn].rearrange("p (h q) -> p h q", q=PMX),
                        axis=AX.X, op=OP.max)
                    if a_out is not None:
                        nc.scalar.activation(a_w[:, n0:n0 + nn], pa[:, :nn], AF.Copy)
                if a_out is not None:
                    dma_engines[(b + lt) % 3].dma_start(a_out[b, lt], a_w)
                st[i]["m1"] = m1

            run_pipeline(g1, g2, g3, fill, batch_cb)

        def reuse_sweep(tag, s4rows, d16, a_in, batch_cb=None):
            """Scoring sweep: stage-1 = vector add of the DRAM A-tile and the
            per-batch delta-o broadcast, then an SBUF fp16 maxpool."""
            st = [dict() for _ in range(NT_ALL)]
            strips = {}
            a_tiles = {}
            dbc = {0: bcast_do(d16, 0), 1: bcast_do(d16, 1)}
            g2, g3 = make_g23(tag, s4rows, st, strips, s2_sbuf=True)

            def fetch(j):
                if j >= NT_ALL:
                    return
                b, lt = TILES[j]
                at = stage.tile([128, H * PMX], f16, tag="ar", bufs=2)
                dma_engines[j % 3].dma_start(at, a_in[b, lt])
                a_tiles[j] = at

            fetch(0)
            fetch(1)

            def g1(i):
                b, lt = TILES[i]
                fetch(i + 2)
                # broadcast the next batch's delta-o two tiles early
                if lt == NTB[b] - 2 and b + 2 < BLOC:
                    dbc[b + 2] = bcast_do(d16, b + 2)
                at = a_tiles.pop(i)
                t = work.tile([128, H * PMX], f16, tag="t_add", bufs=1)
                nc.vector.tensor_tensor(t, at, dbc[b], OP.add)
                m1 = m1_slots[i % 6]
                nc.vector.tensor_reduce(
                    m1, t.rearrange("p (h q) -> p h q", q=PMX),
                    axis=AX.X, op=OP.max)
                st[i]["m1"] = m1

            run_pipeline(g1, g2, g3, None, batch_cb)

        def lstm_update():
            """hx via LSTM cell with hx0=cx0=0 (f-gate and w_hh drop out)."""
            pt_i = ps_d.tile([128, 512], f32, tag="ps_ser")
            pt_go = ps_d.tile([128, 512], f32, tag="ps_ser")
            lhs_for = []
            for (k0, kn, wtile) in W["ih_chunks"]:
                if k0 < 400:
                    c = k0 // 128
                    lhsT = es16[:, c, :] if c < 3 else es16c3  # [17,4] w/ ones
                else:
                    c = (k0 - 400) // 128
                    lhsT = ee16[:, c, :] if c < 3 else ee16c3
                lhs_for.append((lhsT, wtile, kn + (1 if k0 == 384 else 0)))
            n = len(lhs_for)
            for i, (lhsT, wtile, rows) in enumerate(lhs_for):
                nc.tensor.matmul(pt_i[:BLOC, :H], lhsT, wtile[:rows, 0:H],
                                 start=(i == 0), stop=(i == n - 1))
            for i, (lhsT, wtile, rows) in enumerate(lhs_for):
                nc.tensor.matmul(pt_go[:BLOC, :2 * H], lhsT, wtile[:rows, 2 * H:4 * H],
                                 start=(i == 0), stop=(i == n - 1))
            ig = work.tile([BLOC, H], f32, tag="ig", bufs=1)
            nc.scalar.activation(ig, pt_i[:BLOC, :H], AF.Sigmoid)
            gg = work.tile([BLOC, H], f32, tag="gg", bufs=1)
            nc.scalar.activation(gg, pt_go[:BLOC, 0:H], AF.Tanh)
            og = work.tile([BLOC, H], f32, tag="og", bufs=1)
            nc.scalar.activation(og, pt_go[:BLOC, H:2 * H], AF.Sigmoid)
            cx = work.tile([BLOC, H], f32, tag="cx", bufs=1)
            nc.vector.tensor_tensor(cx, ig, gg, OP.mult)
            tcx = work.tile([BLOC, H], f32, tag="tcx", bufs=1)
            nc.scalar.activation(tcx, cx, AF.Tanh)
            h0 = work.tile([BLOC, H], f16, tag="h0", bufs=1)
            nc.vector.tensor_tensor(h0, og, tcx, OP.mult)
            ptr = ps_tr.tile([128, 128], f16, tag="ps_tr")
            nc.tensor.transpose(ptr[:, :BLOC], h0[:, 0:128], ident[:BLOC, :BLOC])
            nc.vector.tensor_copy(h0T1, ptr[:, :BLOC])
            ptr2 = ps_tr.tile([128, 128], f16, tag="ps_tr")
            nc.tensor.transpose(ptr2[:72, :BLOC], h0[:, 128:200], ident[:BLOC, :BLOC])
            nc.vector.tensor_copy(h0T2[:72], ptr2[:72, :BLOC])
            pt = ps_d.tile([128, 512], f32, tag="ps_ser")
            wmlpc1, wmlpc2 = W["wmlp"]
            nc.tensor.matmul(pt[:BLOC, :H], h0T1, wmlpc1, start=True, stop=False)
            nc.tensor.matmul(pt[:BLOC, :H], h0T2, wmlpc2, start=False, stop=True)
            hx_row = work.tile([BLOC, H], f16, tag="hx_row", bufs=1)
            nc.scalar.activation(hx_row, pt[:BLOC, :H], AF.Copy)
            ptr3 = ps_tr.tile([128, 128], f16, tag="ps_tr")
            nc.tensor.transpose(ptr3[:, :BLOC], hx_row[:, 0:128], ident[:BLOC, :BLOC])
            nc.vector.tensor_copy(hxT[0], ptr3[:, :BLOC])
            ptr4 = ps_tr.tile([128, 128], f16, tag="ps_tr")
            nc.tensor.transpose(ptr4[:72, :BLOC], hx_row[:, 128:200], ident[:BLOC, :BLOC])
            nc.vector.tensor_copy(hxT[1], ptr4[:72, :BLOC])

        # ---------------- prologue: s-critical-path loads ----------------
        import itertools
        for _ in load_wd("s", nc.sync, nc.vector):
            pass
        for _ in load_w1r("s", nc.sync, nc.vector):
            pass
        load_E(0, nc.gpsimd)
        for _ in load_w1m("s", nc.sync, nc.scalar):
            pass
        edefer = []
        load_E(1, nc.gpsimd, edefer)
        load_E(2, nc.gpsimd, edefer)
        load_E(3, nc.gpsimd, edefer)

        def edefer_gen():
            for fn in edefer:
                fn()
                yield
        fill_steps = itertools.chain(
            edefer_gen(),
            load_wd("e", nc.sync, nc.scalar),
            load_w1r("e", nc.sync, nc.scalar),
            load_w1m("e", nc.sync, nc.scalar),
            load_scoring2("e", nc.sync, nc.scalar, "stg"),
            load_lstm(nc.sync, nc.scalar))

        # ---------------- the four passes ----------------
        r_row = r_matmul("s", with_hx=False)
        oh, ol = o_rows("s", r_row, save=REUSE)
        for _ in load_scoring2("s", nc.sync, nc.scalar, "stg"):
            pass
        rows1 = {}
        score_sweep("s", rows1, oh, ol, fill=fill_steps,
                    batch_cb=lambda b: argmax_gather_b(rows1[b], b, es16, es16c3),
                    a_out=a1d if REUSE else None)

        r_row = r_matmul("e", with_hx=False)
        oh, ol = o_rows("e", r_row, save=REUSE)
        rows2 = {}
        score_sweep("e", rows2, oh, ol,
                    batch_cb=lambda b: argmax_gather_b(rows2[b], b, ee16, ee16c3),
                    a_out=a2d if REUSE else None)

        lstm_update()

        rows3 = {}
        rows4 = {}

        def cb3(b):
            argmax_gather_b(rows3[b], b, es16, es16c3)
            if b < BLOC - 1:
                lsm_row(rows3[b], b, lp1)

        def cb4(b):
            if b < BLOC - 1:
                lsm_row(rows4[b], b, lp2)

        r_row = r_matmul("s", with_hx=True)
        if REUSE:
            d16 = delta_o_rows("s", r_row)
            reuse_sweep("s", rows3, d16, a1d, batch_cb=cb3)
            r_row = r_matmul("e", with_hx=True)
            d16 = delta_o_rows("e", r_row)
            lsm_row(rows3[BLOC - 1], BLOC - 1, lp1)
            reuse_sweep("e", rows4, d16, a2d, batch_cb=cb4)
        else:
            oh, ol = o_rows("s", r_row)
            score_sweep("s", rows3, oh, ol, batch_cb=cb3)
            r_row = r_matmul("e", with_hx=True)
            oh, ol = o_rows("e", r_row)
            lsm_row(rows3[BLOC - 1], BLOC - 1, lp1)
            score_sweep("e", rows4, oh, ol, batch_cb=cb4)
        lsm_row(rows4[BLOC - 1], BLOC - 1, lp2)

        stage_cm.__exit__(None, None, None)

    nc.compile()
    return nc


def batch_order(lens):
    """order[b*NCORES + c] = original batch index at core c, slot b.
    Sorting by length groups similar lengths into each slot so the
    per-slot tile count (max over cores) stays tight."""
    return np.argsort(np.asarray(lens).astype(np.int64), kind="stable")


def get_program(inputs):
    lens = np.asarray(inputs["passage_lens"]).astype(np.int64)
    order = batch_order(lens)
    nt_b = tuple(int(min(NLT, (int(lens[order[b * NCORES + c]]) + 127) // 128))
                 for b in range(BLOC) for c in range(NCORES))
    if _cache.get("key") != nt_b:
        _cache["nc"] = _build_program(nt_b)
        _cache["key"] = nt_b
    return _cache["nc"]


def _split16(x):
    hi = np.asarray(x, np.float32).astype(np.float16)
    lo = (np.asarray(x, np.float32) - hi.astype(np.float32)).astype(np.float16)
    return hi, lo


def make_in_maps(inputs):
    """Per-core input maps: batch shard + trivial host prep (mask, bias splits)."""
    inputs = {k: np.asarray(v) for k, v in inputs.items()}
    enc = np.ascontiguousarray(inputs["encoding_matrix"], dtype=np.float32)
    lens = np.asarray(inputs["passage_lens"]).astype(np.int64)
    pen_full = np.where(np.arange(L)[None, :] < lens[:, None],
                        np.float32(0.0), BIG).astype(np.float32)

    shared = {}
    for tag in ("s", "e"):
        shared[f"w1_{tag}"] = np.ascontiguousarray(inputs[f"w1_{tag}"], np.float32)
        shared[f"b1_{tag}"] = np.ascontiguousarray(inputs[f"b1_{tag}"], np.float32).reshape(1, -1)
        shared[f"w2_{tag}"] = np.ascontiguousarray(inputs[f"w2_{tag}"], np.float32)
        b2h, b2l = _split16(inputs[f"b2_{tag}"])
        shared[f"b2h_{tag}"] = b2h.reshape(1, -1)
        shared[f"b2l_{tag}"] = b2l.reshape(1, -1)
        shared[f"w3_{tag}"] = np.ascontiguousarray(inputs[f"w3_{tag}"], np.float32)
        b3h, b3l = _split16(inputs[f"b3_{tag}"])
        shared[f"b3h_{tag}"] = b3h.reshape(1, -1)
        shared[f"b3l_{tag}"] = b3l.reshape(1, -1)
        shared[f"wd_{tag}"] = np.ascontiguousarray(inputs[f"wd_{tag}"], np.float32)
    shared["w_ih"] = np.ascontiguousarray(inputs["w_ih"], np.float32)
    shared["b_lstm"] = np.ascontiguousarray(inputs["b_lstm"], np.float32).reshape(1, -1)
    shared["w_mlp"] = np.ascontiguousarray(inputs["w_mlp"], np.float32)
    shared["b_mlp"] = np.ascontiguousarray(inputs["b_mlp"], np.float32).reshape(1, -1)

    order = batch_order(lens)
    in_maps = []
    for core in range(NCORES):
        idx = [int(order[b * NCORES + core]) for b in range(BLOC)]
        m = dict(shared)
        m["enc"] = np.ascontiguousarray(enc[idx])
        m["selmat_d"] = np.kron(np.eye(BLOC, dtype=np.float16),
                                np.ones((1, 128), dtype=np.float16))
        pc = pen_full[idx].reshape(BLOC, NLT, 128)
        m["penT"] = np.ascontiguousarray(pc.transpose(2, 0, 1).reshape(128, BLOC * NLT))
        in_maps.append(m)
    return in_maps


def run_on_hw(inputs, trace=False):
    from concourse import bass_utils
    nc = get_program(inputs)
    in_maps = make_in_maps(inputs)
    res = bass_utils.run_bass_kernel_spmd(nc, in_maps, core_ids=list(range(NCORES)),
                                          trace=trace)
    order = batch_order(inputs["passage_lens"])
    lp1 = np.empty((B, L), np.float32)
    lp2 = np.empty((B, L), np.float32)
    for c in range(NCORES):
        for b in range(BLOC):
            ob = int(order[b * NCORES + c])
            lp1[ob] = np.asarray(res.results[c]["lp1"])[b]
            lp2[ob] = np.asarray(res.results[c]["lp2"])[b]
    return (lp1, lp2), res


def kernel(**inputs):
    out, _ = run_on_hw(inputs, trace=False)
    return out
